# revision 39
# baseline (speedup 1.0000x reference)
"""Trainium2 Bass kernel for nn_AGNN_EFG (GCN -> TopK pool -> GATv2 -> TopK pool -> head).

Self-contained: shards the B=64 graphs across 8 NeuronCores (8 graphs/core),
runs one SPMD Bass program, gathers the [64, 1] head output on host.

v4 design:
- Edges (incl self loops) sorted per graph by dst block (db = dst>>7), each
  db run padded to a fixed spd chunks of 128 -> static chunk->db schedule
  (SPMD-safe; spd chosen on host from the data).
- Scatter-adds are ONE-HOT MATMULS on PE: per chunk a [128e,128d] bf16
  one-hot (DVE is_equal; pad slots carry sentinel 255 -> zero rows)
  accumulates messages into per-graph PSUM. No dma_scatter_add.
- GAT's xr[dst] is a [128d,128e]-orientation one-hot matmul (one-hot built
  on the Scalar engine as relu(1-(dlo-d)^2)) reading xr blocks from SBUF.
- Only 2 indirect passes remain (GpSimd Q7 descriptor generation is the
  machine bottleneck): gather u[src] (GCN) and [xl|xr][src] (GAT).
- Fully per-graph pipelined: utab(g+2) build, dense/pool/gtab(g),
  gcn(g+1) and gat(g) interleave so the GpSimd gather stream never idles.
- GCN u rows split [u_hi|u_lo] bf16 (~f32 accuracy, one 128-wide matmul).
- score1 fused into the lin1 matmul via host-folded [W_lin1 | W_lin1@p1n].
- Degrees are host-precomputed index data (bincount of dst); rsqrt on device.
"""

import sys

sys.path.insert(0, "/opt/trn_rl_repo")

from dataclasses import dataclass

import numpy as np
import ml_dtypes

import concourse.bass as bass
import concourse.mybir as mybir
import concourse.tile as tile
from concourse import bacc
from concourse.bass_utils import run_bass_kernel_spmd
from concourse.masks import make_identity

P = 128
F32 = mybir.dt.float32
BF16 = mybir.dt.bfloat16
I16 = mybir.dt.int16
U8 = mybir.dt.uint8
AF = mybir.ActivationFunctionType
OP = mybir.AluOpType
AX = mybir.AxisListType


@dataclass
class Cfg:
    ng: int = 8          # graphs per core
    npg: int = 2048      # nodes per graph
    hid: int = 64        # feature dim
    eg: int = 32768      # edges per graph (original, without self loops)
    spd: int = 19        # chunks (of 128 slots) per dst-block run; set at runtime
    ch: int = 2048       # gather window (slots per dma_gather call)
    n_bisect: int = 30   # bisection iterations for topk threshold
    psign: float = -1.0  # -sign(sum(att)): poison sign so poisoned e < 0
    pb_mag: float = 200.0  # poison magnitude; set so poisoned e ~ -40 (LUT-safe)

    @property
    def nn(self):
        return self.ng * self.npg

    @property
    def ne(self):
        return self.ng * self.eg

    @property
    def tj(self):
        return self.npg // P  # dst blocks per graph (16)

    @property
    def nt(self):
        return self.ng * self.tj  # 128

    @property
    def tch(self):
        return self.tj * self.spd  # chunks per graph

    @property
    def slots_g(self):
        return self.tch * P  # padded edge slots per graph

    @property
    def slots(self):
        return self.ng * self.slots_g

    @property
    def wpg(self):
        assert self.slots_g % self.ch == 0
        return self.slots_g // self.ch  # gather windows per graph

    @property
    def cpw(self):
        return self.ch // P  # chunks per window

    @property
    def k1(self):
        return self.npg // 2

    @property
    def k2(self):
        return self.npg // 4


def build_core_program(ctx, tc, cfg: Cfg):
    nc = tc.nc
    NG, NPG, HID, NN = cfg.ng, cfg.npg, cfg.hid, cfg.nn
    NT, TJ, SPD = cfg.nt, cfg.tj, cfg.spd
    TCH, WPG, CPW, CH = cfg.tch, cfg.wpg, cfg.cpw, cfg.ch
    SRUN = SPD * P  # slots per db run
    assert HID == 64 and CPW % 8 == 0

    # ---- I/O ----
    xT = nc.dram_tensor("xT", [HID, NN], F32, kind="ExternalInput").ap()
    srcw = nc.dram_tensor("srcw", [P, cfg.slots // 16], I16, kind="ExternalInput").ap()
    dlo_pm = nc.dram_tensor("dlo_pm", [P, NG * TCH], BF16, kind="ExternalInput").ap()
    dlo_fr = nc.dram_tensor("dlo_fr", [NG * TJ * SRUN], BF16, kind="ExternalInput").ap()
    degT = nc.dram_tensor("degT", [P, NT], F32, kind="ExternalInput").ap()
    w_names = ["Wl", "Wr"]
    Wd = {n: nc.dram_tensor(n, [HID, HID], F32, kind="ExternalInput").ap() for n in w_names}
    W1d = nc.dram_tensor("W1dup", [HID, P], F32, kind="ExternalInput").ap()
    W1p = nc.dram_tensor("Wlin1p", [HID, HID + 1], F32, kind="ExternalInput").ap()
    dinvF = nc.dram_tensor("dinvF", [NN], F32, kind="ExternalInput").ap()
    b1c = nc.dram_tensor("b_lin1c", [HID + 1], F32, kind="ExternalInput").ap()
    v_names = ["bn_a", "bn_bp", "att", "b_gat", "p2", "W23"]
    Vd = {n: nc.dram_tensor(n, [HID], F32, kind="ExternalInput").ap() for n in v_names}
    Cd = nc.dram_tensor("Cc", [1], F32, kind="ExternalInput").ap()
    out_d = nc.dram_tensor("out", [NG, 1], F32, kind="ExternalOutput").ap()

    # ---- DRAM scratch (per-graph tables so gathers only depend on their
    # own graph's writes) ----
    utab = [nc.dram_tensor(f"utab{g}", [NPG, P], BF16).ap() for g in range(NG)]
    gtab = [nc.dram_tensor(f"gtab{g}", [NPG, P], BF16).ap() for g in range(NG)]
    xlr_dram = nc.dram_tensor("xlr_dram", [P, NN], BF16).ap()
    ufm_dram = nc.dram_tensor("ufm_dram", [P, NN], BF16).ap()
    sc_dram = nc.dram_tensor("sc_dram", [NN], F32).ap()

    cpool = ctx.enter_context(tc.tile_pool(name="consts", bufs=1))
    mpool = ctx.enter_context(tc.tile_pool(name="main", bufs=1))
    smallps = ctx.enter_context(tc.tile_pool(name="smallps", bufs=1, space="PSUM"))

    # ---- constants ----
    ident = cpool.tile([P, P], F32)
    make_identity(nc, ident[:])
    ones128 = cpool.tile([P, P], F32)
    nc.vector.memset(ones128[:], 1.0)
    nantile = cpool.tile([P, NT], F32)
    nc.vector.memset(nantile[:], float("nan"))
    negbig = cpool.tile([P, NT], F32)
    nc.vector.memset(negbig[:], -1e9)
    io16 = cpool.tile([P, P], I16)
    nc.gpsimd.iota(io16[:], pattern=[[1, P]], base=0, channel_multiplier=0)
    iota_bf = cpool.tile([P, P], BF16)
    nc.vector.tensor_copy(out=iota_bf[:], in_=io16[:])
    ioc16 = cpool.tile([P, 1], I16)
    nc.gpsimd.iota(ioc16[:], pattern=[[0, 1]], base=0, channel_multiplier=1)
    niota_col = cpool.tile([P, 1], F32)
    nc.vector.tensor_scalar(
        out=niota_col[:], in0=ioc16[:], scalar1=-1.0, scalar2=None, op0=OP.mult
    )

    Ws = {}
    for n in w_names:
        t = cpool.tile([HID, HID], F32, tag=f"w_{n}")
        nc.sync.dma_start(out=t[:], in_=Wd[n][:])
        Ws[n] = t
    W1p_sb = cpool.tile([HID, HID + 1], F32, tag="w_Wlin1p")
    nc.sync.dma_start(out=W1p_sb[:], in_=W1p[:])
    W1d_sb = cpool.tile([HID, P], F32, tag="w_W1dup")
    nc.sync.dma_start(out=W1d_sb[:], in_=W1d[:])
    b1c_sb = cpool.tile([HID + 1, 1], F32, tag="v_b1c")
    nc.sync.dma_start(out=b1c_sb[:], in_=b1c[:, None])
    Vs = {}
    for n in v_names:
        t = cpool.tile([HID, 1], F32, tag=f"v_{n}")
        nc.sync.dma_start(out=t[:], in_=Vd[n][:, None])
        Vs[n] = t
    att_rep = cpool.tile([P, HID], BF16)
    nc.gpsimd.dma_start(out=att_rep[:], in_=Vd["att"][None, :].to_broadcast([P, HID]))
    p2_rep = cpool.tile([P, HID], F32)
    nc.sync.dma_start(out=p2_rep[:], in_=Vd["p2"][None, :].to_broadcast([P, HID]))
    bgat_rep = cpool.tile([P, HID], F32)
    nc.sync.dma_start(out=bgat_rep[:], in_=Vd["b_gat"][None, :].to_broadcast([P, HID]))
    Cc_sb = cpool.tile([NG, 1], F32)
    nc.sync.dma_start(out=Cc_sb[:], in_=Cd[None, :].to_broadcast([NG, 1]))

    # whole per-chunk dst-low-bit table (for one-hot builds in [e,d] orientation)
    dlo_sb = cpool.tile([P, NG * TCH], BF16)
    nc.sync.dma_start(out=dlo_sb[:], in_=dlo_pm[:])

    # ---- dinv from host degree counts ----
    dinv_t = mpool.tile([P, NT], F32, tag="dinv_t")
    sqd_t = mpool.tile([P, NT], F32, tag="sqd_t")
    ntmp = mpool.tile([P, NT], F32, tag="ntmp")

    def recip_newton(r_ap, x_ap, tmp_ap):
        nc.vector.tensor_tensor(out=tmp_ap, in0=x_ap, in1=r_ap, op=OP.mult)
        nc.vector.tensor_scalar(
            out=tmp_ap, in0=tmp_ap, scalar1=-1.0, scalar2=2.0, op0=OP.mult, op1=OP.add
        )
        nc.vector.tensor_tensor(out=r_ap, in0=r_ap, in1=tmp_ap, op=OP.mult)

    deg_sb = mpool.tile([P, NT], F32, tag="deg_sb")
    nc.sync.dma_start(out=deg_sb[:], in_=degT[:])
    nc.scalar.sqrt(out=sqd_t[:], in_=deg_sb[:])
    nc.vector.reciprocal(out=dinv_t[:], in_=sqd_t[:])
    recip_newton(dinv_t[:], sqd_t[:], ntmp[:])

    def idx_slice(pool, w_global, tag):
        t = pool.tile([P, CH // 16], I16, tag=tag)
        c0 = w_global * (CH // 16)
        nc.sync.dma_start(out=t[:], in_=srcw[:, c0 : c0 + CH // 16])
        return t

    # ---- pools (all phases interleave; PSUM budget: 2+1+2+2+1 = 8 banks) ----
    ups = ctx.enter_context(tc.tile_pool(name="ups", bufs=2))
    ubc = ctx.enter_context(tc.tile_pool(name="ubc", bufs=1))
    gep = ctx.enter_context(tc.tile_pool(name="gep", bufs=3))
    ohp = ctx.enter_context(tc.tile_pool(name="ohp", bufs=2))
    hps_pool = ctx.enter_context(tc.tile_pool(name="hpsp", bufs=1, space="PSUM"))
    dsa = ctx.enter_context(tc.tile_pool(name="dsa", bufs=1))
    dss = ctx.enter_context(tc.tile_pool(name="dss", bufs=2))
    bis = ctx.enter_context(tc.tile_pool(name="bis", bufs=2))
    gap = ctx.enter_context(tc.tile_pool(name="gap", bufs=2))
    aohp = ctx.enter_context(tc.tile_pool(name="aohp", bufs=2))
    runtmp = ctx.enter_context(tc.tile_pool(name="runtmp", bufs=1))
    runp = ctx.enter_context(tc.tile_pool(name="runp", bufs=2))
    gatps = ctx.enter_context(tc.tile_pool(name="gatps", bufs=1, space="PSUM"))
    mxrp = ctx.enter_context(tc.tile_pool(name="mxrp", bufs=2, space="PSUM"))
    gfin = ctx.enter_context(tc.tile_pool(name="gfin", bufs=1))

    # persistent t-space tiles
    h2t = mpool.tile([P, NT, HID], F32, tag="bigA")
    gstage = mpool.tile([P, NT, P], BF16, tag="bigC")
    score1_t = mpool.tile([P, NT], F32, tag="score1_t")
    tanh1 = mpool.tile([P, NT], F32, tag="tanh1")
    gate1 = mpool.tile([P, NT], F32, tag="gate1")
    gate1z = mpool.tile([P, NT], F32, tag="gate1z")
    padd = mpool.tile([P, NT], F32, tag="padd")
    kept1 = mpool.tile([P, NT], U8, tag="kept1")
    pb = cfg.psign * cfg.pb_mag

    hps_tiles = {}
    nmps_tiles = {}

    # ======== per-graph sections ========

    def utab_build(g):
        # xw (duplicated rows) -> dinv scale -> [hi|lo] bf16 split, all in
        # feature-major; node-major utab rows via one transpose DMA hop.
        xw2 = ups.tile([P, NPG], F32, tag="xw2", name=f"xw2_{g}")
        for jl in range(TJ):
            xTg = ups.tile([HID, P], F32, tag="xTg", name=f"xTg{g}_{jl}")
            nc.sync.dma_start(
                out=xTg[:], in_=xT[:, g * NPG + jl * P : g * NPG + (jl + 1) * P]
            )
            pm = smallps.tile([P, P], F32, tag="smA", name=f"xwps{g}_{jl}")
            nc.tensor.matmul(pm[:], lhsT=W1d_sb[:], rhs=xTg[:], start=True, stop=True)
            nc.scalar.copy(out=xw2[:, jl * P : (jl + 1) * P], in_=pm[:])
        dbc = ubc.tile([P, NPG], F32, tag="dinvbc", name=f"dinvbc{g}")
        nc.sync.dma_start(
            out=dbc[:], in_=dinvF[g * NPG : (g + 1) * NPG][None, :].to_broadcast([P, NPG])
        )
        nc.vector.tensor_tensor(out=xw2[:], in0=xw2[:], in1=dbc[:], op=OP.mult)
        u2 = ups.tile([P, NPG], BF16, tag="u2", name=f"u2_{g}")
        nc.vector.tensor_copy(out=u2[0:HID, :], in_=xw2[0:HID, :])
        nc.vector.tensor_copy(out=u2[HID:P, :], in_=xw2[HID:P, :])
        nc.vector.tensor_tensor(
            out=u2[HID:P, :], in0=xw2[HID:P, :], in1=u2[HID:P, :], op=OP.subtract
        )
        nc.sync.dma_start(out=ufm_dram[:, g * NPG : (g + 1) * NPG], in_=u2[:])
        u_nm = ups.tile([P, TJ, P], BF16, tag="u2", name=f"unm{g}")
        nc.sync.dma_start_transpose(
            out=u_nm[:], in_=ufm_dram[:, g * NPG : (g + 1) * NPG]
        )
        nc.sync.dma_start(
            out=utab[g].rearrange("(j p) f -> p j f", p=P), in_=u_nm[:]
        )

    def gcn_pass(g):
        hps = [
            hps_pool.tile([P, 8, HID], F32, tag=f"hps{t}", name=f"hps{t}_{g}")
            for t in range(2)
        ]
        hps_tiles[g] = hps
        for w in range(WPG):
            wg = g * WPG + w
            ssl = idx_slice(gep, wg, "ssl")
            ub = gep.tile([P, CPW, P], BF16, tag="ub")
            nc.gpsimd.dma_gather(
                out_ap=ub[:], in_ap=utab[g][:], idxs_ap=ssl[:],
                num_idxs=CH, num_idxs_reg=CH, elem_size=P, queue_num=wg % 4,
                single_packet=False,
            )
            for b in range(CPW // 8):
                oh8 = ohp.tile([P, 8, P], BF16, tag="oh8")
                c0 = g * TCH + w * CPW + b * 8
                nc.vector.tensor_tensor(
                    out=oh8[:],
                    in0=iota_bf[:, None, :].to_broadcast([P, 8, P]),
                    in1=dlo_sb[:, c0 : c0 + 8, None].to_broadcast([P, 8, P]),
                    op=OP.is_equal,
                )
                for cl in range(8):
                    c = b * 8 + cl
                    gc = w * CPW + c
                    db, pos = gc // SPD, gc % SPD
                    out_slc = hps[db // 8][:, db % 8, :]
                    nc.tensor.matmul(
                        out_slc, lhsT=oh8[:, cl, :], rhs=ub[:, c, 0:HID],
                        start=(pos == 0), stop=False,
                    )
                    nc.tensor.matmul(
                        out_slc, lhsT=oh8[:, cl, :], rhs=ub[:, c, HID:P],
                        start=False, stop=(pos == SPD - 1),
                    )

    S6C = 512

    def dense_g(g):
        # h = dinv*(hi+lo); BN+leaky (one ACT Lrelu); lin1(+score); xl/xr
        hps = hps_tiles.pop(g)
        gsl = slice(g * TJ, (g + 1) * TJ)
        hsum = dsa.tile([P, TJ, HID], F32, tag="hsum", name=f"hsum{g}")
        for t in range(2):
            nc.vector.tensor_tensor(
                out=hsum[:, t * 8 : (t + 1) * 8, :], in0=hps[t][:],
                in1=dinv_t[:, g * TJ + t * 8 : g * TJ + (t + 1) * 8, None].to_broadcast(
                    [P, 8, HID]
                ),
                op=OP.mult,
            )
        hfm = dsa.tile([HID, NPG], F32, tag="hfm", name=f"hfm{g}")
        for jl in range(TJ):
            pt = smallps.tile([HID, P], F32, tag="smA", name=f"htr{g}_{jl}")
            nc.tensor.transpose(out=pt[:], in_=hsum[:, jl, :], identity=ident[:])
            nc.scalar.copy(out=hfm[:, jl * P : (jl + 1) * P], in_=pt[:])
        nc.scalar.activation(
            out=hfm[:], in_=hfm[:], func=AF.Lrelu, scale=Vs["bn_a"][:],
            bias=Vs["bn_bp"][:], alpha=0.01,
        )
        xlr_g = dss.tile([P, NPG], BF16, tag="xlrg", name=f"xlrg{g}")
        for ol in range(0, NPG, S6C):
            o = g * NPG + ol
            pm = hps_pool.tile([HID + 1, S6C], F32, tag="hps0", name=f"l1ps{o}")
            nc.tensor.matmul(
                pm[:], lhsT=W1p_sb[:], rhs=hfm[:, ol : ol + S6C],
                start=True, stop=True,
            )
            hc = dss.tile([HID + 1, S6C], F32, tag="hc", name=f"hc{o}")
            nc.scalar.activation(
                out=hc[:], in_=pm[:], func=AF.Identity, bias=b1c_sb[:]
            )
            nc.sync.dma_start(out=sc_dram[None, o : o + S6C], in_=hc[HID : HID + 1, :])
            px = hps_pool.tile([P, S6C], F32, tag="hps1", name=f"xlrps{o}")
            nc.tensor.matmul(px[:HID, :], lhsT=Ws["Wl"][:], rhs=hc[0:HID, :], start=True, stop=True)
            nc.tensor.matmul(px[HID:, :], lhsT=Ws["Wr"][:], rhs=hc[0:HID, :], start=True, stop=True)
            nc.scalar.copy(out=xlr_g[:, ol : ol + S6C], in_=px[:])
        nc.sync.dma_start(out=xlr_dram[:, g * NPG : (g + 1) * NPG], in_=xlr_g[:])

    def bisect_multi(score_slc, ngr, target, tag):
        # score_slc: [P, ngr*TJ]; returns per-graph thresholds lo [P, ngr]
        lo = bis.tile([P, ngr], F32, tag="lo", name=f"lo_{tag}")
        hi = bis.tile([P, ngr], F32, tag="hi", name=f"hi_{tag}")
        mid = bis.tile([P, ngr], F32, tag="mid", name=f"mid_{tag}")
        cmp = bis.tile([P, ngr * TJ], F32, tag="cmp", name=f"cmp_{tag}")
        cred = bis.tile([P, ngr], F32, tag="cred", name=f"cred_{tag}")
        ge = bis.tile([P, ngr], U8, tag="ge", name=f"ge_{tag}")
        lt = bis.tile([P, ngr], U8, tag="lt", name=f"lt_{tag}")
        nc.vector.memset(lo[:], -64.0)
        nc.vector.memset(hi[:], 64.0)
        sc_g = score_slc.rearrange("p (g t) -> p g t", g=ngr)
        cmp_g = cmp[:].rearrange("p (g t) -> p g t", g=ngr)
        for it in range(cfg.n_bisect):
            nc.vector.tensor_tensor(out=mid[:], in0=lo[:], in1=hi[:], op=OP.add)
            nc.vector.tensor_scalar(
                out=mid[:], in0=mid[:], scalar1=0.5, scalar2=None, op0=OP.mult
            )
            nc.vector.tensor_tensor(
                out=cmp_g, in0=sc_g,
                in1=mid[:, :, None].to_broadcast([P, ngr, TJ]), op=OP.is_gt,
            )
            nc.vector.tensor_reduce(out=cred[:], in_=cmp_g, axis=AX.X, op=OP.add)
            cps = smallps.tile([P, ngr], F32, tag="smB", name=f"cnt_{tag}_{it}")
            nc.tensor.matmul(cps[:], lhsT=ones128[:], rhs=cred[:], start=True, stop=True)
            nc.vector.tensor_scalar(
                out=ge[:], in0=cps[:], scalar1=float(target), scalar2=None, op0=OP.is_ge
            )
            nc.vector.tensor_scalar(
                out=lt[:], in0=cps[:], scalar1=float(target), scalar2=None, op0=OP.is_lt
            )
            nc.vector.copy_predicated(out=lo[:], mask=ge[:], data=mid[:])
            nc.vector.copy_predicated(out=hi[:], mask=lt[:], data=mid[:])
        return lo

    NH = NG // 2  # graphs per pooling half

    def pool1_h(h):
        g0 = h * NH
        gsl = slice(g0 * TJ, (g0 + NH) * TJ)
        nc.sync.dma_start(
            out=score1_t[:, gsl],
            in_=sc_dram[g0 * NPG : (g0 + NH) * NPG].rearrange("(j p) -> p j", p=P),
        )
        t1 = bisect_multi(score1_t[:, gsl], NH, cfg.k1, f"p1h{h}")
        nc.vector.tensor_tensor(
            out=kept1[:, gsl].rearrange("p (g t) -> p g t", g=NH),
            in0=score1_t[:, gsl].rearrange("p (g t) -> p g t", g=NH),
            in1=t1[:, :, None].to_broadcast([P, NH, TJ]), op=OP.is_gt,
        )
        nc.scalar.activation(out=tanh1[:, gsl], in_=score1_t[:, gsl], func=AF.Tanh)
        nc.vector.tensor_copy(out=gate1[:, gsl], in_=nantile[:, gsl])
        nc.vector.copy_predicated(out=gate1[:, gsl], mask=kept1[:, gsl], data=tanh1[:, gsl])
        nc.vector.memset(gate1z[:, gsl], 0.0)
        nc.vector.copy_predicated(out=gate1z[:, gsl], mask=kept1[:, gsl], data=tanh1[:, gsl])
        nc.vector.tensor_scalar(
            out=padd[:, gsl], in0=kept1[:, gsl], scalar1=-pb, scalar2=pb,
            op0=OP.mult, op1=OP.add,
        )
        # gtab for the half
        gs = gstage[:, gsl, :]
        nc.sync.dma_start_transpose(
            out=gs, in_=xlr_dram[:, g0 * NPG : (g0 + NH) * NPG]
        )
        nc.vector.tensor_tensor(
            out=gs, in0=gs,
            in1=gate1z[:, gsl, None].to_broadcast([P, NH * TJ, P]), op=OP.mult,
        )
        nc.vector.tensor_tensor(
            out=gs, in0=gs,
            in1=padd[:, gsl, None].to_broadcast([P, NH * TJ, P]), op=OP.add,
        )
        for gg in range(g0, g0 + NH):
            nc.sync.dma_start(
                out=gtab[gg].rearrange("(j p) f -> p j f", p=P),
                in_=gstage[:, gg * TJ : (gg + 1) * TJ, :],
            )

    def gat_pass(g):
        nmps = [
            gatps.tile([P, 8, HID], F32, tag=f"nmps{t}", name=f"nmps{t}_{g}")
            for t in range(2)
        ]
        dnps = smallps.tile([P, TJ], F32, tag="smB", name=f"dnps{g}")
        oh_de = {}
        for w in range(WPG):
            wg = g * WPG + w
            ssl = idx_slice(gap, wg, "assl")
            gx = gap.tile([P, CPW, P], BF16, tag="gx")
            nc.gpsimd.dma_gather(
                out_ap=gx[:], in_ap=gtab[g][:], idxs_ap=ssl[:],
                num_idxs=CH, num_idxs_reg=CH, elem_size=P, queue_num=wg % 4,
                single_packet=False,
            )
            for b in range(CPW // 8):
                oh8 = aohp.tile([P, 8, P], BF16, tag="aoh8")
                c0 = g * TCH + w * CPW + b * 8
                nc.vector.tensor_tensor(
                    out=oh8[:],
                    in0=iota_bf[:, None, :].to_broadcast([P, 8, P]),
                    in1=dlo_sb[:, c0 : c0 + 8, None].to_broadcast([P, 8, P]),
                    op=OP.is_equal,
                )
                mxr = mxrp.tile([P, 8, HID], F32, tag="mxr", name=f"mxr{wg}_{b}")
                for cl in range(8):
                    c = b * 8 + cl
                    gc = w * CPW + c
                    r, pos = gc // SPD, gc % SPD
                    if pos == 0:
                        dlo_bc = runtmp.tile(
                            [P, SRUN], BF16, tag="dlobc", name=f"dlobc{g}_{r}"
                        )
                        o = (g * TJ + r) * SRUN
                        nc.sync.dma_start(
                            out=dlo_bc[:],
                            in_=dlo_fr[o : o + SRUN][None, :].to_broadcast([P, SRUN]),
                        )
                        # one-hot on the Scalar engine: relu(1 - (dlo - d)^2)
                        ohsq = runtmp.tile([P, SRUN], BF16, tag="ohsq", name=f"ohsq{g}_{r}")
                        nc.scalar.activation(
                            out=ohsq[:], in_=dlo_bc[:], func=AF.Square,
                            bias=niota_col[:],
                        )
                        ohr = runp.tile([P, SRUN], BF16, tag="ohde", name=f"ohde{g}_{r}")
                        nc.scalar.activation(
                            out=ohr[:], in_=ohsq[:], func=AF.Relu,
                            bias=1.0, scale=-1.0,
                        )
                        oh_de[r] = ohr
                    nc.tensor.matmul(
                        mxr[:, cl, :],
                        lhsT=oh_de[r][:, pos * P : (pos + 1) * P],
                        rhs=gstage[:, g * TJ + r, HID:P],
                        start=True, stop=True,
                    )
                # e = att . leaky(xl_s + xr_d); w = exp(e); pay = w*xl
                gxs = gx[:, b * 8 : (b + 1) * 8, :]
                z = gap.tile([P, 8, HID], BF16, tag="z")
                nc.vector.tensor_tensor(
                    out=z[:], in0=gxs[:, :, 0:HID], in1=mxr[:], op=OP.add
                )
                nc.vector.scalar_tensor_tensor(
                    out=z[:], in0=z[:], scalar=0.2, in1=z[:], op0=OP.mult, op1=OP.max,
                )
                nc.vector.tensor_tensor(
                    out=z[:], in0=z[:],
                    in1=att_rep[:, None, :].to_broadcast([P, 8, HID]), op=OP.mult,
                )
                e8 = gap.tile([P, 8], F32, tag="e8")
                nc.vector.tensor_reduce(out=e8[:], in_=z[:], axis=AX.X, op=OP.add)
                w8b = gap.tile([P, 8], BF16, tag="w8b")
                nc.scalar.activation(out=w8b[:], in_=e8[:], func=AF.Exp)
                pay = gap.tile([P, 8, HID], BF16, tag="pay")
                nc.vector.tensor_tensor(
                    out=pay[:], in0=gxs[:, :, 0:HID],
                    in1=w8b[:, :, None].to_broadcast([P, 8, HID]), op=OP.mult,
                )
                for cl in range(8):
                    c = b * 8 + cl
                    gc = w * CPW + c
                    db, pos = gc // SPD, gc % SPD
                    nc.tensor.matmul(
                        nmps[db // 8][:, db % 8, :],
                        lhsT=oh8[:, cl, :], rhs=pay[:, cl, :],
                        start=(pos == 0), stop=(pos == SPD - 1),
                    )
                    nc.tensor.matmul(
                        dnps[:, db : db + 1],
                        lhsT=oh8[:, cl, :], rhs=w8b[:, cl : cl + 1],
                        start=(pos == 0), stop=(pos == SPD - 1),
                    )

        # ---- finalize graph g: h2 = leaky(numer/denom + b_gat) ----
        numsb = dsa.tile([P, TJ, HID], F32, tag="hsum", name=f"numsb{g}")
        nc.scalar.copy(out=numsb[:, 0:8, :], in_=nmps[0][:])
        nc.scalar.copy(out=numsb[:, 8:TJ, :], in_=nmps[1][:])
        den = gfin.tile([P, TJ], F32, tag="den", name=f"den{g}")
        rec = gfin.tile([P, TJ], F32, tag="rec", name=f"rec{g}")
        dtmp = gfin.tile([P, TJ], F32, tag="dtmp", name=f"dtmp{g}")
        nc.vector.tensor_scalar(
            out=den[:], in0=dnps[:], scalar1=1e-16, scalar2=None, op0=OP.add
        )
        nc.vector.reciprocal(out=rec[:], in_=den[:])
        recip_newton(rec[:], den[:], dtmp[:])
        hslc = h2t[:, g * TJ : (g + 1) * TJ, :]
        nc.vector.tensor_tensor(
            out=hslc, in0=numsb[:],
            in1=rec[:, :, None].to_broadcast([P, TJ, HID]), op=OP.mult,
        )
        nc.vector.tensor_tensor(
            out=hslc, in0=hslc,
            in1=bgat_rep[:, None, :].to_broadcast([P, TJ, HID]), op=OP.add,
        )
        nc.vector.scalar_tensor_tensor(
            out=hslc, in0=hslc, scalar=0.01, in1=hslc, op0=OP.mult, op1=OP.max
        )

    # ======== pipelined emission (two pooling halves) ========
    utab_build(0)
    utab_build(1)
    for g in range(NH):
        gcn_pass(g)
        dense_g(g)
        if g + 2 < NG:
            utab_build(g + 2)
    gcn_pass(NH)
    pool1_h(0)
    dense_g(NH)
    for g in range(NH):
        gat_pass(g)
        if g + NH + 1 < NG:
            gcn_pass(g + NH + 1)
            dense_g(g + NH + 1)
        if g + NH + 2 < NG:
            utab_build(g + NH + 2)
    pool1_h(1)
    for g in range(NH, NG):
        gat_pass(g)

    # ======== score2 (t-space, blocked), mask to kept1 ========
    score2_t = mpool.tile([P, NT], F32, tag="score2_t")
    for t in range(4):
        tsl = slice(t * 32, (t + 1) * 32)
        blk = ups.tile([P, 32, HID], F32, tag="xTg", name=f"s2blk{t}")
        nc.vector.tensor_tensor(
            out=blk[:], in0=h2t[:, tsl, :],
            in1=p2_rep[:, None, :].to_broadcast([P, 32, HID]), op=OP.mult,
        )
        nc.vector.tensor_reduce(out=score2_t[:, tsl], in_=blk[:], axis=AX.X, op=OP.add)
    kept1_t = mpool.tile([P, NT], U8, tag="kept1_t")
    nc.vector.tensor_tensor(out=kept1_t[:], in0=gate1[:], in1=gate1[:], op=OP.is_equal)
    sc2m = mpool.tile([P, NT], F32, tag="sc2m")
    nc.vector.tensor_copy(out=sc2m[:], in_=negbig[:])
    nc.vector.copy_predicated(out=sc2m[:], mask=kept1_t[:], data=score2_t[:])

    # ======== pool2 threshold + gate2 = tanh * mask ========
    gate2 = mpool.tile([P, NT], F32, tag="gate2")
    t2 = bisect_multi(sc2m[:], NG, cfg.k2, "p2")
    nc.vector.tensor_tensor(
        out=gate2[:].rearrange("p (g t) -> p g t", g=NG),
        in0=sc2m[:].rearrange("p (g t) -> p g t", g=NG),
        in1=t2[:, :, None].to_broadcast([P, NG, TJ]), op=OP.is_gt,
    )
    tanh2 = mpool.tile([P, NT], F32, tag="tanh2")
    sc2c = mpool.tile([P, NT], F32, tag="sc2c")
    nc.vector.tensor_scalar(
        out=sc2c[:], in0=sc2m[:], scalar1=-64.0, scalar2=None, op0=OP.max
    )
    nc.scalar.activation(out=tanh2[:], in_=sc2c[:], func=AF.Tanh)
    nc.vector.tensor_tensor(out=gate2[:], in0=gate2[:], in1=tanh2[:], op=OP.mult)

    # ======== T_g = sum_n gate2[n] * h2[n]; out = T @ W23 + C ========
    Tps = smallps.tile([P, NG], F32, tag="smB")
    for j in range(NT):
        g = j // TJ
        nc.tensor.matmul(
            Tps[:HID, g : g + 1], lhsT=h2t[:, j, :], rhs=gate2[:, j : j + 1],
            start=(j % TJ == 0), stop=(j % TJ == TJ - 1),
        )
    Tsb = mpool.tile([HID, NG], F32, tag="Tsb")
    nc.scalar.copy(out=Tsb[:], in_=Tps[:HID, :])
    hps2 = smallps.tile([NG, 1], F32, tag="smB")
    nc.tensor.matmul(hps2[:], lhsT=Tsb[:], rhs=Vs["W23"][:], start=True, stop=True)
    outsb = mpool.tile([NG, 1], F32, tag="outsb")
    nc.vector.tensor_tensor(out=outsb[:], in0=hps2[:], in1=Cc_sb[:], op=OP.add)
    nc.sync.dma_start(out=out_d[:], in_=outsb[:])


# ================= host side =================

def _wrap_idx(ix: np.ndarray) -> np.ndarray:
    n = ix.shape[0]
    w = ix.reshape(n // 16, 16).T.astype(np.int16)
    return np.tile(w, (8, 1)).copy()


def _prep_weights(cfg, W1, b1, bn_gamma, bn_beta, bn_mean, bn_var, W_lin1, b_lin1,
                  p1, Wl, Wr, att, b_gat, p2, W_lin2, b_lin2, W_lin3, b_lin3):
    f32 = np.float32
    bn_a = (bn_gamma / np.sqrt(bn_var + 1e-5)).astype(f32)
    bn_b = (bn_beta - bn_mean * bn_a).astype(f32)
    W23 = (W_lin2 @ W_lin3).reshape(-1).astype(f32)
    Cc = np.array([cfg.k2 * float(b_lin2 @ W_lin3[:, 0]) + float(b_lin3[0])], dtype=f32)
    p1n = (np.asarray(p1) / np.linalg.norm(np.asarray(p1))).astype(np.float64)
    Wlin1p = np.concatenate(
        [np.asarray(W_lin1, np.float64),
         (np.asarray(W_lin1, np.float64) @ p1n)[:, None]], axis=1
    ).astype(f32)
    c1 = np.array([float(p1n @ np.asarray(b_lin1, np.float64))], dtype=f32)
    return {
        "W1dup": np.ascontiguousarray(
            np.concatenate([np.asarray(W1, f32)] * 2, axis=1)
        ),
        "Wlin1p": Wlin1p,
        "Wl": np.ascontiguousarray(Wl, f32), "Wr": np.ascontiguousarray(Wr, f32),
        "bn_a": bn_a, "bn_bp": (np.asarray(b1, f32) * bn_a + bn_b).astype(f32),
        "b_lin1c": np.concatenate([np.asarray(b_lin1, f32), c1]),
        "att": np.ascontiguousarray(att, f32), "b_gat": np.ascontiguousarray(b_gat, f32),
        "p2": (np.asarray(p2) / np.linalg.norm(np.asarray(p2))).astype(f32),
        "W23": W23, "Cc": Cc,
    }


def _prep_core_edges(cfg: Cfg, src_core, dst_core):
    """src/dst core-local [ne]. Per graph: append self loops, bucket edges by
    dst block (db = dst>>7), pad each db run to spd*128 slots. Pad slots get
    src=0 (any valid row; killed by the one-hot) and dlo=255 (matches no
    iota value -> all-zero one-hot row/column)."""
    SPD, SRUN = cfg.spd, cfg.spd * P
    loops = np.arange(cfg.npg, dtype=np.int64)
    src_slots = np.zeros((cfg.ng, cfg.tj, SRUN), np.int64)
    dlo_slots = np.full((cfg.ng, cfg.tj, SRUN), 255, np.int64)
    deg = np.zeros((cfg.ng, cfg.npg), np.int64)
    for g in range(cfg.ng):
        e = slice(g * cfg.eg, (g + 1) * cfg.eg)
        s = np.concatenate([src_core[e] - g * cfg.npg, loops])
        d = np.concatenate([dst_core[e] - g * cfg.npg, loops])
        deg[g] = np.bincount(d, minlength=cfg.npg)
        db = d >> 7
        for b in range(cfg.tj):
            m = db == b
            cnt = int(m.sum())
            assert cnt <= SRUN, f"db run overflow: {cnt} > {SRUN}"
            src_slots[g, b, :cnt] = s[m]
            dlo_slots[g, b, :cnt] = d[m] & 127
    stream_src = src_slots.reshape(-1)
    stream_dlo = dlo_slots.reshape(-1)
    deg_t = np.ascontiguousarray(
        deg.reshape(cfg.ng, cfg.tj, P).transpose(2, 0, 1).reshape(P, cfg.nt)
    ).astype(np.float32)
    bf16 = ml_dtypes.bfloat16
    dinv = (1.0 / np.sqrt(np.maximum(deg.reshape(-1), 1.0))).astype(np.float32)
    return {
        "srcw": _wrap_idx(stream_src),
        "dinvF": dinv,
        "dlo_pm": np.ascontiguousarray(
            stream_dlo.reshape(-1, P).T.astype(bf16)
        ),
        "dlo_fr": np.ascontiguousarray(dlo_slots.reshape(-1).astype(bf16)),
        "degT": deg_t,
    }


def build_bass(cfg: Cfg):
    from contextlib import ExitStack
    nc = bacc.Bacc("TRN2", target_bir_lowering=False, debug=False,
                   num_swdge_queues=4)
    with tile.TileContext(nc) as tc:
        with ExitStack() as ctx:
            build_core_program(ctx, tc, cfg)
    nc.compile()
    return nc


_CFG = Cfg()
_NC_CACHE = {}
TRACE = False
LAST_RESULT = None


def kernel(x, edge_index, batch, W1, b1, bn_gamma, bn_beta, bn_mean, bn_var,
           W_lin1, b_lin1, p1, Wl, Wr, att, b_gat, p2,
           W_lin2, b_lin2, W_lin3, b_lin3):
    cfg = _CFG
    n_cores = 8
    s_att = float(np.sum(np.asarray(att, dtype=np.float64)))
    assert abs(s_att) > 1e-6, "degenerate att sum; poison scheme needs |sum(att)|>0"
    cfg.psign = -1.0 if s_att > 0 else 1.0
    slope = 0.2 if s_att > 0 else 1.0
    cfg.pb_mag = 40.0 / (slope * abs(s_att))
    weights = _prep_weights(cfg, W1, b1, bn_gamma, bn_beta, bn_mean, bn_var,
                            W_lin1, b_lin1, p1, Wl, Wr, att, b_gat, p2,
                            W_lin2, b_lin2, W_lin3, b_lin3)
    src_all = np.asarray(edge_index[0], dtype=np.int64)
    dst_all = np.asarray(edge_index[1], dtype=np.int64)
    x = np.asarray(x, dtype=np.float32)

    # choose the chunks-per-db-run capacity from the data (global max so the
    # single SPMD program fits every core)
    max_run = 0
    for c in range(n_cores):
        for g in range(cfg.ng):
            e0 = c * cfg.ne + g * cfg.eg
            d = dst_all[e0 : e0 + cfg.eg] - (c * cfg.nn + g * cfg.npg)
            cnts = np.bincount(d >> 7, minlength=cfg.tj) + P  # + self loops
            max_run = max(max_run, int(cnts.max()))
    cfg.spd = (max_run + P - 1) // P
    # windows of ch slots must tile a graph's slot range exactly
    while (cfg.tj * cfg.spd * P) % cfg.ch != 0:
        cfg.spd += 1

    in_maps = []
    for c in range(n_cores):
        n0 = c * cfg.nn
        e0 = c * cfg.ne
        d = dict(weights)
        d.update(
            _prep_core_edges(
                cfg, src_all[e0 : e0 + cfg.ne] - n0, dst_all[e0 : e0 + cfg.ne] - n0
            )
        )
        d["xT"] = np.ascontiguousarray(x[n0 : n0 + cfg.nn].T, np.float32)
        in_maps.append(d)

    key = ("nc", cfg.spd, cfg.psign, cfg.pb_mag)
    if key not in _NC_CACHE:
        _NC_CACHE[key] = build_bass(cfg)
    nc = _NC_CACHE[key]
    global LAST_RESULT
    res = run_bass_kernel_spmd(nc, in_maps, core_ids=list(range(n_cores)), trace=TRACE)
    LAST_RESULT = res
    outs = [np.asarray(res.results[c]["out"]).reshape(cfg.ng, 1) for c in range(n_cores)]
    return np.concatenate(outs, axis=0).astype(np.float32)


# revision 40
# speedup vs baseline: 1.1663x; 1.1663x over previous
"""Trainium2 Bass kernel for nn_AGNN_EFG (GCN -> TopK pool -> GATv2 -> TopK pool -> head).

Self-contained: shards the B=64 graphs across 8 NeuronCores (8 graphs/core),
runs one SPMD Bass program, gathers the [64, 1] head output on host.

v4 design:
- Edges (incl self loops) sorted per graph by dst block (db = dst>>7), each
  db run padded to a fixed spd chunks of 128 -> static chunk->db schedule
  (SPMD-safe; spd chosen on host from the data).
- Scatter-adds are ONE-HOT MATMULS on PE: per chunk a [128e,128d] bf16
  one-hot (DVE is_equal; pad slots carry sentinel 255 -> zero rows)
  accumulates messages into per-graph PSUM. No dma_scatter_add.
- GAT's xr[dst] is a [128d,128e]-orientation one-hot matmul (one-hot built
  on the Scalar engine as relu(1-(dlo-d)^2)) reading xr blocks from SBUF.
- Only 2 indirect passes remain (GpSimd Q7 descriptor generation is the
  machine bottleneck): gather u[src] (GCN) and [xl|xr][src] (GAT).
- Fully per-graph pipelined: utab(g+2) build, dense/pool/gtab(g),
  gcn(g+1) and gat(g) interleave so the GpSimd gather stream never idles.
- GCN u rows split [u_hi|u_lo] bf16 (~f32 accuracy, one 128-wide matmul).
- score1 fused into the lin1 matmul via host-folded [W_lin1 | W_lin1@p1n].
- Degrees are host-precomputed index data (bincount of dst); rsqrt on device.
"""

import sys

sys.path.insert(0, "/opt/trn_rl_repo")

from dataclasses import dataclass

import numpy as np
import ml_dtypes

import concourse.bass as bass
import concourse.mybir as mybir
import concourse.tile as tile
from concourse import bacc
from concourse.bass_utils import run_bass_kernel_spmd
from concourse.masks import make_identity

P = 128
F32 = mybir.dt.float32
BF16 = mybir.dt.bfloat16
I16 = mybir.dt.int16
U8 = mybir.dt.uint8
AF = mybir.ActivationFunctionType
OP = mybir.AluOpType
AX = mybir.AxisListType


@dataclass
class Cfg:
    ng: int = 8          # graphs per core
    npg: int = 2048      # nodes per graph
    hid: int = 64        # feature dim
    eg: int = 32768      # edges per graph (original, without self loops)
    spd: int = 19        # chunks (of 128 slots) per dst-block run; set at runtime
    ch: int = 1024       # gather window (slots per dma_gather call; >=2048
                         # overflows the SWDGE descriptor ring and faults)
    n_bisect: int = 30   # bisection iterations for topk threshold
    psign: float = -1.0  # -sign(sum(att)): poison sign so poisoned e < 0
    pb_mag: float = 200.0  # poison magnitude; set so poisoned e ~ -40 (LUT-safe)

    @property
    def nn(self):
        return self.ng * self.npg

    @property
    def ne(self):
        return self.ng * self.eg

    @property
    def tj(self):
        return self.npg // P  # dst blocks per graph (16)

    @property
    def nt(self):
        return self.ng * self.tj  # 128

    @property
    def tch(self):
        return self.tj * self.spd  # chunks per graph

    @property
    def slots_g(self):
        return self.tch * P  # padded edge slots per graph

    @property
    def slots(self):
        return self.ng * self.slots_g

    @property
    def wpg(self):
        assert self.slots_g % self.ch == 0
        return self.slots_g // self.ch  # gather windows per graph

    @property
    def cpw(self):
        return self.ch // P  # chunks per window

    @property
    def k1(self):
        return self.npg // 2

    @property
    def k2(self):
        return self.npg // 4


def build_core_program(ctx, tc, cfg: Cfg):
    nc = tc.nc
    NG, NPG, HID, NN = cfg.ng, cfg.npg, cfg.hid, cfg.nn
    NT, TJ, SPD = cfg.nt, cfg.tj, cfg.spd
    TCH, WPG, CPW, CH = cfg.tch, cfg.wpg, cfg.cpw, cfg.ch
    SRUN = SPD * P  # slots per db run
    assert HID == 64 and CPW % 8 == 0

    # ---- I/O ----
    xT = nc.dram_tensor("xT", [HID, NN], F32, kind="ExternalInput").ap()
    srcw = nc.dram_tensor("srcw", [P, cfg.slots // 16], I16, kind="ExternalInput").ap()
    dlo_pm = nc.dram_tensor("dlo_pm", [P, NG * TCH], BF16, kind="ExternalInput").ap()
    dlo_fr = nc.dram_tensor("dlo_fr", [NG * TJ * SRUN], BF16, kind="ExternalInput").ap()
    degT = nc.dram_tensor("degT", [P, NT], F32, kind="ExternalInput").ap()
    w_names = ["Wl", "Wr"]
    Wd = {n: nc.dram_tensor(n, [HID, HID], F32, kind="ExternalInput").ap() for n in w_names}
    W1d = nc.dram_tensor("W1dup", [HID, P], F32, kind="ExternalInput").ap()
    W1p = nc.dram_tensor("Wlin1p", [HID, HID + 1], F32, kind="ExternalInput").ap()
    dinvF = nc.dram_tensor("dinvF", [NN], F32, kind="ExternalInput").ap()
    b1c = nc.dram_tensor("b_lin1c", [HID + 1], F32, kind="ExternalInput").ap()
    v_names = ["bn_a", "bn_bp", "att", "b_gat", "p2", "W23"]
    Vd = {n: nc.dram_tensor(n, [HID], F32, kind="ExternalInput").ap() for n in v_names}
    Cd = nc.dram_tensor("Cc", [1], F32, kind="ExternalInput").ap()
    out_d = nc.dram_tensor("out", [NG, 1], F32, kind="ExternalOutput").ap()

    # ---- DRAM scratch (per-graph tables so gathers only depend on their
    # own graph's writes) ----
    utab = [nc.dram_tensor(f"utab{g}", [NPG, P], BF16).ap() for g in range(NG)]
    gtab = [nc.dram_tensor(f"gtab{g}", [NPG, P], BF16).ap() for g in range(NG)]
    xlr_dram = nc.dram_tensor("xlr_dram", [P, NN], BF16).ap()
    ufm_dram = nc.dram_tensor("ufm_dram", [P, NN], BF16).ap()
    sc_dram = nc.dram_tensor("sc_dram", [NN], F32).ap()

    cpool = ctx.enter_context(tc.tile_pool(name="consts", bufs=1))
    mpool = ctx.enter_context(tc.tile_pool(name="main", bufs=1))
    smallps = ctx.enter_context(tc.tile_pool(name="smallps", bufs=1, space="PSUM"))

    # ---- constants ----
    ident = cpool.tile([P, P], F32)
    make_identity(nc, ident[:])
    ones128 = cpool.tile([P, P], F32)
    nc.vector.memset(ones128[:], 1.0)
    nantile = cpool.tile([P, NT], F32)
    nc.vector.memset(nantile[:], float("nan"))
    negbig = cpool.tile([P, NT], F32)
    nc.vector.memset(negbig[:], -1e9)
    io16 = cpool.tile([P, P], I16)
    nc.gpsimd.iota(io16[:], pattern=[[1, P]], base=0, channel_multiplier=0)
    iota_bf = cpool.tile([P, P], BF16)
    nc.vector.tensor_copy(out=iota_bf[:], in_=io16[:])
    ioc16 = cpool.tile([P, 1], I16)
    nc.gpsimd.iota(ioc16[:], pattern=[[0, 1]], base=0, channel_multiplier=1)
    niota_col = cpool.tile([P, 1], F32)
    nc.vector.tensor_scalar(
        out=niota_col[:], in0=ioc16[:], scalar1=-1.0, scalar2=None, op0=OP.mult
    )

    Ws = {}
    for n in w_names:
        t = cpool.tile([HID, HID], F32, tag=f"w_{n}")
        nc.sync.dma_start(out=t[:], in_=Wd[n][:])
        Ws[n] = t
    W1p_sb = cpool.tile([HID, HID + 1], F32, tag="w_Wlin1p")
    nc.sync.dma_start(out=W1p_sb[:], in_=W1p[:])
    W1d_sb = cpool.tile([HID, P], F32, tag="w_W1dup")
    nc.sync.dma_start(out=W1d_sb[:], in_=W1d[:])
    b1c_sb = cpool.tile([HID + 1, 1], F32, tag="v_b1c")
    nc.sync.dma_start(out=b1c_sb[:], in_=b1c[:, None])
    Vs = {}
    for n in v_names:
        t = cpool.tile([HID, 1], F32, tag=f"v_{n}")
        nc.sync.dma_start(out=t[:], in_=Vd[n][:, None])
        Vs[n] = t
    att_rep = cpool.tile([P, HID], BF16)
    nc.gpsimd.dma_start(out=att_rep[:], in_=Vd["att"][None, :].to_broadcast([P, HID]))
    p2_rep = cpool.tile([P, HID], F32)
    nc.sync.dma_start(out=p2_rep[:], in_=Vd["p2"][None, :].to_broadcast([P, HID]))
    bgat_rep = cpool.tile([P, HID], F32)
    nc.sync.dma_start(out=bgat_rep[:], in_=Vd["b_gat"][None, :].to_broadcast([P, HID]))
    Cc_sb = cpool.tile([NG, 1], F32)
    nc.sync.dma_start(out=Cc_sb[:], in_=Cd[None, :].to_broadcast([NG, 1]))

    # whole per-chunk dst-low-bit table (for one-hot builds in [e,d] orientation)
    dlo_sb = cpool.tile([P, NG * TCH], BF16)
    nc.sync.dma_start(out=dlo_sb[:], in_=dlo_pm[:])

    # ---- dinv from host degree counts ----
    dinv_t = mpool.tile([P, NT], F32, tag="dinv_t")
    sqd_t = mpool.tile([P, NT], F32, tag="sqd_t")
    ntmp = mpool.tile([P, NT], F32, tag="ntmp")

    def recip_newton(r_ap, x_ap, tmp_ap):
        nc.vector.tensor_tensor(out=tmp_ap, in0=x_ap, in1=r_ap, op=OP.mult)
        nc.vector.tensor_scalar(
            out=tmp_ap, in0=tmp_ap, scalar1=-1.0, scalar2=2.0, op0=OP.mult, op1=OP.add
        )
        nc.vector.tensor_tensor(out=r_ap, in0=r_ap, in1=tmp_ap, op=OP.mult)

    deg_sb = mpool.tile([P, NT], F32, tag="deg_sb")
    nc.sync.dma_start(out=deg_sb[:], in_=degT[:])
    nc.scalar.sqrt(out=sqd_t[:], in_=deg_sb[:])
    nc.vector.reciprocal(out=dinv_t[:], in_=sqd_t[:])
    recip_newton(dinv_t[:], sqd_t[:], ntmp[:])

    def idx_slice(pool, w_global, tag):
        t = pool.tile([P, CH // 16], I16, tag=tag)
        c0 = w_global * (CH // 16)
        nc.sync.dma_start(out=t[:], in_=srcw[:, c0 : c0 + CH // 16])
        return t

    # ---- pools (all phases interleave; PSUM budget: 2+1+2+2+1 = 8 banks) ----
    ups = ctx.enter_context(tc.tile_pool(name="ups", bufs=2))
    ubc = ctx.enter_context(tc.tile_pool(name="ubc", bufs=1))
    gep = ctx.enter_context(tc.tile_pool(name="gep", bufs=5))
    ohp = ctx.enter_context(tc.tile_pool(name="ohp", bufs=2))
    hps_pool = ctx.enter_context(tc.tile_pool(name="hpsp", bufs=1, space="PSUM"))
    dsa = ctx.enter_context(tc.tile_pool(name="dsa", bufs=1))
    dss = ctx.enter_context(tc.tile_pool(name="dss", bufs=2))
    bis = ctx.enter_context(tc.tile_pool(name="bis", bufs=2))
    gap = ctx.enter_context(tc.tile_pool(name="gap", bufs=3))
    aohp = ctx.enter_context(tc.tile_pool(name="aohp", bufs=2))
    runtmp = ctx.enter_context(tc.tile_pool(name="runtmp", bufs=1))
    runp = ctx.enter_context(tc.tile_pool(name="runp", bufs=2))
    gatps = ctx.enter_context(tc.tile_pool(name="gatps", bufs=1, space="PSUM"))
    mxrp = ctx.enter_context(tc.tile_pool(name="mxrp", bufs=2, space="PSUM"))
    gfin = ctx.enter_context(tc.tile_pool(name="gfin", bufs=1))

    # persistent t-space tiles
    h2t = mpool.tile([P, NT, HID], F32, tag="bigA")
    gstage = mpool.tile([P, NT, P], BF16, tag="bigC")
    score1_t = mpool.tile([P, NT], F32, tag="score1_t")
    tanh1 = mpool.tile([P, NT], F32, tag="tanh1")
    gate1 = mpool.tile([P, NT], F32, tag="gate1")
    gate1z = mpool.tile([P, NT], F32, tag="gate1z")
    padd = mpool.tile([P, NT], F32, tag="padd")
    kept1 = mpool.tile([P, NT], U8, tag="kept1")
    pb = cfg.psign * cfg.pb_mag

    hps_tiles = {}
    nmps_tiles = {}

    # ======== per-graph sections ========

    def utab_build(g):
        # xw (duplicated rows) -> dinv scale -> [hi|lo] bf16 split, all in
        # feature-major; node-major utab rows via one transpose DMA hop.
        xw2 = ups.tile([P, NPG], F32, tag="xw2", name=f"xw2_{g}")
        for jl in range(TJ):
            xTg = ups.tile([HID, P], F32, tag="xTg", name=f"xTg{g}_{jl}")
            nc.sync.dma_start(
                out=xTg[:], in_=xT[:, g * NPG + jl * P : g * NPG + (jl + 1) * P]
            )
            pm = smallps.tile([P, P], F32, tag="smA", name=f"xwps{g}_{jl}")
            nc.tensor.matmul(pm[:], lhsT=W1d_sb[:], rhs=xTg[:], start=True, stop=True)
            nc.scalar.copy(out=xw2[:, jl * P : (jl + 1) * P], in_=pm[:])
        dbc = ubc.tile([P, NPG], F32, tag="dinvbc", name=f"dinvbc{g}")
        nc.sync.dma_start(
            out=dbc[:], in_=dinvF[g * NPG : (g + 1) * NPG][None, :].to_broadcast([P, NPG])
        )
        nc.vector.tensor_tensor(out=xw2[:], in0=xw2[:], in1=dbc[:], op=OP.mult)
        u2 = ups.tile([P, NPG], BF16, tag="u2", name=f"u2_{g}")
        nc.vector.tensor_copy(out=u2[0:HID, :], in_=xw2[0:HID, :])
        nc.vector.tensor_copy(out=u2[HID:P, :], in_=xw2[HID:P, :])
        nc.vector.tensor_tensor(
            out=u2[HID:P, :], in0=xw2[HID:P, :], in1=u2[HID:P, :], op=OP.subtract
        )
        nc.sync.dma_start(out=ufm_dram[:, g * NPG : (g + 1) * NPG], in_=u2[:])
        u_nm = ups.tile([P, TJ, P], BF16, tag="u2", name=f"unm{g}")
        nc.sync.dma_start_transpose(
            out=u_nm[:], in_=ufm_dram[:, g * NPG : (g + 1) * NPG]
        )
        nc.sync.dma_start(
            out=utab[g].rearrange("(j p) f -> p j f", p=P), in_=u_nm[:]
        )

    def gcn_pass(g):
        hps = [
            hps_pool.tile([P, 8, HID], F32, tag=f"hps{t}", name=f"hps{t}_{g}")
            for t in range(2)
        ]
        hps_tiles[g] = hps
        for w in range(WPG):
            wg = g * WPG + w
            ssl = idx_slice(gep, wg, "ssl")
            ub = gep.tile([P, CPW, P], BF16, tag="ub")
            nc.gpsimd.dma_gather(
                out_ap=ub[:], in_ap=utab[g][:], idxs_ap=ssl[:],
                num_idxs=CH, num_idxs_reg=CH, elem_size=P, queue_num=wg % 4,
            )
            for b in range(CPW // 8):
                oh8 = ohp.tile([P, 8, P], BF16, tag="oh8")
                c0 = g * TCH + w * CPW + b * 8
                nc.vector.tensor_tensor(
                    out=oh8[:],
                    in0=iota_bf[:, None, :].to_broadcast([P, 8, P]),
                    in1=dlo_sb[:, c0 : c0 + 8, None].to_broadcast([P, 8, P]),
                    op=OP.is_equal,
                )
                for cl in range(8):
                    c = b * 8 + cl
                    gc = w * CPW + c
                    db, pos = gc // SPD, gc % SPD
                    out_slc = hps[db // 8][:, db % 8, :]
                    nc.tensor.matmul(
                        out_slc, lhsT=oh8[:, cl, :], rhs=ub[:, c, 0:HID],
                        start=(pos == 0), stop=False,
                    )
                    nc.tensor.matmul(
                        out_slc, lhsT=oh8[:, cl, :], rhs=ub[:, c, HID:P],
                        start=False, stop=(pos == SPD - 1),
                    )

    S6C = 512

    def dense_g(g):
        # h = dinv*(hi+lo); BN+leaky (one ACT Lrelu); lin1(+score); xl/xr
        hps = hps_tiles.pop(g)
        gsl = slice(g * TJ, (g + 1) * TJ)
        hsum = dsa.tile([P, TJ, HID], F32, tag="hsum", name=f"hsum{g}")
        for t in range(2):
            nc.vector.tensor_tensor(
                out=hsum[:, t * 8 : (t + 1) * 8, :], in0=hps[t][:],
                in1=dinv_t[:, g * TJ + t * 8 : g * TJ + (t + 1) * 8, None].to_broadcast(
                    [P, 8, HID]
                ),
                op=OP.mult,
            )
        hfm = dsa.tile([HID, NPG], F32, tag="hfm", name=f"hfm{g}")
        for jl in range(TJ):
            pt = smallps.tile([HID, P], F32, tag="smA", name=f"htr{g}_{jl}")
            nc.tensor.transpose(out=pt[:], in_=hsum[:, jl, :], identity=ident[:])
            nc.scalar.copy(out=hfm[:, jl * P : (jl + 1) * P], in_=pt[:])
        nc.scalar.activation(
            out=hfm[:], in_=hfm[:], func=AF.Lrelu, scale=Vs["bn_a"][:],
            bias=Vs["bn_bp"][:], alpha=0.01,
        )
        xlr_g = dss.tile([P, NPG], BF16, tag="xlrg", name=f"xlrg{g}")
        for ol in range(0, NPG, S6C):
            o = g * NPG + ol
            pm = hps_pool.tile([HID + 1, S6C], F32, tag="hps0", name=f"l1ps{o}")
            nc.tensor.matmul(
                pm[:], lhsT=W1p_sb[:], rhs=hfm[:, ol : ol + S6C],
                start=True, stop=True,
            )
            hc = dss.tile([HID + 1, S6C], F32, tag="hc", name=f"hc{o}")
            nc.scalar.activation(
                out=hc[:], in_=pm[:], func=AF.Identity, bias=b1c_sb[:]
            )
            nc.sync.dma_start(out=sc_dram[None, o : o + S6C], in_=hc[HID : HID + 1, :])
            px = hps_pool.tile([P, S6C], F32, tag="hps1", name=f"xlrps{o}")
            nc.tensor.matmul(px[:HID, :], lhsT=Ws["Wl"][:], rhs=hc[0:HID, :], start=True, stop=True)
            nc.tensor.matmul(px[HID:, :], lhsT=Ws["Wr"][:], rhs=hc[0:HID, :], start=True, stop=True)
            nc.scalar.copy(out=xlr_g[:, ol : ol + S6C], in_=px[:])
        nc.sync.dma_start(out=xlr_dram[:, g * NPG : (g + 1) * NPG], in_=xlr_g[:])

    def bisect_multi(score_slc, ngr, target, tag):
        # score_slc: [P, ngr*TJ]; returns per-graph thresholds lo [P, ngr]
        lo = bis.tile([P, ngr], F32, tag="lo", name=f"lo_{tag}")
        hi = bis.tile([P, ngr], F32, tag="hi", name=f"hi_{tag}")
        mid = bis.tile([P, ngr], F32, tag="mid", name=f"mid_{tag}")
        cmp = bis.tile([P, ngr * TJ], F32, tag="cmp", name=f"cmp_{tag}")
        cred = bis.tile([P, ngr], F32, tag="cred", name=f"cred_{tag}")
        ge = bis.tile([P, ngr], U8, tag="ge", name=f"ge_{tag}")
        lt = bis.tile([P, ngr], U8, tag="lt", name=f"lt_{tag}")
        nc.vector.memset(lo[:], -64.0)
        nc.vector.memset(hi[:], 64.0)
        sc_g = score_slc.rearrange("p (g t) -> p g t", g=ngr)
        cmp_g = cmp[:].rearrange("p (g t) -> p g t", g=ngr)
        for it in range(cfg.n_bisect):
            nc.vector.tensor_tensor(out=mid[:], in0=lo[:], in1=hi[:], op=OP.add)
            nc.vector.tensor_scalar(
                out=mid[:], in0=mid[:], scalar1=0.5, scalar2=None, op0=OP.mult
            )
            nc.vector.tensor_tensor(
                out=cmp_g, in0=sc_g,
                in1=mid[:, :, None].to_broadcast([P, ngr, TJ]), op=OP.is_gt,
            )
            nc.vector.tensor_reduce(out=cred[:], in_=cmp_g, axis=AX.X, op=OP.add)
            cps = smallps.tile([P, ngr], F32, tag="smB", name=f"cnt_{tag}_{it}")
            nc.tensor.matmul(cps[:], lhsT=ones128[:], rhs=cred[:], start=True, stop=True)
            nc.vector.tensor_scalar(
                out=ge[:], in0=cps[:], scalar1=float(target), scalar2=None, op0=OP.is_ge
            )
            nc.vector.tensor_scalar(
                out=lt[:], in0=cps[:], scalar1=float(target), scalar2=None, op0=OP.is_lt
            )
            nc.vector.copy_predicated(out=lo[:], mask=ge[:], data=mid[:])
            nc.vector.copy_predicated(out=hi[:], mask=lt[:], data=mid[:])
        return lo

    NH = NG // 2  # graphs per pooling half

    def pool1_h(h):
        g0 = h * NH
        gsl = slice(g0 * TJ, (g0 + NH) * TJ)
        nc.sync.dma_start(
            out=score1_t[:, gsl],
            in_=sc_dram[g0 * NPG : (g0 + NH) * NPG].rearrange("(j p) -> p j", p=P),
        )
        t1 = bisect_multi(score1_t[:, gsl], NH, cfg.k1, f"p1h{h}")
        nc.vector.tensor_tensor(
            out=kept1[:, gsl].rearrange("p (g t) -> p g t", g=NH),
            in0=score1_t[:, gsl].rearrange("p (g t) -> p g t", g=NH),
            in1=t1[:, :, None].to_broadcast([P, NH, TJ]), op=OP.is_gt,
        )
        nc.scalar.activation(out=tanh1[:, gsl], in_=score1_t[:, gsl], func=AF.Tanh)
        nc.vector.tensor_copy(out=gate1[:, gsl], in_=nantile[:, gsl])
        nc.vector.copy_predicated(out=gate1[:, gsl], mask=kept1[:, gsl], data=tanh1[:, gsl])
        nc.vector.memset(gate1z[:, gsl], 0.0)
        nc.vector.copy_predicated(out=gate1z[:, gsl], mask=kept1[:, gsl], data=tanh1[:, gsl])
        nc.vector.tensor_scalar(
            out=padd[:, gsl], in0=kept1[:, gsl], scalar1=-pb, scalar2=pb,
            op0=OP.mult, op1=OP.add,
        )
        # gtab for the half
        gs = gstage[:, gsl, :]
        nc.sync.dma_start_transpose(
            out=gs, in_=xlr_dram[:, g0 * NPG : (g0 + NH) * NPG]
        )
        nc.vector.tensor_tensor(
            out=gs, in0=gs,
            in1=gate1z[:, gsl, None].to_broadcast([P, NH * TJ, P]), op=OP.mult,
        )
        nc.vector.tensor_tensor(
            out=gs, in0=gs,
            in1=padd[:, gsl, None].to_broadcast([P, NH * TJ, P]), op=OP.add,
        )
        for gg in range(g0, g0 + NH):
            nc.sync.dma_start(
                out=gtab[gg].rearrange("(j p) f -> p j f", p=P),
                in_=gstage[:, gg * TJ : (gg + 1) * TJ, :],
            )

    def gat_pass(g):
        nmps = [
            gatps.tile([P, 8, HID], F32, tag=f"nmps{t}", name=f"nmps{t}_{g}")
            for t in range(2)
        ]
        dnps = smallps.tile([P, TJ], F32, tag="smB", name=f"dnps{g}")
        oh_de = {}
        for w in range(WPG):
            wg = g * WPG + w
            ssl = idx_slice(gap, wg, "assl")
            gx = gap.tile([P, CPW, P], BF16, tag="gx")
            nc.gpsimd.dma_gather(
                out_ap=gx[:], in_ap=gtab[g][:], idxs_ap=ssl[:],
                num_idxs=CH, num_idxs_reg=CH, elem_size=P, queue_num=wg % 4,
            )
            for b in range(CPW // 8):
                oh8 = aohp.tile([P, 8, P], BF16, tag="aoh8")
                c0 = g * TCH + w * CPW + b * 8
                nc.vector.tensor_tensor(
                    out=oh8[:],
                    in0=iota_bf[:, None, :].to_broadcast([P, 8, P]),
                    in1=dlo_sb[:, c0 : c0 + 8, None].to_broadcast([P, 8, P]),
                    op=OP.is_equal,
                )
                mxr = mxrp.tile([P, 8, HID], F32, tag="mxr", name=f"mxr{wg}_{b}")
                for cl in range(8):
                    c = b * 8 + cl
                    gc = w * CPW + c
                    r, pos = gc // SPD, gc % SPD
                    if pos == 0:
                        dlo_bc = runtmp.tile(
                            [P, SRUN], BF16, tag="dlobc", name=f"dlobc{g}_{r}"
                        )
                        o = (g * TJ + r) * SRUN
                        nc.sync.dma_start(
                            out=dlo_bc[:],
                            in_=dlo_fr[o : o + SRUN][None, :].to_broadcast([P, SRUN]),
                        )
                        # one-hot on the Scalar engine: relu(1 - (dlo - d)^2)
                        ohsq = runtmp.tile([P, SRUN], BF16, tag="ohsq", name=f"ohsq{g}_{r}")
                        nc.scalar.activation(
                            out=ohsq[:], in_=dlo_bc[:], func=AF.Square,
                            bias=niota_col[:],
                        )
                        ohr = runp.tile([P, SRUN], BF16, tag="ohde", name=f"ohde{g}_{r}")
                        nc.scalar.activation(
                            out=ohr[:], in_=ohsq[:], func=AF.Relu,
                            bias=1.0, scale=-1.0,
                        )
                        oh_de[r] = ohr
                    nc.tensor.matmul(
                        mxr[:, cl, :],
                        lhsT=oh_de[r][:, pos * P : (pos + 1) * P],
                        rhs=gstage[:, g * TJ + r, HID:P],
                        start=True, stop=True,
                    )
                # e = att . leaky(xl_s + xr_d); w = exp(e); pay = w*xl
                gxs = gx[:, b * 8 : (b + 1) * 8, :]
                z = gap.tile([P, 8, HID], BF16, tag="z")
                nc.vector.tensor_tensor(
                    out=z[:], in0=gxs[:, :, 0:HID], in1=mxr[:], op=OP.add
                )
                nc.vector.scalar_tensor_tensor(
                    out=z[:], in0=z[:], scalar=0.2, in1=z[:], op0=OP.mult, op1=OP.max,
                )
                nc.vector.tensor_tensor(
                    out=z[:], in0=z[:],
                    in1=att_rep[:, None, :].to_broadcast([P, 8, HID]), op=OP.mult,
                )
                e8 = gap.tile([P, 8], F32, tag="e8")
                nc.vector.tensor_reduce(out=e8[:], in_=z[:], axis=AX.X, op=OP.add)
                w8b = gap.tile([P, 8], BF16, tag="w8b")
                nc.scalar.activation(out=w8b[:], in_=e8[:], func=AF.Exp)
                pay = gap.tile([P, 8, HID], BF16, tag="pay")
                nc.vector.tensor_tensor(
                    out=pay[:], in0=gxs[:, :, 0:HID],
                    in1=w8b[:, :, None].to_broadcast([P, 8, HID]), op=OP.mult,
                )
                for cl in range(8):
                    c = b * 8 + cl
                    gc = w * CPW + c
                    db, pos = gc // SPD, gc % SPD
                    nc.tensor.matmul(
                        nmps[db // 8][:, db % 8, :],
                        lhsT=oh8[:, cl, :], rhs=pay[:, cl, :],
                        start=(pos == 0), stop=(pos == SPD - 1),
                    )
                    nc.tensor.matmul(
                        dnps[:, db : db + 1],
                        lhsT=oh8[:, cl, :], rhs=w8b[:, cl : cl + 1],
                        start=(pos == 0), stop=(pos == SPD - 1),
                    )

        # ---- finalize graph g: h2 = leaky(numer/denom + b_gat) ----
        numsb = dsa.tile([P, TJ, HID], F32, tag="hsum", name=f"numsb{g}")
        nc.scalar.copy(out=numsb[:, 0:8, :], in_=nmps[0][:])
        nc.scalar.copy(out=numsb[:, 8:TJ, :], in_=nmps[1][:])
        den = gfin.tile([P, TJ], F32, tag="den", name=f"den{g}")
        rec = gfin.tile([P, TJ], F32, tag="rec", name=f"rec{g}")
        dtmp = gfin.tile([P, TJ], F32, tag="dtmp", name=f"dtmp{g}")
        nc.vector.tensor_scalar(
            out=den[:], in0=dnps[:], scalar1=1e-16, scalar2=None, op0=OP.add
        )
        nc.vector.reciprocal(out=rec[:], in_=den[:])
        recip_newton(rec[:], den[:], dtmp[:])
        hslc = h2t[:, g * TJ : (g + 1) * TJ, :]
        nc.vector.tensor_tensor(
            out=hslc, in0=numsb[:],
            in1=rec[:, :, None].to_broadcast([P, TJ, HID]), op=OP.mult,
        )
        nc.vector.tensor_tensor(
            out=hslc, in0=hslc,
            in1=bgat_rep[:, None, :].to_broadcast([P, TJ, HID]), op=OP.add,
        )
        nc.vector.scalar_tensor_tensor(
            out=hslc, in0=hslc, scalar=0.01, in1=hslc, op0=OP.mult, op1=OP.max
        )

    # ======== pipelined emission (two pooling halves) ========
    utab_build(0)
    utab_build(1)
    for g in range(NH):
        gcn_pass(g)
        dense_g(g)
        if g + 2 < NG:
            utab_build(g + 2)
    gcn_pass(NH)
    pool1_h(0)
    dense_g(NH)
    for g in range(NH):
        gat_pass(g)
        if g + NH + 1 < NG:
            gcn_pass(g + NH + 1)
            dense_g(g + NH + 1)
        if g + NH + 2 < NG:
            utab_build(g + NH + 2)
    pool1_h(1)
    for g in range(NH, NG):
        gat_pass(g)

    # ======== score2 (t-space, blocked), mask to kept1 ========
    score2_t = mpool.tile([P, NT], F32, tag="score2_t")
    for t in range(4):
        tsl = slice(t * 32, (t + 1) * 32)
        blk = ups.tile([P, 32, HID], F32, tag="xTg", name=f"s2blk{t}")
        nc.vector.tensor_tensor(
            out=blk[:], in0=h2t[:, tsl, :],
            in1=p2_rep[:, None, :].to_broadcast([P, 32, HID]), op=OP.mult,
        )
        nc.vector.tensor_reduce(out=score2_t[:, tsl], in_=blk[:], axis=AX.X, op=OP.add)
    kept1_t = mpool.tile([P, NT], U8, tag="kept1_t")
    nc.vector.tensor_tensor(out=kept1_t[:], in0=gate1[:], in1=gate1[:], op=OP.is_equal)
    sc2m = mpool.tile([P, NT], F32, tag="sc2m")
    nc.vector.tensor_copy(out=sc2m[:], in_=negbig[:])
    nc.vector.copy_predicated(out=sc2m[:], mask=kept1_t[:], data=score2_t[:])

    # ======== pool2 threshold + gate2 = tanh * mask ========
    gate2 = mpool.tile([P, NT], F32, tag="gate2")
    t2 = bisect_multi(sc2m[:], NG, cfg.k2, "p2")
    nc.vector.tensor_tensor(
        out=gate2[:].rearrange("p (g t) -> p g t", g=NG),
        in0=sc2m[:].rearrange("p (g t) -> p g t", g=NG),
        in1=t2[:, :, None].to_broadcast([P, NG, TJ]), op=OP.is_gt,
    )
    tanh2 = mpool.tile([P, NT], F32, tag="tanh2")
    sc2c = mpool.tile([P, NT], F32, tag="sc2c")
    nc.vector.tensor_scalar(
        out=sc2c[:], in0=sc2m[:], scalar1=-64.0, scalar2=None, op0=OP.max
    )
    nc.scalar.activation(out=tanh2[:], in_=sc2c[:], func=AF.Tanh)
    nc.vector.tensor_tensor(out=gate2[:], in0=gate2[:], in1=tanh2[:], op=OP.mult)

    # ======== T_g = sum_n gate2[n] * h2[n]; out = T @ W23 + C ========
    Tps = smallps.tile([P, NG], F32, tag="smB")
    for j in range(NT):
        g = j // TJ
        nc.tensor.matmul(
            Tps[:HID, g : g + 1], lhsT=h2t[:, j, :], rhs=gate2[:, j : j + 1],
            start=(j % TJ == 0), stop=(j % TJ == TJ - 1),
        )
    Tsb = mpool.tile([HID, NG], F32, tag="Tsb")
    nc.scalar.copy(out=Tsb[:], in_=Tps[:HID, :])
    hps2 = smallps.tile([NG, 1], F32, tag="smB")
    nc.tensor.matmul(hps2[:], lhsT=Tsb[:], rhs=Vs["W23"][:], start=True, stop=True)
    outsb = mpool.tile([NG, 1], F32, tag="outsb")
    nc.vector.tensor_tensor(out=outsb[:], in0=hps2[:], in1=Cc_sb[:], op=OP.add)
    nc.sync.dma_start(out=out_d[:], in_=outsb[:])


# ================= host side =================

def _wrap_idx(ix: np.ndarray) -> np.ndarray:
    n = ix.shape[0]
    w = ix.reshape(n // 16, 16).T.astype(np.int16)
    return np.tile(w, (8, 1)).copy()


def _prep_weights(cfg, W1, b1, bn_gamma, bn_beta, bn_mean, bn_var, W_lin1, b_lin1,
                  p1, Wl, Wr, att, b_gat, p2, W_lin2, b_lin2, W_lin3, b_lin3):
    f32 = np.float32
    bn_a = (bn_gamma / np.sqrt(bn_var + 1e-5)).astype(f32)
    bn_b = (bn_beta - bn_mean * bn_a).astype(f32)
    W23 = (W_lin2 @ W_lin3).reshape(-1).astype(f32)
    Cc = np.array([cfg.k2 * float(b_lin2 @ W_lin3[:, 0]) + float(b_lin3[0])], dtype=f32)
    p1n = (np.asarray(p1) / np.linalg.norm(np.asarray(p1))).astype(np.float64)
    Wlin1p = np.concatenate(
        [np.asarray(W_lin1, np.float64),
         (np.asarray(W_lin1, np.float64) @ p1n)[:, None]], axis=1
    ).astype(f32)
    c1 = np.array([float(p1n @ np.asarray(b_lin1, np.float64))], dtype=f32)
    return {
        "W1dup": np.ascontiguousarray(
            np.concatenate([np.asarray(W1, f32)] * 2, axis=1)
        ),
        "Wlin1p": Wlin1p,
        "Wl": np.ascontiguousarray(Wl, f32), "Wr": np.ascontiguousarray(Wr, f32),
        "bn_a": bn_a, "bn_bp": (np.asarray(b1, f32) * bn_a + bn_b).astype(f32),
        "b_lin1c": np.concatenate([np.asarray(b_lin1, f32), c1]),
        "att": np.ascontiguousarray(att, f32), "b_gat": np.ascontiguousarray(b_gat, f32),
        "p2": (np.asarray(p2) / np.linalg.norm(np.asarray(p2))).astype(f32),
        "W23": W23, "Cc": Cc,
    }


def _prep_core_edges(cfg: Cfg, src_core, dst_core):
    """src/dst core-local [ne]. Per graph: append self loops, bucket edges by
    dst block (db = dst>>7), pad each db run to spd*128 slots. Pad slots get
    src=0 (any valid row; killed by the one-hot) and dlo=255 (matches no
    iota value -> all-zero one-hot row/column)."""
    SPD, SRUN = cfg.spd, cfg.spd * P
    loops = np.arange(cfg.npg, dtype=np.int64)
    src_slots = np.zeros((cfg.ng, cfg.tj, SRUN), np.int64)
    dlo_slots = np.full((cfg.ng, cfg.tj, SRUN), 255, np.int64)
    deg = np.zeros((cfg.ng, cfg.npg), np.int64)
    for g in range(cfg.ng):
        e = slice(g * cfg.eg, (g + 1) * cfg.eg)
        s = np.concatenate([src_core[e] - g * cfg.npg, loops])
        d = np.concatenate([dst_core[e] - g * cfg.npg, loops])
        deg[g] = np.bincount(d, minlength=cfg.npg)
        db = d >> 7
        for b in range(cfg.tj):
            m = db == b
            cnt = int(m.sum())
            assert cnt <= SRUN, f"db run overflow: {cnt} > {SRUN}"
            src_slots[g, b, :cnt] = s[m]
            dlo_slots[g, b, :cnt] = d[m] & 127
    stream_src = src_slots.reshape(-1)
    stream_dlo = dlo_slots.reshape(-1)
    deg_t = np.ascontiguousarray(
        deg.reshape(cfg.ng, cfg.tj, P).transpose(2, 0, 1).reshape(P, cfg.nt)
    ).astype(np.float32)
    bf16 = ml_dtypes.bfloat16
    dinv = (1.0 / np.sqrt(np.maximum(deg.reshape(-1), 1.0))).astype(np.float32)
    return {
        "srcw": _wrap_idx(stream_src),
        "dinvF": dinv,
        "dlo_pm": np.ascontiguousarray(
            stream_dlo.reshape(-1, P).T.astype(bf16)
        ),
        "dlo_fr": np.ascontiguousarray(dlo_slots.reshape(-1).astype(bf16)),
        "degT": deg_t,
    }


def build_bass(cfg: Cfg):
    from contextlib import ExitStack
    nc = bacc.Bacc("TRN2", target_bir_lowering=False, debug=False,
                   num_swdge_queues=4)
    with tile.TileContext(nc) as tc:
        with ExitStack() as ctx:
            build_core_program(ctx, tc, cfg)
    nc.compile()
    return nc


_CFG = Cfg()
_NC_CACHE = {}
TRACE = False
LAST_RESULT = None


def kernel(x, edge_index, batch, W1, b1, bn_gamma, bn_beta, bn_mean, bn_var,
           W_lin1, b_lin1, p1, Wl, Wr, att, b_gat, p2,
           W_lin2, b_lin2, W_lin3, b_lin3):
    cfg = _CFG
    n_cores = 8
    s_att = float(np.sum(np.asarray(att, dtype=np.float64)))
    assert abs(s_att) > 1e-6, "degenerate att sum; poison scheme needs |sum(att)|>0"
    cfg.psign = -1.0 if s_att > 0 else 1.0
    slope = 0.2 if s_att > 0 else 1.0
    cfg.pb_mag = 40.0 / (slope * abs(s_att))
    weights = _prep_weights(cfg, W1, b1, bn_gamma, bn_beta, bn_mean, bn_var,
                            W_lin1, b_lin1, p1, Wl, Wr, att, b_gat, p2,
                            W_lin2, b_lin2, W_lin3, b_lin3)
    src_all = np.asarray(edge_index[0], dtype=np.int64)
    dst_all = np.asarray(edge_index[1], dtype=np.int64)
    x = np.asarray(x, dtype=np.float32)

    # choose the chunks-per-db-run capacity from the data (global max so the
    # single SPMD program fits every core)
    max_run = 0
    for c in range(n_cores):
        for g in range(cfg.ng):
            e0 = c * cfg.ne + g * cfg.eg
            d = dst_all[e0 : e0 + cfg.eg] - (c * cfg.nn + g * cfg.npg)
            cnts = np.bincount(d >> 7, minlength=cfg.tj) + P  # + self loops
            max_run = max(max_run, int(cnts.max()))
    cfg.spd = (max_run + P - 1) // P
    # windows of ch slots must tile a graph's slot range exactly
    while (cfg.tj * cfg.spd * P) % cfg.ch != 0:
        cfg.spd += 1

    in_maps = []
    for c in range(n_cores):
        n0 = c * cfg.nn
        e0 = c * cfg.ne
        d = dict(weights)
        d.update(
            _prep_core_edges(
                cfg, src_all[e0 : e0 + cfg.ne] - n0, dst_all[e0 : e0 + cfg.ne] - n0
            )
        )
        d["xT"] = np.ascontiguousarray(x[n0 : n0 + cfg.nn].T, np.float32)
        in_maps.append(d)

    key = ("nc", cfg.spd, cfg.psign, cfg.pb_mag)
    if key not in _NC_CACHE:
        _NC_CACHE[key] = build_bass(cfg)
    nc = _NC_CACHE[key]
    global LAST_RESULT
    res = run_bass_kernel_spmd(nc, in_maps, core_ids=list(range(n_cores)), trace=TRACE)
    LAST_RESULT = res
    outs = [np.asarray(res.results[c]["out"]).reshape(cfg.ng, 1) for c in range(n_cores)]
    return np.concatenate(outs, axis=0).astype(np.float32)


# revision 41
# speedup vs baseline: 1.2599x; 1.0803x over previous
"""Trainium2 Bass kernel for nn_AGNN_EFG (GCN -> TopK pool -> GATv2 -> TopK pool -> head).

Self-contained: shards the B=64 graphs across 8 NeuronCores (8 graphs/core),
runs one SPMD Bass program, gathers the [64, 1] head output on host.

v4 design:
- Edges (incl self loops) sorted per graph by dst block (db = dst>>7), each
  db run padded to a fixed spd chunks of 128 -> static chunk->db schedule
  (SPMD-safe; spd chosen on host from the data).
- Scatter-adds are ONE-HOT MATMULS on PE: per chunk a [128e,128d] bf16
  one-hot (DVE is_equal; pad slots carry sentinel 255 -> zero rows)
  accumulates messages into per-graph PSUM. No dma_scatter_add.
- GAT's xr[dst] is a [128d,128e]-orientation one-hot matmul (one-hot built
  on the Scalar engine as relu(1-(dlo-d)^2)) reading xr blocks from SBUF.
- Only 2 indirect passes remain (GpSimd Q7 descriptor generation is the
  machine bottleneck): gather u[src] (GCN) and [xl|xr][src] (GAT).
- Fully per-graph pipelined: utab(g+2) build, dense/pool/gtab(g),
  gcn(g+1) and gat(g) interleave so the GpSimd gather stream never idles.
- GCN u rows split [u_hi|u_lo] bf16 (~f32 accuracy, one 128-wide matmul).
- score1 fused into the lin1 matmul via host-folded [W_lin1 | W_lin1@p1n].
- Degrees are host-precomputed index data (bincount of dst); rsqrt on device.
"""

import sys

sys.path.insert(0, "/opt/trn_rl_repo")

from dataclasses import dataclass

import numpy as np
import ml_dtypes

import concourse.bass as bass
import concourse.mybir as mybir
import concourse.tile as tile
from concourse import bacc
from concourse.bass_utils import run_bass_kernel_spmd
from concourse.masks import make_identity

P = 128
F32 = mybir.dt.float32
BF16 = mybir.dt.bfloat16
I16 = mybir.dt.int16
U8 = mybir.dt.uint8
AF = mybir.ActivationFunctionType
OP = mybir.AluOpType
AX = mybir.AxisListType


@dataclass
class Cfg:
    ng: int = 8          # graphs per core
    npg: int = 2048      # nodes per graph
    hid: int = 64        # feature dim
    eg: int = 32768      # edges per graph (original, without self loops)
    spd: int = 19        # chunks (of 128 slots) per dst-block run; set at runtime
    ch: int = 1024       # gather window (slots per dma_gather call; >=2048
                         # overflows the SWDGE descriptor ring and faults)
    n_bisect: int = 30   # bisection iterations for topk threshold
    psign: float = -1.0  # -sign(sum(att)): poison sign so poisoned e < 0
    pb_mag: float = 200.0  # poison magnitude; set so poisoned e ~ -40 (LUT-safe)

    @property
    def nn(self):
        return self.ng * self.npg

    @property
    def ne(self):
        return self.ng * self.eg

    @property
    def tj(self):
        return self.npg // P  # dst blocks per graph (16)

    @property
    def nt(self):
        return self.ng * self.tj  # 128

    @property
    def tch(self):
        return self.tj * self.spd  # chunks per graph

    @property
    def slots_g(self):
        return self.tch * P  # padded edge slots per graph

    @property
    def slots(self):
        return self.ng * self.slots_g

    @property
    def wpg(self):
        assert self.slots_g % self.ch == 0
        return self.slots_g // self.ch  # gather windows per graph

    @property
    def cpw(self):
        return self.ch // P  # chunks per window

    @property
    def k1(self):
        return self.npg // 2

    @property
    def k2(self):
        return self.npg // 4


def build_core_program(ctx, tc, cfg: Cfg):
    nc = tc.nc
    NG, NPG, HID, NN = cfg.ng, cfg.npg, cfg.hid, cfg.nn
    NT, TJ, SPD = cfg.nt, cfg.tj, cfg.spd
    TCH, WPG, CPW, CH = cfg.tch, cfg.wpg, cfg.cpw, cfg.ch
    SRUN = SPD * P  # slots per db run
    assert HID == 64 and CPW % 8 == 0

    # ---- I/O ----
    xT = nc.dram_tensor("xT", [HID, NN], F32, kind="ExternalInput").ap()
    srcw = nc.dram_tensor("srcw", [P, cfg.slots // 16], I16, kind="ExternalInput").ap()
    dlo_pm = nc.dram_tensor("dlo_pm", [P, NG * TCH], BF16, kind="ExternalInput").ap()
    dlo_fr = nc.dram_tensor("dlo_fr", [NG * TJ * SRUN], BF16, kind="ExternalInput").ap()
    degT = nc.dram_tensor("degT", [P, NT], F32, kind="ExternalInput").ap()
    w_names = ["Wl", "Wr"]
    Wd = {n: nc.dram_tensor(n, [HID, HID], F32, kind="ExternalInput").ap() for n in w_names}
    W1d = nc.dram_tensor("W1dup", [HID, P], F32, kind="ExternalInput").ap()
    W1p = nc.dram_tensor("Wlin1p", [HID, HID + 1], F32, kind="ExternalInput").ap()
    dinvF = nc.dram_tensor("dinvF", [NN], F32, kind="ExternalInput").ap()
    b1c = nc.dram_tensor("b_lin1c", [HID + 1], F32, kind="ExternalInput").ap()
    v_names = ["bn_a", "bn_bp", "att", "b_gat", "p2", "W23"]
    Vd = {n: nc.dram_tensor(n, [HID], F32, kind="ExternalInput").ap() for n in v_names}
    Cd = nc.dram_tensor("Cc", [1], F32, kind="ExternalInput").ap()
    out_d = nc.dram_tensor("out", [NG, 1], F32, kind="ExternalOutput").ap()

    # ---- DRAM scratch (per-graph tables so gathers only depend on their
    # own graph's writes) ----
    utab = [nc.dram_tensor(f"utab{g}", [NPG, P], BF16).ap() for g in range(NG)]
    gtab = [nc.dram_tensor(f"gtab{g}", [NPG, P], BF16).ap() for g in range(NG)]
    xlr_dram = nc.dram_tensor("xlr_dram", [P, NN], BF16).ap()
    ufm_dram = nc.dram_tensor("ufm_dram", [P, NN], BF16).ap()
    sc_dram = nc.dram_tensor("sc_dram", [NN], F32).ap()

    cpool = ctx.enter_context(tc.tile_pool(name="consts", bufs=1))
    mpool = ctx.enter_context(tc.tile_pool(name="main", bufs=1))
    smallps = ctx.enter_context(tc.tile_pool(name="smallps", bufs=1, space="PSUM"))

    # ---- constants ----
    ident = cpool.tile([P, P], F32)
    make_identity(nc, ident[:])
    ones128 = cpool.tile([P, P], F32)
    nc.vector.memset(ones128[:], 1.0)
    nantile = cpool.tile([P, NT], F32)
    nc.vector.memset(nantile[:], float("nan"))
    negbig = cpool.tile([P, NT], F32)
    nc.vector.memset(negbig[:], -1e9)
    io16 = cpool.tile([P, P], I16)
    nc.gpsimd.iota(io16[:], pattern=[[1, P]], base=0, channel_multiplier=0)
    iota_bf = cpool.tile([P, P], BF16)
    nc.vector.tensor_copy(out=iota_bf[:], in_=io16[:])
    ioc16 = cpool.tile([P, 1], I16)
    nc.gpsimd.iota(ioc16[:], pattern=[[0, 1]], base=0, channel_multiplier=1)
    niota_col = cpool.tile([P, 1], F32)
    nc.vector.tensor_scalar(
        out=niota_col[:], in0=ioc16[:], scalar1=-1.0, scalar2=None, op0=OP.mult
    )

    Ws = {}
    for n in w_names:
        t = cpool.tile([HID, HID], F32, tag=f"w_{n}")
        nc.sync.dma_start(out=t[:], in_=Wd[n][:])
        Ws[n] = t
    W1p_sb = cpool.tile([HID, HID + 1], F32, tag="w_Wlin1p")
    nc.sync.dma_start(out=W1p_sb[:], in_=W1p[:])
    W1d_sb = cpool.tile([HID, P], F32, tag="w_W1dup")
    nc.sync.dma_start(out=W1d_sb[:], in_=W1d[:])
    b1c_sb = cpool.tile([HID + 1, 1], F32, tag="v_b1c")
    nc.sync.dma_start(out=b1c_sb[:], in_=b1c[:, None])
    Vs = {}
    for n in v_names:
        t = cpool.tile([HID, 1], F32, tag=f"v_{n}")
        nc.sync.dma_start(out=t[:], in_=Vd[n][:, None])
        Vs[n] = t
    att_rep = cpool.tile([P, HID], BF16)
    nc.gpsimd.dma_start(out=att_rep[:], in_=Vd["att"][None, :].to_broadcast([P, HID]))
    p2_rep = cpool.tile([P, HID], F32)
    nc.sync.dma_start(out=p2_rep[:], in_=Vd["p2"][None, :].to_broadcast([P, HID]))
    bgat_rep = cpool.tile([P, HID], F32)
    nc.sync.dma_start(out=bgat_rep[:], in_=Vd["b_gat"][None, :].to_broadcast([P, HID]))
    Cc_sb = cpool.tile([NG, 1], F32)
    nc.sync.dma_start(out=Cc_sb[:], in_=Cd[None, :].to_broadcast([NG, 1]))

    # whole per-chunk dst-low-bit table (for one-hot builds in [e,d] orientation)
    dlo_sb = cpool.tile([P, NG * TCH], BF16)
    nc.sync.dma_start(out=dlo_sb[:], in_=dlo_pm[:])

    # ---- dinv from host degree counts ----
    dinv_t = mpool.tile([P, NT], F32, tag="dinv_t")
    sqd_t = mpool.tile([P, NT], F32, tag="sqd_t")
    ntmp = mpool.tile([P, NT], F32, tag="ntmp")

    def recip_newton(r_ap, x_ap, tmp_ap):
        nc.vector.tensor_tensor(out=tmp_ap, in0=x_ap, in1=r_ap, op=OP.mult)
        nc.vector.tensor_scalar(
            out=tmp_ap, in0=tmp_ap, scalar1=-1.0, scalar2=2.0, op0=OP.mult, op1=OP.add
        )
        nc.vector.tensor_tensor(out=r_ap, in0=r_ap, in1=tmp_ap, op=OP.mult)

    deg_sb = mpool.tile([P, NT], F32, tag="deg_sb")
    nc.sync.dma_start(out=deg_sb[:], in_=degT[:])
    nc.scalar.sqrt(out=sqd_t[:], in_=deg_sb[:])
    nc.vector.reciprocal(out=dinv_t[:], in_=sqd_t[:])
    recip_newton(dinv_t[:], sqd_t[:], ntmp[:])

    def idx_slice(pool, w_global, tag):
        t = pool.tile([P, CH // 16], I16, tag=tag)
        c0 = w_global * (CH // 16)
        nc.sync.dma_start(out=t[:], in_=srcw[:, c0 : c0 + CH // 16])
        return t

    # ---- pools (all phases interleave; PSUM budget: 2+1+2+2+1 = 8 banks) ----
    ups = ctx.enter_context(tc.tile_pool(name="ups", bufs=2))
    ubc = ctx.enter_context(tc.tile_pool(name="ubc", bufs=1))
    gep = ctx.enter_context(tc.tile_pool(name="gep", bufs=5))
    ohp = ctx.enter_context(tc.tile_pool(name="ohp", bufs=2))
    hps_pool = ctx.enter_context(tc.tile_pool(name="hpsp", bufs=1, space="PSUM"))
    dsa = ctx.enter_context(tc.tile_pool(name="dsa", bufs=1))
    dss = ctx.enter_context(tc.tile_pool(name="dss", bufs=2))
    bis = ctx.enter_context(tc.tile_pool(name="bis", bufs=2))
    gap = ctx.enter_context(tc.tile_pool(name="gap", bufs=3))
    aohp = ctx.enter_context(tc.tile_pool(name="aohp", bufs=2))
    runtmp = ctx.enter_context(tc.tile_pool(name="runtmp", bufs=1))
    runp = ctx.enter_context(tc.tile_pool(name="runp", bufs=2))
    gatps = ctx.enter_context(tc.tile_pool(name="gatps", bufs=1, space="PSUM"))
    mxrp = ctx.enter_context(tc.tile_pool(name="mxrp", bufs=2, space="PSUM"))
    gfin = ctx.enter_context(tc.tile_pool(name="gfin", bufs=1))

    # persistent t-space tiles
    h2t = mpool.tile([P, NT, HID], F32, tag="bigA")
    gstage = mpool.tile([P, NT, P], BF16, tag="bigC")
    score1_t = mpool.tile([P, NT], F32, tag="score1_t")
    tanh1 = mpool.tile([P, NT], F32, tag="tanh1")
    gate1 = mpool.tile([P, NT], F32, tag="gate1")
    gate1z = mpool.tile([P, NT], F32, tag="gate1z")
    padd = mpool.tile([P, NT], F32, tag="padd")
    kept1 = mpool.tile([P, NT], U8, tag="kept1")
    pb = cfg.psign * cfg.pb_mag

    hps_tiles = {}
    nmps_tiles = {}

    # ======== per-graph sections ========

    def utab_build(g):
        # xw (duplicated rows) -> dinv scale -> [hi|lo] bf16 split, all in
        # feature-major; node-major utab rows via one transpose DMA hop.
        xw2 = ups.tile([P, NPG], F32, tag="xw2", name=f"xw2_{g}")
        for jl in range(TJ):
            xTg = ups.tile([HID, P], F32, tag="xTg", name=f"xTg{g}_{jl}")
            nc.sync.dma_start(
                out=xTg[:], in_=xT[:, g * NPG + jl * P : g * NPG + (jl + 1) * P]
            )
            pm = smallps.tile([P, P], F32, tag="smA", name=f"xwps{g}_{jl}")
            nc.tensor.matmul(pm[:], lhsT=W1d_sb[:], rhs=xTg[:], start=True, stop=True)
            nc.scalar.copy(out=xw2[:, jl * P : (jl + 1) * P], in_=pm[:])
        dbc = ubc.tile([P, NPG], F32, tag="dinvbc", name=f"dinvbc{g}")
        nc.sync.dma_start(
            out=dbc[:], in_=dinvF[g * NPG : (g + 1) * NPG][None, :].to_broadcast([P, NPG])
        )
        nc.vector.tensor_tensor(out=xw2[:], in0=xw2[:], in1=dbc[:], op=OP.mult)
        u2 = ups.tile([P, NPG], BF16, tag="u2", name=f"u2_{g}")
        nc.vector.tensor_copy(out=u2[0:HID, :], in_=xw2[0:HID, :])
        nc.vector.tensor_copy(out=u2[HID:P, :], in_=xw2[HID:P, :])
        nc.vector.tensor_tensor(
            out=u2[HID:P, :], in0=xw2[HID:P, :], in1=u2[HID:P, :], op=OP.subtract
        )
        nc.sync.dma_start(out=ufm_dram[:, g * NPG : (g + 1) * NPG], in_=u2[:])
        u_nm = ups.tile([P, TJ, P], BF16, tag="u2", name=f"unm{g}")
        nc.sync.dma_start_transpose(
            out=u_nm[:], in_=ufm_dram[:, g * NPG : (g + 1) * NPG]
        )
        nc.sync.dma_start(
            out=utab[g].rearrange("(j p) f -> p j f", p=P), in_=u_nm[:]
        )

    def gcn_pass(g):
        hps = [
            hps_pool.tile([P, 8, HID], F32, tag=f"hps{t}", name=f"hps{t}_{g}")
            for t in range(2)
        ]
        hps_tiles[g] = hps
        for w in range(WPG):
            wg = g * WPG + w
            ssl = idx_slice(gep, wg, "ssl")
            ub = gep.tile([P, CPW, P], BF16, tag="ub")
            nc.gpsimd.dma_gather(
                out_ap=ub[:], in_ap=utab[g][:], idxs_ap=ssl[:],
                num_idxs=CH, num_idxs_reg=CH, elem_size=P, queue_num=wg % 4,
            )
            for b in range(CPW // 8):
                oh8 = ohp.tile([P, 8, P], BF16, tag="oh8")
                c0 = g * TCH + w * CPW + b * 8
                nc.vector.tensor_tensor(
                    out=oh8[:],
                    in0=iota_bf[:, None, :].to_broadcast([P, 8, P]),
                    in1=dlo_sb[:, c0 : c0 + 8, None].to_broadcast([P, 8, P]),
                    op=OP.is_equal,
                )
                for cl in range(8):
                    c = b * 8 + cl
                    gc = w * CPW + c
                    db, pos = gc // SPD, gc % SPD
                    out_slc = hps[db // 8][:, db % 8, :]
                    nc.tensor.matmul(
                        out_slc, lhsT=oh8[:, cl, :], rhs=ub[:, c, 0:HID],
                        start=(pos == 0), stop=False,
                    )
                    nc.tensor.matmul(
                        out_slc, lhsT=oh8[:, cl, :], rhs=ub[:, c, HID:P],
                        start=False, stop=(pos == SPD - 1),
                    )

    S6C = 512

    def dense_g(g):
        # h = dinv*(hi+lo); BN+leaky (one ACT Lrelu); lin1(+score); xl/xr
        hps = hps_tiles.pop(g)
        gsl = slice(g * TJ, (g + 1) * TJ)
        hsum = dsa.tile([P, TJ, HID], F32, tag="hsum", name=f"hsum{g}")
        for t in range(2):
            nc.vector.tensor_tensor(
                out=hsum[:, t * 8 : (t + 1) * 8, :], in0=hps[t][:],
                in1=dinv_t[:, g * TJ + t * 8 : g * TJ + (t + 1) * 8, None].to_broadcast(
                    [P, 8, HID]
                ),
                op=OP.mult,
            )
        hfm = dsa.tile([HID, NPG], F32, tag="hfm", name=f"hfm{g}")
        for jl in range(TJ):
            pt = smallps.tile([HID, P], F32, tag="smA", name=f"htr{g}_{jl}")
            nc.tensor.transpose(out=pt[:], in_=hsum[:, jl, :], identity=ident[:])
            nc.scalar.copy(out=hfm[:, jl * P : (jl + 1) * P], in_=pt[:])
        nc.scalar.activation(
            out=hfm[:], in_=hfm[:], func=AF.Lrelu, scale=Vs["bn_a"][:],
            bias=Vs["bn_bp"][:], alpha=0.01,
        )
        xlr_g = dss.tile([P, NPG], BF16, tag="xlrg", name=f"xlrg{g}")
        for ol in range(0, NPG, S6C):
            o = g * NPG + ol
            pm = hps_pool.tile([HID + 1, S6C], F32, tag="hps0", name=f"l1ps{o}")
            nc.tensor.matmul(
                pm[:], lhsT=W1p_sb[:], rhs=hfm[:, ol : ol + S6C],
                start=True, stop=True,
            )
            hc = dss.tile([HID + 1, S6C], F32, tag="hc", name=f"hc{o}")
            nc.scalar.activation(
                out=hc[:], in_=pm[:], func=AF.Identity, bias=b1c_sb[:]
            )
            nc.sync.dma_start(out=sc_dram[None, o : o + S6C], in_=hc[HID : HID + 1, :])
            px = hps_pool.tile([P, S6C], F32, tag="hps1", name=f"xlrps{o}")
            nc.tensor.matmul(px[:HID, :], lhsT=Ws["Wl"][:], rhs=hc[0:HID, :], start=True, stop=True)
            nc.tensor.matmul(px[HID:, :], lhsT=Ws["Wr"][:], rhs=hc[0:HID, :], start=True, stop=True)
            nc.scalar.copy(out=xlr_g[:, ol : ol + S6C], in_=px[:])
        nc.sync.dma_start(out=xlr_dram[:, g * NPG : (g + 1) * NPG], in_=xlr_g[:])

    def bisect_multi(score_slc, ngr, target, tag):
        # score_slc: [P, ngr*TJ]; returns per-graph thresholds lo [P, ngr]
        lo = bis.tile([P, ngr], F32, tag="lo", name=f"lo_{tag}")
        hi = bis.tile([P, ngr], F32, tag="hi", name=f"hi_{tag}")
        mid = bis.tile([P, ngr], F32, tag="mid", name=f"mid_{tag}")
        cmp = bis.tile([P, ngr * TJ], F32, tag="cmp", name=f"cmp_{tag}")
        cred = bis.tile([P, ngr], F32, tag="cred", name=f"cred_{tag}")
        ge = bis.tile([P, ngr], U8, tag="ge", name=f"ge_{tag}")
        lt = bis.tile([P, ngr], U8, tag="lt", name=f"lt_{tag}")
        nc.vector.memset(lo[:], -64.0)
        nc.vector.memset(hi[:], 64.0)
        sc_g = score_slc.rearrange("p (g t) -> p g t", g=ngr)
        cmp_g = cmp[:].rearrange("p (g t) -> p g t", g=ngr)
        for it in range(cfg.n_bisect):
            nc.vector.tensor_tensor(out=mid[:], in0=lo[:], in1=hi[:], op=OP.add)
            nc.vector.tensor_scalar(
                out=mid[:], in0=mid[:], scalar1=0.5, scalar2=None, op0=OP.mult
            )
            nc.vector.tensor_tensor(
                out=cmp_g, in0=sc_g,
                in1=mid[:, :, None].to_broadcast([P, ngr, TJ]), op=OP.is_gt,
            )
            nc.vector.tensor_reduce(out=cred[:], in_=cmp_g, axis=AX.X, op=OP.add)
            cps = smallps.tile([P, ngr], F32, tag="smB", name=f"cnt_{tag}_{it}")
            nc.tensor.matmul(cps[:], lhsT=ones128[:], rhs=cred[:], start=True, stop=True)
            nc.vector.tensor_scalar(
                out=ge[:], in0=cps[:], scalar1=float(target), scalar2=None, op0=OP.is_ge
            )
            nc.vector.tensor_scalar(
                out=lt[:], in0=cps[:], scalar1=float(target), scalar2=None, op0=OP.is_lt
            )
            nc.vector.copy_predicated(out=lo[:], mask=ge[:], data=mid[:])
            nc.vector.copy_predicated(out=hi[:], mask=lt[:], data=mid[:])
        return lo

    NH = NG // 2  # graphs per pooling half

    def pool1_h(h):
        g0 = h * NH
        gsl = slice(g0 * TJ, (g0 + NH) * TJ)
        nc.sync.dma_start(
            out=score1_t[:, gsl],
            in_=sc_dram[g0 * NPG : (g0 + NH) * NPG].rearrange("(j p) -> p j", p=P),
        )
        t1 = bisect_multi(score1_t[:, gsl], NH, cfg.k1, f"p1h{h}")
        nc.vector.tensor_tensor(
            out=kept1[:, gsl].rearrange("p (g t) -> p g t", g=NH),
            in0=score1_t[:, gsl].rearrange("p (g t) -> p g t", g=NH),
            in1=t1[:, :, None].to_broadcast([P, NH, TJ]), op=OP.is_gt,
        )
        nc.scalar.activation(out=tanh1[:, gsl], in_=score1_t[:, gsl], func=AF.Tanh)
        nc.vector.tensor_copy(out=gate1[:, gsl], in_=nantile[:, gsl])
        nc.vector.copy_predicated(out=gate1[:, gsl], mask=kept1[:, gsl], data=tanh1[:, gsl])
        nc.vector.memset(gate1z[:, gsl], 0.0)
        nc.vector.copy_predicated(out=gate1z[:, gsl], mask=kept1[:, gsl], data=tanh1[:, gsl])
        nc.vector.tensor_scalar(
            out=padd[:, gsl], in0=kept1[:, gsl], scalar1=-pb, scalar2=pb,
            op0=OP.mult, op1=OP.add,
        )
        # gtab for the half
        gs = gstage[:, gsl, :]
        nc.sync.dma_start_transpose(
            out=gs, in_=xlr_dram[:, g0 * NPG : (g0 + NH) * NPG]
        )
        nc.vector.tensor_tensor(
            out=gs, in0=gs,
            in1=gate1z[:, gsl, None].to_broadcast([P, NH * TJ, P]), op=OP.mult,
        )
        nc.vector.tensor_tensor(
            out=gs, in0=gs,
            in1=padd[:, gsl, None].to_broadcast([P, NH * TJ, P]), op=OP.add,
        )
        for gg in range(g0, g0 + NH):
            nc.sync.dma_start(
                out=gtab[gg].rearrange("(j p) f -> p j f", p=P),
                in_=gstage[:, gg * TJ : (gg + 1) * TJ, :],
            )

    def gat_pass(g):
        nmps = [
            gatps.tile([P, 8, HID], F32, tag=f"nmps{t}", name=f"nmps{t}_{g}")
            for t in range(2)
        ]
        dnps = smallps.tile([P, TJ], F32, tag="smB", name=f"dnps{g}")
        oh_de = {}
        for w in range(WPG):
            wg = g * WPG + w
            ssl = idx_slice(gap, wg, "assl")
            gx = gap.tile([P, CPW, P], BF16, tag="gx")
            nc.gpsimd.dma_gather(
                out_ap=gx[:], in_ap=gtab[g][:], idxs_ap=ssl[:],
                num_idxs=CH, num_idxs_reg=CH, elem_size=P, queue_num=wg % 4,
            )
            for b in range(CPW // 8):
                oh8 = aohp.tile([P, 8, P], BF16, tag="aoh8")
                c0 = g * TCH + w * CPW + b * 8
                nc.vector.tensor_tensor(
                    out=oh8[:],
                    in0=iota_bf[:, None, :].to_broadcast([P, 8, P]),
                    in1=dlo_sb[:, c0 : c0 + 8, None].to_broadcast([P, 8, P]),
                    op=OP.is_equal,
                )
                mxr = mxrp.tile([P, 8, HID], F32, tag="mxr", name=f"mxr{wg}_{b}")
                for cl in range(8):
                    c = b * 8 + cl
                    gc = w * CPW + c
                    r, pos = gc // SPD, gc % SPD
                    if pos == 0:
                        dlo_bc = runtmp.tile(
                            [P, SRUN], BF16, tag="dlobc", name=f"dlobc{g}_{r}"
                        )
                        o = (g * TJ + r) * SRUN
                        nc.sync.dma_start(
                            out=dlo_bc[:],
                            in_=dlo_fr[o : o + SRUN][None, :].to_broadcast([P, SRUN]),
                        )
                        # one-hot on the Scalar engine: relu(1 - (dlo - d)^2)
                        ohsq = runtmp.tile([P, SRUN], BF16, tag="ohsq", name=f"ohsq{g}_{r}")
                        nc.scalar.activation(
                            out=ohsq[:], in_=dlo_bc[:], func=AF.Square,
                            bias=niota_col[:],
                        )
                        ohr = runp.tile([P, SRUN], BF16, tag="ohde", name=f"ohde{g}_{r}")
                        nc.scalar.activation(
                            out=ohr[:], in_=ohsq[:], func=AF.Relu,
                            bias=1.0, scale=-1.0,
                        )
                        oh_de[r] = ohr
                    nc.tensor.matmul(
                        mxr[:, cl, :],
                        lhsT=oh_de[r][:, pos * P : (pos + 1) * P],
                        rhs=gstage[:, g * TJ + r, HID:P],
                        start=True, stop=True,
                    )
                # e = att . leaky(xl_s + xr_d); w = exp(e); pay = w*xl
                gxs = gx[:, b * 8 : (b + 1) * 8, :]
                z = gap.tile([P, 8, HID], BF16, tag="z")
                nc.vector.tensor_tensor(
                    out=z[:], in0=gxs[:, :, 0:HID], in1=mxr[:], op=OP.add
                )
                nc.vector.scalar_tensor_tensor(
                    out=z[:], in0=z[:], scalar=0.2, in1=z[:], op0=OP.mult, op1=OP.max,
                )
                nc.vector.tensor_tensor(
                    out=z[:], in0=z[:],
                    in1=att_rep[:, None, :].to_broadcast([P, 8, HID]), op=OP.mult,
                )
                e8 = gap.tile([P, 8], F32, tag="e8")
                nc.vector.tensor_reduce(out=e8[:], in_=z[:], axis=AX.X, op=OP.add)
                w8b = gap.tile([P, 8], BF16, tag="w8b")
                nc.scalar.activation(out=w8b[:], in_=e8[:], func=AF.Exp)
                pay = gap.tile([P, 8, HID], BF16, tag="pay")
                nc.vector.tensor_tensor(
                    out=pay[:], in0=gxs[:, :, 0:HID],
                    in1=w8b[:, :, None].to_broadcast([P, 8, HID]), op=OP.mult,
                )
                for cl in range(8):
                    c = b * 8 + cl
                    gc = w * CPW + c
                    db, pos = gc // SPD, gc % SPD
                    nc.tensor.matmul(
                        nmps[db // 8][:, db % 8, :],
                        lhsT=oh8[:, cl, :], rhs=pay[:, cl, :],
                        start=(pos == 0), stop=(pos == SPD - 1),
                    )
                    nc.tensor.matmul(
                        dnps[:, db : db + 1],
                        lhsT=oh8[:, cl, :], rhs=w8b[:, cl : cl + 1],
                        start=(pos == 0), stop=(pos == SPD - 1),
                    )

        # ---- finalize graph g: h2 = leaky(numer/denom + b_gat) ----
        numsb = dsa.tile([P, TJ, HID], F32, tag="hsum", name=f"numsb{g}")
        nc.scalar.copy(out=numsb[:, 0:8, :], in_=nmps[0][:])
        nc.scalar.copy(out=numsb[:, 8:TJ, :], in_=nmps[1][:])
        den = gfin.tile([P, TJ], F32, tag="den", name=f"den{g}")
        rec = gfin.tile([P, TJ], F32, tag="rec", name=f"rec{g}")
        dtmp = gfin.tile([P, TJ], F32, tag="dtmp", name=f"dtmp{g}")
        nc.vector.tensor_scalar(
            out=den[:], in0=dnps[:], scalar1=1e-16, scalar2=None, op0=OP.add
        )
        nc.vector.reciprocal(out=rec[:], in_=den[:])
        recip_newton(rec[:], den[:], dtmp[:])
        hslc = h2t[:, g * TJ : (g + 1) * TJ, :]
        nc.vector.tensor_tensor(
            out=hslc, in0=numsb[:],
            in1=rec[:, :, None].to_broadcast([P, TJ, HID]), op=OP.mult,
        )
        nc.vector.tensor_tensor(
            out=hslc, in0=hslc,
            in1=bgat_rep[:, None, :].to_broadcast([P, TJ, HID]), op=OP.add,
        )
        nc.vector.scalar_tensor_tensor(
            out=hslc, in0=hslc, scalar=0.01, in1=hslc, op0=OP.mult, op1=OP.max
        )

    # ======== emission: gcn stream, pools, gat stream ========
    utab_build(0)
    utab_build(1)
    for g in range(NG):
        gcn_pass(g)
        dense_g(g)
        if g + 2 < NG:
            utab_build(g + 2)
    pool1_h(0)
    pool1_h(1)
    for g in range(NG):
        gat_pass(g)

    # ======== score2 (t-space, blocked), mask to kept1 ========
    score2_t = mpool.tile([P, NT], F32, tag="score2_t")
    for t in range(4):
        tsl = slice(t * 32, (t + 1) * 32)
        blk = ups.tile([P, 32, HID], F32, tag="xTg", name=f"s2blk{t}")
        nc.vector.tensor_tensor(
            out=blk[:], in0=h2t[:, tsl, :],
            in1=p2_rep[:, None, :].to_broadcast([P, 32, HID]), op=OP.mult,
        )
        nc.vector.tensor_reduce(out=score2_t[:, tsl], in_=blk[:], axis=AX.X, op=OP.add)
    kept1_t = mpool.tile([P, NT], U8, tag="kept1_t")
    nc.vector.tensor_tensor(out=kept1_t[:], in0=gate1[:], in1=gate1[:], op=OP.is_equal)
    sc2m = mpool.tile([P, NT], F32, tag="sc2m")
    nc.vector.tensor_copy(out=sc2m[:], in_=negbig[:])
    nc.vector.copy_predicated(out=sc2m[:], mask=kept1_t[:], data=score2_t[:])

    # ======== pool2 threshold + gate2 = tanh * mask ========
    gate2 = mpool.tile([P, NT], F32, tag="gate2")
    t2 = bisect_multi(sc2m[:], NG, cfg.k2, "p2")
    nc.vector.tensor_tensor(
        out=gate2[:].rearrange("p (g t) -> p g t", g=NG),
        in0=sc2m[:].rearrange("p (g t) -> p g t", g=NG),
        in1=t2[:, :, None].to_broadcast([P, NG, TJ]), op=OP.is_gt,
    )
    tanh2 = mpool.tile([P, NT], F32, tag="tanh2")
    sc2c = mpool.tile([P, NT], F32, tag="sc2c")
    nc.vector.tensor_scalar(
        out=sc2c[:], in0=sc2m[:], scalar1=-64.0, scalar2=None, op0=OP.max
    )
    nc.scalar.activation(out=tanh2[:], in_=sc2c[:], func=AF.Tanh)
    nc.vector.tensor_tensor(out=gate2[:], in0=gate2[:], in1=tanh2[:], op=OP.mult)

    # ======== T_g = sum_n gate2[n] * h2[n]; out = T @ W23 + C ========
    Tps = smallps.tile([P, NG], F32, tag="smB")
    for j in range(NT):
        g = j // TJ
        nc.tensor.matmul(
            Tps[:HID, g : g + 1], lhsT=h2t[:, j, :], rhs=gate2[:, j : j + 1],
            start=(j % TJ == 0), stop=(j % TJ == TJ - 1),
        )
    Tsb = mpool.tile([HID, NG], F32, tag="Tsb")
    nc.scalar.copy(out=Tsb[:], in_=Tps[:HID, :])
    hps2 = smallps.tile([NG, 1], F32, tag="smB")
    nc.tensor.matmul(hps2[:], lhsT=Tsb[:], rhs=Vs["W23"][:], start=True, stop=True)
    outsb = mpool.tile([NG, 1], F32, tag="outsb")
    nc.vector.tensor_tensor(out=outsb[:], in0=hps2[:], in1=Cc_sb[:], op=OP.add)
    nc.sync.dma_start(out=out_d[:], in_=outsb[:])


# ================= host side =================

def _wrap_idx(ix: np.ndarray) -> np.ndarray:
    n = ix.shape[0]
    w = ix.reshape(n // 16, 16).T.astype(np.int16)
    return np.tile(w, (8, 1)).copy()


def _prep_weights(cfg, W1, b1, bn_gamma, bn_beta, bn_mean, bn_var, W_lin1, b_lin1,
                  p1, Wl, Wr, att, b_gat, p2, W_lin2, b_lin2, W_lin3, b_lin3):
    f32 = np.float32
    bn_a = (bn_gamma / np.sqrt(bn_var + 1e-5)).astype(f32)
    bn_b = (bn_beta - bn_mean * bn_a).astype(f32)
    W23 = (W_lin2 @ W_lin3).reshape(-1).astype(f32)
    Cc = np.array([cfg.k2 * float(b_lin2 @ W_lin3[:, 0]) + float(b_lin3[0])], dtype=f32)
    p1n = (np.asarray(p1) / np.linalg.norm(np.asarray(p1))).astype(np.float64)
    Wlin1p = np.concatenate(
        [np.asarray(W_lin1, np.float64),
         (np.asarray(W_lin1, np.float64) @ p1n)[:, None]], axis=1
    ).astype(f32)
    c1 = np.array([float(p1n @ np.asarray(b_lin1, np.float64))], dtype=f32)
    return {
        "W1dup": np.ascontiguousarray(
            np.concatenate([np.asarray(W1, f32)] * 2, axis=1)
        ),
        "Wlin1p": Wlin1p,
        "Wl": np.ascontiguousarray(Wl, f32), "Wr": np.ascontiguousarray(Wr, f32),
        "bn_a": bn_a, "bn_bp": (np.asarray(b1, f32) * bn_a + bn_b).astype(f32),
        "b_lin1c": np.concatenate([np.asarray(b_lin1, f32), c1]),
        "att": np.ascontiguousarray(att, f32), "b_gat": np.ascontiguousarray(b_gat, f32),
        "p2": (np.asarray(p2) / np.linalg.norm(np.asarray(p2))).astype(f32),
        "W23": W23, "Cc": Cc,
    }


def _prep_core_edges(cfg: Cfg, src_core, dst_core):
    """src/dst core-local [ne]. Per graph: append self loops, bucket edges by
    dst block (db = dst>>7), pad each db run to spd*128 slots. Pad slots get
    src=0 (any valid row; killed by the one-hot) and dlo=255 (matches no
    iota value -> all-zero one-hot row/column)."""
    SPD, SRUN = cfg.spd, cfg.spd * P
    loops = np.arange(cfg.npg, dtype=np.int64)
    src_slots = np.zeros((cfg.ng, cfg.tj, SRUN), np.int64)
    dlo_slots = np.full((cfg.ng, cfg.tj, SRUN), 255, np.int64)
    deg = np.zeros((cfg.ng, cfg.npg), np.int64)
    for g in range(cfg.ng):
        e = slice(g * cfg.eg, (g + 1) * cfg.eg)
        s = np.concatenate([src_core[e] - g * cfg.npg, loops])
        d = np.concatenate([dst_core[e] - g * cfg.npg, loops])
        deg[g] = np.bincount(d, minlength=cfg.npg)
        db = d >> 7
        for b in range(cfg.tj):
            m = db == b
            cnt = int(m.sum())
            assert cnt <= SRUN, f"db run overflow: {cnt} > {SRUN}"
            src_slots[g, b, :cnt] = s[m]
            dlo_slots[g, b, :cnt] = d[m] & 127
    stream_src = src_slots.reshape(-1)
    stream_dlo = dlo_slots.reshape(-1)
    deg_t = np.ascontiguousarray(
        deg.reshape(cfg.ng, cfg.tj, P).transpose(2, 0, 1).reshape(P, cfg.nt)
    ).astype(np.float32)
    bf16 = ml_dtypes.bfloat16
    dinv = (1.0 / np.sqrt(np.maximum(deg.reshape(-1), 1.0))).astype(np.float32)
    return {
        "srcw": _wrap_idx(stream_src),
        "dinvF": dinv,
        "dlo_pm": np.ascontiguousarray(
            stream_dlo.reshape(-1, P).T.astype(bf16)
        ),
        "dlo_fr": np.ascontiguousarray(dlo_slots.reshape(-1).astype(bf16)),
        "degT": deg_t,
    }


def build_bass(cfg: Cfg):
    from contextlib import ExitStack
    nc = bacc.Bacc("TRN2", target_bir_lowering=False, debug=False,
                   num_swdge_queues=4)
    with tile.TileContext(nc) as tc:
        with ExitStack() as ctx:
            build_core_program(ctx, tc, cfg)
    nc.compile()
    return nc


_CFG = Cfg()
_NC_CACHE = {}
TRACE = False
LAST_RESULT = None


def kernel(x, edge_index, batch, W1, b1, bn_gamma, bn_beta, bn_mean, bn_var,
           W_lin1, b_lin1, p1, Wl, Wr, att, b_gat, p2,
           W_lin2, b_lin2, W_lin3, b_lin3):
    cfg = _CFG
    n_cores = 8
    s_att = float(np.sum(np.asarray(att, dtype=np.float64)))
    assert abs(s_att) > 1e-6, "degenerate att sum; poison scheme needs |sum(att)|>0"
    cfg.psign = -1.0 if s_att > 0 else 1.0
    slope = 0.2 if s_att > 0 else 1.0
    cfg.pb_mag = 40.0 / (slope * abs(s_att))
    weights = _prep_weights(cfg, W1, b1, bn_gamma, bn_beta, bn_mean, bn_var,
                            W_lin1, b_lin1, p1, Wl, Wr, att, b_gat, p2,
                            W_lin2, b_lin2, W_lin3, b_lin3)
    src_all = np.asarray(edge_index[0], dtype=np.int64)
    dst_all = np.asarray(edge_index[1], dtype=np.int64)
    x = np.asarray(x, dtype=np.float32)

    # choose the chunks-per-db-run capacity from the data (global max so the
    # single SPMD program fits every core)
    max_run = 0
    for c in range(n_cores):
        for g in range(cfg.ng):
            e0 = c * cfg.ne + g * cfg.eg
            d = dst_all[e0 : e0 + cfg.eg] - (c * cfg.nn + g * cfg.npg)
            cnts = np.bincount(d >> 7, minlength=cfg.tj) + P  # + self loops
            max_run = max(max_run, int(cnts.max()))
    cfg.spd = (max_run + P - 1) // P
    # windows of ch slots must tile a graph's slot range exactly
    while (cfg.tj * cfg.spd * P) % cfg.ch != 0:
        cfg.spd += 1

    in_maps = []
    for c in range(n_cores):
        n0 = c * cfg.nn
        e0 = c * cfg.ne
        d = dict(weights)
        d.update(
            _prep_core_edges(
                cfg, src_all[e0 : e0 + cfg.ne] - n0, dst_all[e0 : e0 + cfg.ne] - n0
            )
        )
        d["xT"] = np.ascontiguousarray(x[n0 : n0 + cfg.nn].T, np.float32)
        in_maps.append(d)

    key = ("nc", cfg.spd, cfg.psign, cfg.pb_mag)
    if key not in _NC_CACHE:
        _NC_CACHE[key] = build_bass(cfg)
    nc = _NC_CACHE[key]
    global LAST_RESULT
    res = run_bass_kernel_spmd(nc, in_maps, core_ids=list(range(n_cores)), trace=TRACE)
    LAST_RESULT = res
    outs = [np.asarray(res.results[c]["out"]).reshape(cfg.ng, 1) for c in range(n_cores)]
    return np.concatenate(outs, axis=0).astype(np.float32)


# revision 42
# speedup vs baseline: 1.3216x; 1.0490x over previous
"""Trainium2 Bass kernel for nn_AGNN_EFG (GCN -> TopK pool -> GATv2 -> TopK pool -> head).

Self-contained: shards the B=64 graphs across 8 NeuronCores (8 graphs/core),
runs one SPMD Bass program, gathers the [64, 1] head output on host.

v4 design:
- Edges (incl self loops) sorted per graph by dst block (db = dst>>7), each
  db run padded to a fixed spd chunks of 128 -> static chunk->db schedule
  (SPMD-safe; spd chosen on host from the data).
- Scatter-adds are ONE-HOT MATMULS on PE: per chunk a [128e,128d] bf16
  one-hot (DVE is_equal; pad slots carry sentinel 255 -> zero rows)
  accumulates messages into per-graph PSUM. No dma_scatter_add.
- GAT's xr[dst] is a [128d,128e]-orientation one-hot matmul (one-hot built
  on the Scalar engine as relu(1-(dlo-d)^2)) reading xr blocks from SBUF.
- Only 2 indirect passes remain (GpSimd Q7 descriptor generation is the
  machine bottleneck): gather u[src] (GCN) and [xl|xr][src] (GAT).
- Fully per-graph pipelined: utab(g+2) build, dense/pool/gtab(g),
  gcn(g+1) and gat(g) interleave so the GpSimd gather stream never idles.
- GCN u rows split [u_hi|u_lo] bf16 (~f32 accuracy, one 128-wide matmul).
- score1 fused into the lin1 matmul via host-folded [W_lin1 | W_lin1@p1n].
- Degrees are host-precomputed index data (bincount of dst); rsqrt on device.
"""

import sys

sys.path.insert(0, "/opt/trn_rl_repo")

from dataclasses import dataclass

import numpy as np
import ml_dtypes

import concourse.bass as bass
import concourse.mybir as mybir
import concourse.tile as tile
from concourse import bacc
from concourse.bass_utils import run_bass_kernel_spmd
from concourse.masks import make_identity

P = 128
F32 = mybir.dt.float32
BF16 = mybir.dt.bfloat16
I16 = mybir.dt.int16
U8 = mybir.dt.uint8
AF = mybir.ActivationFunctionType
OP = mybir.AluOpType
AX = mybir.AxisListType


@dataclass
class Cfg:
    ng: int = 8          # graphs per core
    npg: int = 2048      # nodes per graph
    hid: int = 64        # feature dim
    eg: int = 32768      # edges per graph (original, without self loops)
    spd: int = 19        # chunks (of 128 slots) per dst-block run; set at runtime
    ch: int = 1024       # gather window (slots per dma_gather call; >=2048
                         # overflows the SWDGE descriptor ring and faults)
    n_bisect: int = 30   # bisection iterations for topk threshold
    psign: float = -1.0  # -sign(sum(att)): poison sign so poisoned e < 0
    pb_mag: float = 200.0  # poison magnitude; set so poisoned e ~ -40 (LUT-safe)

    @property
    def nn(self):
        return self.ng * self.npg

    @property
    def ne(self):
        return self.ng * self.eg

    @property
    def tj(self):
        return self.npg // P  # dst blocks per graph (16)

    @property
    def nt(self):
        return self.ng * self.tj  # 128

    @property
    def tch(self):
        return self.tj * self.spd  # chunks per graph

    @property
    def slots_g(self):
        return self.tch * P  # padded edge slots per graph

    @property
    def slots(self):
        return self.ng * self.slots_g

    @property
    def wpg(self):
        assert self.slots_g % self.ch == 0
        return self.slots_g // self.ch  # gather windows per graph

    @property
    def cpw(self):
        return self.ch // P  # chunks per window

    @property
    def k1(self):
        return self.npg // 2

    @property
    def k2(self):
        return self.npg // 4


def build_core_program(ctx, tc, cfg: Cfg):
    nc = tc.nc
    NG, NPG, HID, NN = cfg.ng, cfg.npg, cfg.hid, cfg.nn
    NT, TJ, SPD = cfg.nt, cfg.tj, cfg.spd
    TCH, WPG, CPW, CH = cfg.tch, cfg.wpg, cfg.cpw, cfg.ch
    SRUN = SPD * P  # slots per db run
    assert HID == 64 and CPW % 8 == 0

    # ---- I/O ----
    xT = nc.dram_tensor("xT", [HID, NN], F32, kind="ExternalInput").ap()
    srcw = nc.dram_tensor("srcw", [P, cfg.slots // 16], I16, kind="ExternalInput").ap()
    dlo_pm = nc.dram_tensor("dlo_pm", [P, NG * TCH], BF16, kind="ExternalInput").ap()
    dlo_rep = nc.dram_tensor("dlo_rep", [NG * TJ * P, SRUN], U8, kind="ExternalInput").ap()
    degT = nc.dram_tensor("degT", [P, NT], F32, kind="ExternalInput").ap()
    w_names = ["Wl", "Wr"]
    Wd = {n: nc.dram_tensor(n, [HID, HID], F32, kind="ExternalInput").ap() for n in w_names}
    W1d = nc.dram_tensor("W1dup", [HID, P], F32, kind="ExternalInput").ap()
    W1p = nc.dram_tensor("Wlin1p", [HID, HID + 1], F32, kind="ExternalInput").ap()
    dinvF = nc.dram_tensor("dinvF", [NN], F32, kind="ExternalInput").ap()
    b1c = nc.dram_tensor("b_lin1c", [HID + 1], F32, kind="ExternalInput").ap()
    v_names = ["bn_a", "bn_bp", "att", "b_gat", "p2", "W23"]
    Vd = {n: nc.dram_tensor(n, [HID], F32, kind="ExternalInput").ap() for n in v_names}
    Cd = nc.dram_tensor("Cc", [1], F32, kind="ExternalInput").ap()
    out_d = nc.dram_tensor("out", [NG, 1], F32, kind="ExternalOutput").ap()

    # ---- DRAM scratch (per-graph tables so gathers only depend on their
    # own graph's writes) ----
    utab = [nc.dram_tensor(f"utab{g}", [NPG, P], BF16).ap() for g in range(NG)]
    gtab = [nc.dram_tensor(f"gtab{g}", [NPG, P], BF16).ap() for g in range(NG)]
    xlr_dram = nc.dram_tensor("xlr_dram", [P, NN], BF16).ap()
    ufm_dram = nc.dram_tensor("ufm_dram", [P, NN], BF16).ap()
    sc_dram = nc.dram_tensor("sc_dram", [NN], F32).ap()

    cpool = ctx.enter_context(tc.tile_pool(name="consts", bufs=1))
    mpool = ctx.enter_context(tc.tile_pool(name="main", bufs=1))
    smallps = ctx.enter_context(tc.tile_pool(name="smallps", bufs=1, space="PSUM"))

    # ---- constants ----
    ident = cpool.tile([P, P], F32)
    make_identity(nc, ident[:])
    ones128 = cpool.tile([P, P], F32)
    nc.vector.memset(ones128[:], 1.0)
    nantile = cpool.tile([P, NT], F32)
    nc.vector.memset(nantile[:], float("nan"))
    negbig = cpool.tile([P, NT], F32)
    nc.vector.memset(negbig[:], -1e9)
    io16 = cpool.tile([P, P], I16)
    nc.gpsimd.iota(io16[:], pattern=[[1, P]], base=0, channel_multiplier=0)
    iota_bf = cpool.tile([P, P], BF16)
    nc.vector.tensor_copy(out=iota_bf[:], in_=io16[:])
    ioc16 = cpool.tile([P, 1], I16)
    nc.gpsimd.iota(ioc16[:], pattern=[[0, 1]], base=0, channel_multiplier=1)
    niota_col = cpool.tile([P, 1], F32)
    nc.vector.tensor_scalar(
        out=niota_col[:], in0=ioc16[:], scalar1=-1.0, scalar2=None, op0=OP.mult
    )

    Ws = {}
    for n in w_names:
        t = cpool.tile([HID, HID], F32, tag=f"w_{n}")
        nc.sync.dma_start(out=t[:], in_=Wd[n][:])
        Ws[n] = t
    W1p_sb = cpool.tile([HID, HID + 1], F32, tag="w_Wlin1p")
    nc.sync.dma_start(out=W1p_sb[:], in_=W1p[:])
    W1d_sb = cpool.tile([HID, P], F32, tag="w_W1dup")
    nc.sync.dma_start(out=W1d_sb[:], in_=W1d[:])
    b1c_sb = cpool.tile([HID + 1, 1], F32, tag="v_b1c")
    nc.sync.dma_start(out=b1c_sb[:], in_=b1c[:, None])
    Vs = {}
    for n in v_names:
        t = cpool.tile([HID, 1], F32, tag=f"v_{n}")
        nc.sync.dma_start(out=t[:], in_=Vd[n][:, None])
        Vs[n] = t
    att_rep = cpool.tile([P, HID], BF16)
    nc.gpsimd.dma_start(out=att_rep[:], in_=Vd["att"][None, :].to_broadcast([P, HID]))
    p2_rep = cpool.tile([P, HID], F32)
    nc.sync.dma_start(out=p2_rep[:], in_=Vd["p2"][None, :].to_broadcast([P, HID]))
    bgat_rep = cpool.tile([P, HID], F32)
    nc.sync.dma_start(out=bgat_rep[:], in_=Vd["b_gat"][None, :].to_broadcast([P, HID]))
    Cc_sb = cpool.tile([NG, 1], F32)
    nc.sync.dma_start(out=Cc_sb[:], in_=Cd[None, :].to_broadcast([NG, 1]))

    # whole per-chunk dst-low-bit table (for one-hot builds in [e,d] orientation)
    dlo_sb = cpool.tile([P, NG * TCH], BF16)
    nc.sync.dma_start(out=dlo_sb[:], in_=dlo_pm[:])

    # ---- dinv from host degree counts ----
    dinv_t = mpool.tile([P, NT], F32, tag="dinv_t")
    sqd_t = mpool.tile([P, NT], F32, tag="sqd_t")
    ntmp = mpool.tile([P, NT], F32, tag="ntmp")

    def recip_newton(r_ap, x_ap, tmp_ap):
        nc.vector.tensor_tensor(out=tmp_ap, in0=x_ap, in1=r_ap, op=OP.mult)
        nc.vector.tensor_scalar(
            out=tmp_ap, in0=tmp_ap, scalar1=-1.0, scalar2=2.0, op0=OP.mult, op1=OP.add
        )
        nc.vector.tensor_tensor(out=r_ap, in0=r_ap, in1=tmp_ap, op=OP.mult)

    deg_sb = mpool.tile([P, NT], F32, tag="deg_sb")
    nc.sync.dma_start(out=deg_sb[:], in_=degT[:])
    nc.scalar.sqrt(out=sqd_t[:], in_=deg_sb[:])
    nc.vector.reciprocal(out=dinv_t[:], in_=sqd_t[:])
    recip_newton(dinv_t[:], sqd_t[:], ntmp[:])

    def idx_slice(pool, w_global, tag):
        t = pool.tile([P, CH // 16], I16, tag=tag)
        c0 = w_global * (CH // 16)
        nc.sync.dma_start(out=t[:], in_=srcw[:, c0 : c0 + CH // 16])
        return t

    # ---- pools (all phases interleave; PSUM budget: 2+1+2+2+1 = 8 banks) ----
    ups = ctx.enter_context(tc.tile_pool(name="ups", bufs=2))
    ubc = ctx.enter_context(tc.tile_pool(name="ubc", bufs=1))
    gep = ctx.enter_context(tc.tile_pool(name="gep", bufs=5))
    ohp = ctx.enter_context(tc.tile_pool(name="ohp", bufs=2))
    hps_pool = ctx.enter_context(tc.tile_pool(name="hpsp", bufs=1, space="PSUM"))
    dsa = ctx.enter_context(tc.tile_pool(name="dsa", bufs=1))
    dss = ctx.enter_context(tc.tile_pool(name="dss", bufs=2))
    bis = ctx.enter_context(tc.tile_pool(name="bis", bufs=2))
    gap = ctx.enter_context(tc.tile_pool(name="gap", bufs=3))
    aohp = ctx.enter_context(tc.tile_pool(name="aohp", bufs=2))
    runtmp = ctx.enter_context(tc.tile_pool(name="runtmp", bufs=1))
    runp = ctx.enter_context(tc.tile_pool(name="runp", bufs=2))
    gatps = ctx.enter_context(tc.tile_pool(name="gatps", bufs=1, space="PSUM"))
    mxrp = ctx.enter_context(tc.tile_pool(name="mxrp", bufs=2, space="PSUM"))
    gfin = ctx.enter_context(tc.tile_pool(name="gfin", bufs=1))

    # persistent t-space tiles
    h2t = mpool.tile([P, NT, HID], F32, tag="bigA")
    gstage = mpool.tile([P, NT, P], BF16, tag="bigC")
    score1_t = mpool.tile([P, NT], F32, tag="score1_t")
    tanh1 = mpool.tile([P, NT], F32, tag="tanh1")
    gate1 = mpool.tile([P, NT], F32, tag="gate1")
    gate1z = mpool.tile([P, NT], F32, tag="gate1z")
    padd = mpool.tile([P, NT], F32, tag="padd")
    kept1 = mpool.tile([P, NT], U8, tag="kept1")
    pb = cfg.psign * cfg.pb_mag

    hps_tiles = {}
    nmps_tiles = {}

    # ======== per-graph sections ========

    def utab_build(g):
        # xw (duplicated rows) -> dinv scale -> [hi|lo] bf16 split, all in
        # feature-major; node-major utab rows via one transpose DMA hop.
        xw2 = ups.tile([P, NPG], F32, tag="xw2", name=f"xw2_{g}")
        for jl in range(TJ):
            xTg = ups.tile([HID, P], F32, tag="xTg", name=f"xTg{g}_{jl}")
            nc.scalar.dma_start(
                out=xTg[:], in_=xT[:, g * NPG + jl * P : g * NPG + (jl + 1) * P]
            )
            pm = smallps.tile([P, P], F32, tag="smA", name=f"xwps{g}_{jl}")
            nc.tensor.matmul(pm[:], lhsT=W1d_sb[:], rhs=xTg[:], start=True, stop=True)
            nc.scalar.copy(out=xw2[:, jl * P : (jl + 1) * P], in_=pm[:])
        dbc = ubc.tile([P, NPG], F32, tag="dinvbc", name=f"dinvbc{g}")
        nc.scalar.dma_start(
            out=dbc[:], in_=dinvF[g * NPG : (g + 1) * NPG][None, :].to_broadcast([P, NPG])
        )
        nc.vector.tensor_tensor(out=xw2[:], in0=xw2[:], in1=dbc[:], op=OP.mult)
        u2 = ups.tile([P, NPG], BF16, tag="u2", name=f"u2_{g}")
        nc.vector.tensor_copy(out=u2[0:HID, :], in_=xw2[0:HID, :])
        nc.vector.tensor_copy(out=u2[HID:P, :], in_=xw2[HID:P, :])
        nc.vector.tensor_tensor(
            out=u2[HID:P, :], in0=xw2[HID:P, :], in1=u2[HID:P, :], op=OP.subtract
        )
        nc.scalar.dma_start(out=ufm_dram[:, g * NPG : (g + 1) * NPG], in_=u2[:])
        u_nm = ups.tile([P, TJ, P], BF16, tag="u2", name=f"unm{g}")
        nc.sync.dma_start_transpose(
            out=u_nm[:], in_=ufm_dram[:, g * NPG : (g + 1) * NPG]
        )
        nc.scalar.dma_start(
            out=utab[g].rearrange("(j p) f -> p j f", p=P), in_=u_nm[:]
        )

    def gcn_pass(g):
        hps = [
            hps_pool.tile([P, 8, HID], F32, tag=f"hps{t}", name=f"hps{t}_{g}")
            for t in range(2)
        ]
        hps_tiles[g] = hps
        for w in range(WPG):
            wg = g * WPG + w
            ssl = idx_slice(gep, wg, "ssl")
            ub = gep.tile([P, CPW, P], BF16, tag="ub")
            nc.gpsimd.dma_gather(
                out_ap=ub[:], in_ap=utab[g][:], idxs_ap=ssl[:],
                num_idxs=CH, num_idxs_reg=CH, elem_size=P, queue_num=wg % 4,
            )
            for b in range(CPW // 8):
                oh8 = ohp.tile([P, 8, P], BF16, tag="oh8")
                c0 = g * TCH + w * CPW + b * 8
                nc.vector.tensor_tensor(
                    out=oh8[:],
                    in0=iota_bf[:, None, :].to_broadcast([P, 8, P]),
                    in1=dlo_sb[:, c0 : c0 + 8, None].to_broadcast([P, 8, P]),
                    op=OP.is_equal,
                )
                for cl in range(8):
                    c = b * 8 + cl
                    gc = w * CPW + c
                    db, pos = gc // SPD, gc % SPD
                    out_slc = hps[db // 8][:, db % 8, :]
                    nc.tensor.matmul(
                        out_slc, lhsT=oh8[:, cl, :], rhs=ub[:, c, 0:HID],
                        start=(pos == 0), stop=False,
                    )
                    nc.tensor.matmul(
                        out_slc, lhsT=oh8[:, cl, :], rhs=ub[:, c, HID:P],
                        start=False, stop=(pos == SPD - 1),
                    )

    S6C = 512

    def dense_g(g):
        # h = dinv*(hi+lo); BN+leaky (one ACT Lrelu); lin1(+score); xl/xr
        hps = hps_tiles.pop(g)
        gsl = slice(g * TJ, (g + 1) * TJ)
        hsum = dsa.tile([P, TJ, HID], F32, tag="hsum", name=f"hsum{g}")
        for t in range(2):
            nc.vector.tensor_tensor(
                out=hsum[:, t * 8 : (t + 1) * 8, :], in0=hps[t][:],
                in1=dinv_t[:, g * TJ + t * 8 : g * TJ + (t + 1) * 8, None].to_broadcast(
                    [P, 8, HID]
                ),
                op=OP.mult,
            )
        hfm = dsa.tile([HID, NPG], F32, tag="hfm", name=f"hfm{g}")
        for jl in range(TJ):
            pt = smallps.tile([HID, P], F32, tag="smA", name=f"htr{g}_{jl}")
            nc.tensor.transpose(out=pt[:], in_=hsum[:, jl, :], identity=ident[:])
            nc.scalar.copy(out=hfm[:, jl * P : (jl + 1) * P], in_=pt[:])
        nc.scalar.activation(
            out=hfm[:], in_=hfm[:], func=AF.Lrelu, scale=Vs["bn_a"][:],
            bias=Vs["bn_bp"][:], alpha=0.01,
        )
        xlr_g = dss.tile([P, NPG], BF16, tag="xlrg", name=f"xlrg{g}")
        for ol in range(0, NPG, S6C):
            o = g * NPG + ol
            pm = hps_pool.tile([HID + 1, S6C], F32, tag="hps0", name=f"l1ps{o}")
            nc.tensor.matmul(
                pm[:], lhsT=W1p_sb[:], rhs=hfm[:, ol : ol + S6C],
                start=True, stop=True,
            )
            hc = dss.tile([HID + 1, S6C], F32, tag="hc", name=f"hc{o}")
            nc.scalar.activation(
                out=hc[:], in_=pm[:], func=AF.Identity, bias=b1c_sb[:]
            )
            nc.sync.dma_start(out=sc_dram[None, o : o + S6C], in_=hc[HID : HID + 1, :])
            px = hps_pool.tile([P, S6C], F32, tag="hps1", name=f"xlrps{o}")
            nc.tensor.matmul(px[:HID, :], lhsT=Ws["Wl"][:], rhs=hc[0:HID, :], start=True, stop=True)
            nc.tensor.matmul(px[HID:, :], lhsT=Ws["Wr"][:], rhs=hc[0:HID, :], start=True, stop=True)
            nc.scalar.copy(out=xlr_g[:, ol : ol + S6C], in_=px[:])
        nc.scalar.dma_start(out=xlr_dram[:, g * NPG : (g + 1) * NPG], in_=xlr_g[:])

    def bisect_multi(score_slc, ngr, target, tag):
        # score_slc: [P, ngr*TJ]; returns per-graph thresholds lo [P, ngr]
        lo = bis.tile([P, ngr], F32, tag="lo", name=f"lo_{tag}")
        hi = bis.tile([P, ngr], F32, tag="hi", name=f"hi_{tag}")
        mid = bis.tile([P, ngr], F32, tag="mid", name=f"mid_{tag}")
        cmp = bis.tile([P, ngr * TJ], F32, tag="cmp", name=f"cmp_{tag}")
        cred = bis.tile([P, ngr], F32, tag="cred", name=f"cred_{tag}")
        ge = bis.tile([P, ngr], U8, tag="ge", name=f"ge_{tag}")
        lt = bis.tile([P, ngr], U8, tag="lt", name=f"lt_{tag}")
        nc.vector.memset(lo[:], -64.0)
        nc.vector.memset(hi[:], 64.0)
        sc_g = score_slc.rearrange("p (g t) -> p g t", g=ngr)
        cmp_g = cmp[:].rearrange("p (g t) -> p g t", g=ngr)
        for it in range(cfg.n_bisect):
            nc.vector.tensor_tensor(out=mid[:], in0=lo[:], in1=hi[:], op=OP.add)
            nc.vector.tensor_scalar(
                out=mid[:], in0=mid[:], scalar1=0.5, scalar2=None, op0=OP.mult
            )
            nc.vector.tensor_tensor(
                out=cmp_g, in0=sc_g,
                in1=mid[:, :, None].to_broadcast([P, ngr, TJ]), op=OP.is_gt,
            )
            nc.vector.tensor_reduce(out=cred[:], in_=cmp_g, axis=AX.X, op=OP.add)
            cps = smallps.tile([P, ngr], F32, tag="smB", name=f"cnt_{tag}_{it}")
            nc.tensor.matmul(cps[:], lhsT=ones128[:], rhs=cred[:], start=True, stop=True)
            nc.vector.tensor_scalar(
                out=ge[:], in0=cps[:], scalar1=float(target), scalar2=None, op0=OP.is_ge
            )
            nc.vector.tensor_scalar(
                out=lt[:], in0=cps[:], scalar1=float(target), scalar2=None, op0=OP.is_lt
            )
            nc.vector.copy_predicated(out=lo[:], mask=ge[:], data=mid[:])
            nc.vector.copy_predicated(out=hi[:], mask=lt[:], data=mid[:])
        return lo

    NH = NG // 2  # graphs per pooling half

    def pool1_h(h):
        g0 = h * NH
        gsl = slice(g0 * TJ, (g0 + NH) * TJ)
        nc.sync.dma_start(
            out=score1_t[:, gsl],
            in_=sc_dram[g0 * NPG : (g0 + NH) * NPG].rearrange("(j p) -> p j", p=P),
        )
        t1 = bisect_multi(score1_t[:, gsl], NH, cfg.k1, f"p1h{h}")
        nc.vector.tensor_tensor(
            out=kept1[:, gsl].rearrange("p (g t) -> p g t", g=NH),
            in0=score1_t[:, gsl].rearrange("p (g t) -> p g t", g=NH),
            in1=t1[:, :, None].to_broadcast([P, NH, TJ]), op=OP.is_gt,
        )
        nc.scalar.activation(out=tanh1[:, gsl], in_=score1_t[:, gsl], func=AF.Tanh)
        nc.vector.tensor_copy(out=gate1[:, gsl], in_=nantile[:, gsl])
        nc.vector.copy_predicated(out=gate1[:, gsl], mask=kept1[:, gsl], data=tanh1[:, gsl])
        nc.vector.memset(gate1z[:, gsl], 0.0)
        nc.vector.copy_predicated(out=gate1z[:, gsl], mask=kept1[:, gsl], data=tanh1[:, gsl])
        nc.vector.tensor_scalar(
            out=padd[:, gsl], in0=kept1[:, gsl], scalar1=-pb, scalar2=pb,
            op0=OP.mult, op1=OP.add,
        )
        # gtab for the half
        gs = gstage[:, gsl, :]
        nc.sync.dma_start_transpose(
            out=gs, in_=xlr_dram[:, g0 * NPG : (g0 + NH) * NPG]
        )
        nc.vector.tensor_tensor(
            out=gs, in0=gs,
            in1=gate1z[:, gsl, None].to_broadcast([P, NH * TJ, P]), op=OP.mult,
        )
        nc.vector.tensor_tensor(
            out=gs, in0=gs,
            in1=padd[:, gsl, None].to_broadcast([P, NH * TJ, P]), op=OP.add,
        )
        for gg in range(g0, g0 + NH):
            nc.sync.dma_start(
                out=gtab[gg].rearrange("(j p) f -> p j f", p=P),
                in_=gstage[:, gg * TJ : (gg + 1) * TJ, :],
            )

    def gat_pass(g):
        nmps = [
            gatps.tile([P, 8, HID], F32, tag=f"nmps{t}", name=f"nmps{t}_{g}")
            for t in range(2)
        ]
        dnps = smallps.tile([P, TJ], F32, tag="smB", name=f"dnps{g}")
        oh_de = {}
        for w in range(WPG):
            wg = g * WPG + w
            ssl = idx_slice(gap, wg, "assl")
            gx = gap.tile([P, CPW, P], BF16, tag="gx")
            nc.gpsimd.dma_gather(
                out_ap=gx[:], in_ap=gtab[g][:], idxs_ap=ssl[:],
                num_idxs=CH, num_idxs_reg=CH, elem_size=P, queue_num=wg % 4,
            )
            for b in range(CPW // 8):
                oh8 = aohp.tile([P, 8, P], BF16, tag="aoh8")
                c0 = g * TCH + w * CPW + b * 8
                nc.vector.tensor_tensor(
                    out=oh8[:],
                    in0=iota_bf[:, None, :].to_broadcast([P, 8, P]),
                    in1=dlo_sb[:, c0 : c0 + 8, None].to_broadcast([P, 8, P]),
                    op=OP.is_equal,
                )
                mxr = mxrp.tile([P, 8, HID], F32, tag="mxr", name=f"mxr{wg}_{b}")
                for cl in range(8):
                    c = b * 8 + cl
                    gc = w * CPW + c
                    r, pos = gc // SPD, gc % SPD
                    if pos == 0:
                        dlo_bc = runtmp.tile(
                            [P, SRUN], U8, tag="dlobc", name=f"dlobc{g}_{r}"
                        )
                        o = (g * TJ + r) * P
                        nc.sync.dma_start(
                            out=dlo_bc[:], in_=dlo_rep[o : o + P, :]
                        )
                        # one-hot on the Scalar engine: relu(1 - (dlo - d)^2)
                        ohsq = runtmp.tile([P, SRUN], BF16, tag="ohsq", name=f"ohsq{g}_{r}")
                        nc.scalar.activation(
                            out=ohsq[:], in_=dlo_bc[:], func=AF.Square,
                            bias=niota_col[:],
                        )
                        ohr = runp.tile([P, SRUN], BF16, tag="ohde", name=f"ohde{g}_{r}")
                        nc.scalar.activation(
                            out=ohr[:], in_=ohsq[:], func=AF.Relu,
                            bias=1.0, scale=-1.0,
                        )
                        oh_de[r] = ohr
                    nc.tensor.matmul(
                        mxr[:, cl, :],
                        lhsT=oh_de[r][:, pos * P : (pos + 1) * P],
                        rhs=gstage[:, g * TJ + r, HID:P],
                        start=True, stop=True,
                    )
                # e = att . leaky(xl_s + xr_d); w = exp(e); pay = w*xl
                gxs = gx[:, b * 8 : (b + 1) * 8, :]
                z = gap.tile([P, 8, HID], BF16, tag="z")
                nc.vector.tensor_tensor(
                    out=z[:], in0=gxs[:, :, 0:HID], in1=mxr[:], op=OP.add
                )
                nc.vector.scalar_tensor_tensor(
                    out=z[:], in0=z[:], scalar=0.2, in1=z[:], op0=OP.mult, op1=OP.max,
                )
                nc.vector.tensor_tensor(
                    out=z[:], in0=z[:],
                    in1=att_rep[:, None, :].to_broadcast([P, 8, HID]), op=OP.mult,
                )
                e8 = gap.tile([P, 8], F32, tag="e8")
                nc.vector.tensor_reduce(out=e8[:], in_=z[:], axis=AX.X, op=OP.add)
                w8b = gap.tile([P, 8], BF16, tag="w8b")
                nc.scalar.activation(out=w8b[:], in_=e8[:], func=AF.Exp)
                pay = gap.tile([P, 8, HID], BF16, tag="pay")
                nc.vector.tensor_tensor(
                    out=pay[:], in0=gxs[:, :, 0:HID],
                    in1=w8b[:, :, None].to_broadcast([P, 8, HID]), op=OP.mult,
                )
                for cl in range(8):
                    c = b * 8 + cl
                    gc = w * CPW + c
                    db, pos = gc // SPD, gc % SPD
                    nc.tensor.matmul(
                        nmps[db // 8][:, db % 8, :],
                        lhsT=oh8[:, cl, :], rhs=pay[:, cl, :],
                        start=(pos == 0), stop=(pos == SPD - 1),
                    )
                    nc.tensor.matmul(
                        dnps[:, db : db + 1],
                        lhsT=oh8[:, cl, :], rhs=w8b[:, cl : cl + 1],
                        start=(pos == 0), stop=(pos == SPD - 1),
                    )

        # ---- finalize graph g: h2 = leaky(numer/denom + b_gat) ----
        numsb = dsa.tile([P, TJ, HID], F32, tag="hsum", name=f"numsb{g}")
        nc.scalar.copy(out=numsb[:, 0:8, :], in_=nmps[0][:])
        nc.scalar.copy(out=numsb[:, 8:TJ, :], in_=nmps[1][:])
        den = gfin.tile([P, TJ], F32, tag="den", name=f"den{g}")
        rec = gfin.tile([P, TJ], F32, tag="rec", name=f"rec{g}")
        dtmp = gfin.tile([P, TJ], F32, tag="dtmp", name=f"dtmp{g}")
        nc.vector.tensor_scalar(
            out=den[:], in0=dnps[:], scalar1=1e-16, scalar2=None, op0=OP.add
        )
        nc.vector.reciprocal(out=rec[:], in_=den[:])
        recip_newton(rec[:], den[:], dtmp[:])
        hslc = h2t[:, g * TJ : (g + 1) * TJ, :]
        nc.vector.tensor_tensor(
            out=hslc, in0=numsb[:],
            in1=rec[:, :, None].to_broadcast([P, TJ, HID]), op=OP.mult,
        )
        nc.vector.tensor_tensor(
            out=hslc, in0=hslc,
            in1=bgat_rep[:, None, :].to_broadcast([P, TJ, HID]), op=OP.add,
        )
        nc.vector.scalar_tensor_tensor(
            out=hslc, in0=hslc, scalar=0.01, in1=hslc, op0=OP.mult, op1=OP.max
        )

    # ======== emission: gcn stream, pools, gat stream ========
    utab_build(0)
    utab_build(1)
    for g in range(NG):
        gcn_pass(g)
        dense_g(g)
        if g + 2 < NG:
            utab_build(g + 2)
    pool1_h(0)
    pool1_h(1)
    for g in range(NG):
        gat_pass(g)

    # ======== score2 (t-space, blocked), mask to kept1 ========
    score2_t = mpool.tile([P, NT], F32, tag="score2_t")
    for t in range(4):
        tsl = slice(t * 32, (t + 1) * 32)
        blk = ups.tile([P, 32, HID], F32, tag="xTg", name=f"s2blk{t}")
        nc.vector.tensor_tensor(
            out=blk[:], in0=h2t[:, tsl, :],
            in1=p2_rep[:, None, :].to_broadcast([P, 32, HID]), op=OP.mult,
        )
        nc.vector.tensor_reduce(out=score2_t[:, tsl], in_=blk[:], axis=AX.X, op=OP.add)
    kept1_t = mpool.tile([P, NT], U8, tag="kept1_t")
    nc.vector.tensor_tensor(out=kept1_t[:], in0=gate1[:], in1=gate1[:], op=OP.is_equal)
    sc2m = mpool.tile([P, NT], F32, tag="sc2m")
    nc.vector.tensor_copy(out=sc2m[:], in_=negbig[:])
    nc.vector.copy_predicated(out=sc2m[:], mask=kept1_t[:], data=score2_t[:])

    # ======== pool2 threshold + gate2 = tanh * mask ========
    gate2 = mpool.tile([P, NT], F32, tag="gate2")
    t2 = bisect_multi(sc2m[:], NG, cfg.k2, "p2")
    nc.vector.tensor_tensor(
        out=gate2[:].rearrange("p (g t) -> p g t", g=NG),
        in0=sc2m[:].rearrange("p (g t) -> p g t", g=NG),
        in1=t2[:, :, None].to_broadcast([P, NG, TJ]), op=OP.is_gt,
    )
    tanh2 = mpool.tile([P, NT], F32, tag="tanh2")
    sc2c = mpool.tile([P, NT], F32, tag="sc2c")
    nc.vector.tensor_scalar(
        out=sc2c[:], in0=sc2m[:], scalar1=-64.0, scalar2=None, op0=OP.max
    )
    nc.scalar.activation(out=tanh2[:], in_=sc2c[:], func=AF.Tanh)
    nc.vector.tensor_tensor(out=gate2[:], in0=gate2[:], in1=tanh2[:], op=OP.mult)

    # ======== T_g = sum_n gate2[n] * h2[n]; out = T @ W23 + C ========
    Tps = smallps.tile([P, NG], F32, tag="smB")
    for j in range(NT):
        g = j // TJ
        nc.tensor.matmul(
            Tps[:HID, g : g + 1], lhsT=h2t[:, j, :], rhs=gate2[:, j : j + 1],
            start=(j % TJ == 0), stop=(j % TJ == TJ - 1),
        )
    Tsb = mpool.tile([HID, NG], F32, tag="Tsb")
    nc.scalar.copy(out=Tsb[:], in_=Tps[:HID, :])
    hps2 = smallps.tile([NG, 1], F32, tag="smB")
    nc.tensor.matmul(hps2[:], lhsT=Tsb[:], rhs=Vs["W23"][:], start=True, stop=True)
    outsb = mpool.tile([NG, 1], F32, tag="outsb")
    nc.vector.tensor_tensor(out=outsb[:], in0=hps2[:], in1=Cc_sb[:], op=OP.add)
    nc.sync.dma_start(out=out_d[:], in_=outsb[:])


# ================= host side =================

def _wrap_idx(ix: np.ndarray) -> np.ndarray:
    n = ix.shape[0]
    w = ix.reshape(n // 16, 16).T.astype(np.int16)
    return np.tile(w, (8, 1)).copy()


def _prep_weights(cfg, W1, b1, bn_gamma, bn_beta, bn_mean, bn_var, W_lin1, b_lin1,
                  p1, Wl, Wr, att, b_gat, p2, W_lin2, b_lin2, W_lin3, b_lin3):
    f32 = np.float32
    bn_a = (bn_gamma / np.sqrt(bn_var + 1e-5)).astype(f32)
    bn_b = (bn_beta - bn_mean * bn_a).astype(f32)
    W23 = (W_lin2 @ W_lin3).reshape(-1).astype(f32)
    Cc = np.array([cfg.k2 * float(b_lin2 @ W_lin3[:, 0]) + float(b_lin3[0])], dtype=f32)
    p1n = (np.asarray(p1) / np.linalg.norm(np.asarray(p1))).astype(np.float64)
    Wlin1p = np.concatenate(
        [np.asarray(W_lin1, np.float64),
         (np.asarray(W_lin1, np.float64) @ p1n)[:, None]], axis=1
    ).astype(f32)
    c1 = np.array([float(p1n @ np.asarray(b_lin1, np.float64))], dtype=f32)
    return {
        "W1dup": np.ascontiguousarray(
            np.concatenate([np.asarray(W1, f32)] * 2, axis=1)
        ),
        "Wlin1p": Wlin1p,
        "Wl": np.ascontiguousarray(Wl, f32), "Wr": np.ascontiguousarray(Wr, f32),
        "bn_a": bn_a, "bn_bp": (np.asarray(b1, f32) * bn_a + bn_b).astype(f32),
        "b_lin1c": np.concatenate([np.asarray(b_lin1, f32), c1]),
        "att": np.ascontiguousarray(att, f32), "b_gat": np.ascontiguousarray(b_gat, f32),
        "p2": (np.asarray(p2) / np.linalg.norm(np.asarray(p2))).astype(f32),
        "W23": W23, "Cc": Cc,
    }


def _prep_core_edges(cfg: Cfg, src_core, dst_core):
    """src/dst core-local [ne]. Per graph: append self loops, bucket edges by
    dst block (db = dst>>7), pad each db run to spd*128 slots. Pad slots get
    src=0 (any valid row; killed by the one-hot) and dlo=255 (matches no
    iota value -> all-zero one-hot row/column)."""
    SPD, SRUN = cfg.spd, cfg.spd * P
    loops = np.arange(cfg.npg, dtype=np.int64)
    src_slots = np.zeros((cfg.ng, cfg.tj, SRUN), np.int64)
    dlo_slots = np.full((cfg.ng, cfg.tj, SRUN), 255, np.int64)
    deg = np.zeros((cfg.ng, cfg.npg), np.int64)
    for g in range(cfg.ng):
        e = slice(g * cfg.eg, (g + 1) * cfg.eg)
        s = np.concatenate([src_core[e] - g * cfg.npg, loops])
        d = np.concatenate([dst_core[e] - g * cfg.npg, loops])
        deg[g] = np.bincount(d, minlength=cfg.npg)
        db = d >> 7
        for b in range(cfg.tj):
            m = db == b
            cnt = int(m.sum())
            assert cnt <= SRUN, f"db run overflow: {cnt} > {SRUN}"
            src_slots[g, b, :cnt] = s[m]
            dlo_slots[g, b, :cnt] = d[m] & 127
    stream_src = src_slots.reshape(-1)
    stream_dlo = dlo_slots.reshape(-1)
    deg_t = np.ascontiguousarray(
        deg.reshape(cfg.ng, cfg.tj, P).transpose(2, 0, 1).reshape(P, cfg.nt)
    ).astype(np.float32)
    bf16 = ml_dtypes.bfloat16
    dinv = (1.0 / np.sqrt(np.maximum(deg.reshape(-1), 1.0))).astype(np.float32)
    return {
        "srcw": _wrap_idx(stream_src),
        "dinvF": dinv,
        "dlo_pm": np.ascontiguousarray(
            stream_dlo.reshape(-1, P).T.astype(bf16)
        ),
        "dlo_rep": np.ascontiguousarray(
            np.repeat(
                dlo_slots.reshape(cfg.ng * cfg.tj, 1, SRUN), P, axis=1
            ).astype(np.uint8)
        ),
        "degT": deg_t,
    }


def build_bass(cfg: Cfg):
    from contextlib import ExitStack
    nc = bacc.Bacc("TRN2", target_bir_lowering=False, debug=False,
                   num_swdge_queues=4)
    with tile.TileContext(nc) as tc:
        with ExitStack() as ctx:
            build_core_program(ctx, tc, cfg)
    nc.compile()
    return nc


_CFG = Cfg()
_NC_CACHE = {}
TRACE = False
LAST_RESULT = None


def kernel(x, edge_index, batch, W1, b1, bn_gamma, bn_beta, bn_mean, bn_var,
           W_lin1, b_lin1, p1, Wl, Wr, att, b_gat, p2,
           W_lin2, b_lin2, W_lin3, b_lin3):
    cfg = _CFG
    n_cores = 8
    s_att = float(np.sum(np.asarray(att, dtype=np.float64)))
    assert abs(s_att) > 1e-6, "degenerate att sum; poison scheme needs |sum(att)|>0"
    cfg.psign = -1.0 if s_att > 0 else 1.0
    slope = 0.2 if s_att > 0 else 1.0
    cfg.pb_mag = 40.0 / (slope * abs(s_att))
    weights = _prep_weights(cfg, W1, b1, bn_gamma, bn_beta, bn_mean, bn_var,
                            W_lin1, b_lin1, p1, Wl, Wr, att, b_gat, p2,
                            W_lin2, b_lin2, W_lin3, b_lin3)
    src_all = np.asarray(edge_index[0], dtype=np.int64)
    dst_all = np.asarray(edge_index[1], dtype=np.int64)
    x = np.asarray(x, dtype=np.float32)

    # choose the chunks-per-db-run capacity from the data (global max so the
    # single SPMD program fits every core)
    max_run = 0
    for c in range(n_cores):
        for g in range(cfg.ng):
            e0 = c * cfg.ne + g * cfg.eg
            d = dst_all[e0 : e0 + cfg.eg] - (c * cfg.nn + g * cfg.npg)
            cnts = np.bincount(d >> 7, minlength=cfg.tj) + P  # + self loops
            max_run = max(max_run, int(cnts.max()))
    cfg.spd = (max_run + P - 1) // P
    # windows of ch slots must tile a graph's slot range exactly
    while (cfg.tj * cfg.spd * P) % cfg.ch != 0:
        cfg.spd += 1

    in_maps = []
    for c in range(n_cores):
        n0 = c * cfg.nn
        e0 = c * cfg.ne
        d = dict(weights)
        d.update(
            _prep_core_edges(
                cfg, src_all[e0 : e0 + cfg.ne] - n0, dst_all[e0 : e0 + cfg.ne] - n0
            )
        )
        d["xT"] = np.ascontiguousarray(x[n0 : n0 + cfg.nn].T, np.float32)
        in_maps.append(d)

    key = ("nc", cfg.spd, cfg.psign, cfg.pb_mag)
    if key not in _NC_CACHE:
        _NC_CACHE[key] = build_bass(cfg)
    nc = _NC_CACHE[key]
    global LAST_RESULT
    res = run_bass_kernel_spmd(nc, in_maps, core_ids=list(range(n_cores)), trace=TRACE)
    LAST_RESULT = res
    outs = [np.asarray(res.results[c]["out"]).reshape(cfg.ng, 1) for c in range(n_cores)]
    return np.concatenate(outs, axis=0).astype(np.float32)


# revision 45
# speedup vs baseline: 1.3433x; 1.0164x over previous
"""Trainium2 Bass kernel for nn_AGNN_EFG (GCN -> TopK pool -> GATv2 -> TopK pool -> head).

Self-contained: shards the B=64 graphs across 8 NeuronCores (8 graphs/core),
runs one SPMD Bass program, gathers the [64, 1] head output on host.

v4 design:
- Edges (incl self loops) sorted per graph by dst block (db = dst>>7), each
  db run padded to a fixed spd chunks of 128 -> static chunk->db schedule
  (SPMD-safe; spd chosen on host from the data).
- Scatter-adds are ONE-HOT MATMULS on PE: per chunk a [128e,128d] bf16
  one-hot (DVE is_equal; pad slots carry sentinel 255 -> zero rows)
  accumulates messages into per-graph PSUM. No dma_scatter_add.
- GAT's xr[dst] is a [128d,128e]-orientation one-hot matmul (one-hot built
  on the Scalar engine as relu(1-(dlo-d)^2)) reading xr blocks from SBUF.
- Only 2 indirect passes remain (GpSimd Q7 descriptor generation is the
  machine bottleneck): gather u[src] (GCN) and [xl|xr][src] (GAT).
- Fully per-graph pipelined: utab(g+2) build, dense/pool/gtab(g),
  gcn(g+1) and gat(g) interleave so the GpSimd gather stream never idles.
- GCN u rows split [u_hi|u_lo] bf16 (~f32 accuracy, one 128-wide matmul).
- score1 fused into the lin1 matmul via host-folded [W_lin1 | W_lin1@p1n].
- Degrees are host-precomputed index data (bincount of dst); rsqrt on device.
"""

import sys

sys.path.insert(0, "/opt/trn_rl_repo")

from dataclasses import dataclass

import numpy as np
import ml_dtypes

import concourse.bass as bass
import concourse.mybir as mybir
import concourse.tile as tile
from concourse import bacc
from concourse.bass_utils import run_bass_kernel_spmd
from concourse.masks import make_identity

P = 128
F32 = mybir.dt.float32
BF16 = mybir.dt.bfloat16
I16 = mybir.dt.int16
U8 = mybir.dt.uint8
AF = mybir.ActivationFunctionType
OP = mybir.AluOpType
AX = mybir.AxisListType


@dataclass
class Cfg:
    ng: int = 8          # graphs per core
    npg: int = 2048      # nodes per graph
    hid: int = 64        # feature dim
    eg: int = 32768      # edges per graph (original, without self loops)
    spd: int = 19        # chunks (of 128 slots) per dst-block run; set at runtime
    ch: int = 1024       # gather window (slots per dma_gather call; >=2048
                         # overflows the SWDGE descriptor ring and faults)
    n_bisect: int = 30   # bisection iterations for topk threshold
    psign: float = -1.0  # -sign(sum(att)): poison sign so poisoned e < 0
    pb_mag: float = 200.0  # poison magnitude; set so poisoned e ~ -40 (LUT-safe)

    @property
    def nn(self):
        return self.ng * self.npg

    @property
    def ne(self):
        return self.ng * self.eg

    @property
    def tj(self):
        return self.npg // P  # dst blocks per graph (16)

    @property
    def nt(self):
        return self.ng * self.tj  # 128

    @property
    def tch(self):
        return self.tj * self.spd  # chunks per graph

    @property
    def slots_g(self):
        return self.tch * P  # padded edge slots per graph

    @property
    def slots(self):
        return self.ng * self.slots_g

    @property
    def wpg(self):
        assert self.slots_g % self.ch == 0
        return self.slots_g // self.ch  # gather windows per graph

    @property
    def cpw(self):
        return self.ch // P  # chunks per window

    @property
    def k1(self):
        return self.npg // 2

    @property
    def k2(self):
        return self.npg // 4


def build_core_program(ctx, tc, cfg: Cfg):
    nc = tc.nc
    NG, NPG, HID, NN = cfg.ng, cfg.npg, cfg.hid, cfg.nn
    NT, TJ, SPD = cfg.nt, cfg.tj, cfg.spd
    TCH, WPG, CPW, CH = cfg.tch, cfg.wpg, cfg.cpw, cfg.ch
    SRUN = SPD * P  # slots per db run
    assert HID == 64 and CPW % 8 == 0

    # ---- I/O ----
    xT = nc.dram_tensor("xT", [HID, NN], F32, kind="ExternalInput").ap()
    srcw = nc.dram_tensor("srcw", [P, cfg.slots // 16], I16, kind="ExternalInput").ap()
    dlo_pm = nc.dram_tensor("dlo_pm", [P, NG * TCH], BF16, kind="ExternalInput").ap()
    dlo_rep = nc.dram_tensor("dlo_rep", [NG * TJ * P, SRUN], U8, kind="ExternalInput").ap()
    degT = nc.dram_tensor("degT", [P, NT], F32, kind="ExternalInput").ap()
    w_names = ["Wl", "Wr"]
    Wd = {n: nc.dram_tensor(n, [HID, HID], F32, kind="ExternalInput").ap() for n in w_names}
    W1d = nc.dram_tensor("W1dup", [HID, P], F32, kind="ExternalInput").ap()
    W1p = nc.dram_tensor("Wlin1p", [HID, HID + 1], F32, kind="ExternalInput").ap()
    dinvF = nc.dram_tensor("dinvF", [NN], F32, kind="ExternalInput").ap()
    b1c = nc.dram_tensor("b_lin1c", [HID + 1], F32, kind="ExternalInput").ap()
    v_names = ["bn_a", "bn_bp", "att", "b_gat", "p2", "W23"]
    Vd = {n: nc.dram_tensor(n, [HID], F32, kind="ExternalInput").ap() for n in v_names}
    Cd = nc.dram_tensor("Cc", [1], F32, kind="ExternalInput").ap()
    out_d = nc.dram_tensor("out", [NG, 1], F32, kind="ExternalOutput").ap()

    # ---- DRAM scratch (per-graph tables so gathers only depend on their
    # own graph's writes) ----
    utab = [nc.dram_tensor(f"utab{g}", [NPG, P], BF16).ap() for g in range(NG)]
    gtab = [nc.dram_tensor(f"gtab{g}", [NPG, P], BF16).ap() for g in range(NG)]
    xlr_dram = nc.dram_tensor("xlr_dram", [P, NN], BF16).ap()
    ufm_dram = nc.dram_tensor("ufm_dram", [P, NN], BF16).ap()
    sc_dram = nc.dram_tensor("sc_dram", [NN], F32).ap()

    cpool = ctx.enter_context(tc.tile_pool(name="consts", bufs=1))
    mpool = ctx.enter_context(tc.tile_pool(name="main", bufs=1))
    smallps = ctx.enter_context(tc.tile_pool(name="smallps", bufs=1, space="PSUM"))

    # ---- constants ----
    ident = cpool.tile([P, P], F32)
    make_identity(nc, ident[:])
    ones128 = cpool.tile([P, P], F32)
    nc.vector.memset(ones128[:], 1.0)
    nantile = cpool.tile([P, NT], F32)
    nc.vector.memset(nantile[:], float("nan"))
    negbig = cpool.tile([P, NT], F32)
    nc.vector.memset(negbig[:], -1e9)
    io16 = cpool.tile([P, P], I16)
    nc.gpsimd.iota(io16[:], pattern=[[1, P]], base=0, channel_multiplier=0)
    iota_bf = cpool.tile([P, P], BF16)
    nc.vector.tensor_copy(out=iota_bf[:], in_=io16[:])
    ioc16 = cpool.tile([P, 1], I16)
    nc.gpsimd.iota(ioc16[:], pattern=[[0, 1]], base=0, channel_multiplier=1)
    niota_col = cpool.tile([P, 1], F32)
    nc.vector.tensor_scalar(
        out=niota_col[:], in0=ioc16[:], scalar1=-1.0, scalar2=None, op0=OP.mult
    )

    Ws = {}
    for n in w_names:
        t = cpool.tile([HID, HID], F32, tag=f"w_{n}")
        nc.sync.dma_start(out=t[:], in_=Wd[n][:])
        Ws[n] = t
    W1p_sb = cpool.tile([HID, HID + 1], F32, tag="w_Wlin1p")
    nc.sync.dma_start(out=W1p_sb[:], in_=W1p[:])
    W1d_sb = cpool.tile([HID, P], F32, tag="w_W1dup")
    nc.sync.dma_start(out=W1d_sb[:], in_=W1d[:])
    b1c_sb = cpool.tile([HID + 1, 1], F32, tag="v_b1c")
    nc.sync.dma_start(out=b1c_sb[:], in_=b1c[:, None])
    Vs = {}
    for n in v_names:
        t = cpool.tile([HID, 1], F32, tag=f"v_{n}")
        nc.sync.dma_start(out=t[:], in_=Vd[n][:, None])
        Vs[n] = t
    att_rep = cpool.tile([P, HID], BF16)
    nc.gpsimd.dma_start(out=att_rep[:], in_=Vd["att"][None, :].to_broadcast([P, HID]))
    p2_rep = cpool.tile([P, HID], F32)
    nc.sync.dma_start(out=p2_rep[:], in_=Vd["p2"][None, :].to_broadcast([P, HID]))
    bgat_rep = cpool.tile([P, HID], F32)
    nc.sync.dma_start(out=bgat_rep[:], in_=Vd["b_gat"][None, :].to_broadcast([P, HID]))
    Cc_sb = cpool.tile([NG, 1], F32)
    nc.sync.dma_start(out=Cc_sb[:], in_=Cd[None, :].to_broadcast([NG, 1]))

    # whole per-chunk dst-low-bit table (for one-hot builds in [e,d] orientation)
    dlo_sb = cpool.tile([P, NG * TCH], BF16)
    nc.sync.dma_start(out=dlo_sb[:], in_=dlo_pm[:])

    # ---- dinv from host degree counts ----
    dinv_t = mpool.tile([P, NT], F32, tag="dinv_t")
    sqd_t = mpool.tile([P, NT], F32, tag="sqd_t")
    ntmp = mpool.tile([P, NT], F32, tag="ntmp")

    def recip_newton(r_ap, x_ap, tmp_ap):
        nc.vector.tensor_tensor(out=tmp_ap, in0=x_ap, in1=r_ap, op=OP.mult)
        nc.vector.tensor_scalar(
            out=tmp_ap, in0=tmp_ap, scalar1=-1.0, scalar2=2.0, op0=OP.mult, op1=OP.add
        )
        nc.vector.tensor_tensor(out=r_ap, in0=r_ap, in1=tmp_ap, op=OP.mult)

    deg_sb = mpool.tile([P, NT], F32, tag="deg_sb")
    nc.sync.dma_start(out=deg_sb[:], in_=degT[:])
    nc.scalar.sqrt(out=sqd_t[:], in_=deg_sb[:])
    nc.vector.reciprocal(out=dinv_t[:], in_=sqd_t[:])
    recip_newton(dinv_t[:], sqd_t[:], ntmp[:])

    def idx_slice(pool, w_global, tag):
        t = pool.tile([P, CH // 16], I16, tag=tag)
        c0 = w_global * (CH // 16)
        nc.sync.dma_start(out=t[:], in_=srcw[:, c0 : c0 + CH // 16])
        return t

    # ---- pools (all phases interleave; PSUM budget: 2+1+2+2+1 = 8 banks) ----
    ups = ctx.enter_context(tc.tile_pool(name="ups", bufs=2))
    ubc = ctx.enter_context(tc.tile_pool(name="ubc", bufs=1))
    gep = ctx.enter_context(tc.tile_pool(name="gep", bufs=5))
    ohp = ctx.enter_context(tc.tile_pool(name="ohp", bufs=2))
    hps_pool = ctx.enter_context(tc.tile_pool(name="hpsp", bufs=1, space="PSUM"))
    dsa = ctx.enter_context(tc.tile_pool(name="dsa", bufs=1))
    dss = ctx.enter_context(tc.tile_pool(name="dss", bufs=2))
    bis = ctx.enter_context(tc.tile_pool(name="bis", bufs=2))
    gap = ctx.enter_context(tc.tile_pool(name="gap", bufs=3))
    aohp = ctx.enter_context(tc.tile_pool(name="aohp", bufs=2))
    runtmp = ctx.enter_context(tc.tile_pool(name="runtmp", bufs=1))
    runp = ctx.enter_context(tc.tile_pool(name="runp", bufs=2))
    gatps = ctx.enter_context(tc.tile_pool(name="gatps", bufs=1, space="PSUM"))
    mxrp = ctx.enter_context(tc.tile_pool(name="mxrp", bufs=2, space="PSUM"))
    gfin = ctx.enter_context(tc.tile_pool(name="gfin", bufs=1))

    # persistent t-space tiles
    h2t = mpool.tile([P, NT, HID], F32, tag="bigA")
    gstage = mpool.tile([P, NT, P], BF16, tag="bigC")
    score1_t = mpool.tile([P, NT], F32, tag="score1_t")
    tanh1 = mpool.tile([P, NT], F32, tag="tanh1")
    gate1 = mpool.tile([P, NT], F32, tag="gate1")
    gate1z = mpool.tile([P, NT], F32, tag="gate1z")
    padd = mpool.tile([P, NT], F32, tag="padd")
    kept1 = mpool.tile([P, NT], U8, tag="kept1")
    pb = cfg.psign * cfg.pb_mag

    hps_tiles = {}
    nmps_tiles = {}

    # ======== per-graph sections ========

    def utab_build(g):
        # xw (duplicated rows) -> dinv scale -> [hi|lo] bf16 split, all in
        # feature-major; node-major utab rows via one transpose DMA hop.
        xw2 = ups.tile([P, NPG], F32, tag="xw2", name=f"xw2_{g}")
        for jl in range(TJ):
            xTg = ups.tile([HID, P], F32, tag="xTg", name=f"xTg{g}_{jl}")
            nc.scalar.dma_start(
                out=xTg[:], in_=xT[:, g * NPG + jl * P : g * NPG + (jl + 1) * P]
            )
            pm = smallps.tile([P, P], F32, tag="smA", name=f"xwps{g}_{jl}")
            nc.tensor.matmul(pm[:], lhsT=W1d_sb[:], rhs=xTg[:], start=True, stop=True)
            nc.scalar.copy(out=xw2[:, jl * P : (jl + 1) * P], in_=pm[:])
        dbc = ubc.tile([P, NPG], F32, tag="dinvbc", name=f"dinvbc{g}")
        nc.scalar.dma_start(
            out=dbc[:], in_=dinvF[g * NPG : (g + 1) * NPG][None, :].to_broadcast([P, NPG])
        )
        nc.vector.tensor_tensor(out=xw2[:], in0=xw2[:], in1=dbc[:], op=OP.mult)
        u2 = ups.tile([P, NPG], BF16, tag="u2", name=f"u2_{g}")
        nc.vector.tensor_copy(out=u2[0:HID, :], in_=xw2[0:HID, :])
        nc.vector.tensor_copy(out=u2[HID:P, :], in_=xw2[HID:P, :])
        nc.vector.tensor_tensor(
            out=u2[HID:P, :], in0=xw2[HID:P, :], in1=u2[HID:P, :], op=OP.subtract
        )
        nc.scalar.dma_start(out=ufm_dram[:, g * NPG : (g + 1) * NPG], in_=u2[:])
        u_nm = ups.tile([P, TJ, P], BF16, tag="u2", name=f"unm{g}")
        nc.sync.dma_start_transpose(
            out=u_nm[:], in_=ufm_dram[:, g * NPG : (g + 1) * NPG]
        )
        nc.scalar.dma_start(
            out=utab[g].rearrange("(j p) f -> p j f", p=P), in_=u_nm[:]
        )

    def gcn_pass(g):
        hps = [
            hps_pool.tile([P, 8, HID], F32, tag=f"hps{t}", name=f"hps{t}_{g}")
            for t in range(2)
        ]
        hps_tiles[g] = hps
        for w in range(WPG):
            wg = g * WPG + w
            ssl = idx_slice(gep, wg, "ssl")
            ub = gep.tile([P, CPW, P], BF16, tag="ub")
            nc.gpsimd.dma_gather(
                out_ap=ub[:], in_ap=utab[g][:], idxs_ap=ssl[:],
                num_idxs=CH, num_idxs_reg=CH, elem_size=P, queue_num=wg % 4,
            )
            for b in range(CPW // 8):
                oh8 = ohp.tile([P, 8, P], BF16, tag="oh8")
                c0 = g * TCH + w * CPW + b * 8
                nc.vector.tensor_tensor(
                    out=oh8[:],
                    in0=iota_bf[:, None, :].to_broadcast([P, 8, P]),
                    in1=dlo_sb[:, c0 : c0 + 8, None].to_broadcast([P, 8, P]),
                    op=OP.is_equal,
                )
                for cl in range(8):
                    c = b * 8 + cl
                    gc = w * CPW + c
                    db, pos = gc // SPD, gc % SPD
                    out_slc = hps[db // 8][:, db % 8, :]
                    nc.tensor.matmul(
                        out_slc, lhsT=oh8[:, cl, :], rhs=ub[:, c, 0:HID],
                        start=(pos == 0), stop=False,
                    )
                    nc.tensor.matmul(
                        out_slc, lhsT=oh8[:, cl, :], rhs=ub[:, c, HID:P],
                        start=False, stop=(pos == SPD - 1),
                    )

    S6C = 512

    def dense_g(g):
        # h = dinv*(hi+lo); BN+leaky (one ACT Lrelu); lin1(+score); xl/xr
        hps = hps_tiles.pop(g)
        gsl = slice(g * TJ, (g + 1) * TJ)
        hsum = dsa.tile([P, TJ, HID], F32, tag="hsum", name=f"hsum{g}")
        for t in range(2):
            nc.vector.tensor_tensor(
                out=hsum[:, t * 8 : (t + 1) * 8, :], in0=hps[t][:],
                in1=dinv_t[:, g * TJ + t * 8 : g * TJ + (t + 1) * 8, None].to_broadcast(
                    [P, 8, HID]
                ),
                op=OP.mult,
            )
        hfm = dsa.tile([HID, NPG], F32, tag="hfm", name=f"hfm{g}")
        for jl in range(TJ):
            pt = smallps.tile([HID, P], F32, tag="smA", name=f"htr{g}_{jl}")
            nc.tensor.transpose(out=pt[:], in_=hsum[:, jl, :], identity=ident[:])
            nc.scalar.copy(out=hfm[:, jl * P : (jl + 1) * P], in_=pt[:])
        nc.scalar.activation(
            out=hfm[:], in_=hfm[:], func=AF.Lrelu, scale=Vs["bn_a"][:],
            bias=Vs["bn_bp"][:], alpha=0.01,
        )
        xlr_g = dss.tile([P, NPG], BF16, tag="xlrg", name=f"xlrg{g}")
        for ol in range(0, NPG, S6C):
            o = g * NPG + ol
            pm = hps_pool.tile([HID + 1, S6C], F32, tag="hps0", name=f"l1ps{o}")
            nc.tensor.matmul(
                pm[:], lhsT=W1p_sb[:], rhs=hfm[:, ol : ol + S6C],
                start=True, stop=True,
            )
            hc = dss.tile([HID + 1, S6C], F32, tag="hc", name=f"hc{o}")
            nc.scalar.activation(
                out=hc[:], in_=pm[:], func=AF.Identity, bias=b1c_sb[:]
            )
            nc.sync.dma_start(out=sc_dram[None, o : o + S6C], in_=hc[HID : HID + 1, :])
            px = hps_pool.tile([P, S6C], F32, tag="hps1", name=f"xlrps{o}")
            nc.tensor.matmul(px[:HID, :], lhsT=Ws["Wl"][:], rhs=hc[0:HID, :], start=True, stop=True)
            nc.tensor.matmul(px[HID:, :], lhsT=Ws["Wr"][:], rhs=hc[0:HID, :], start=True, stop=True)
            nc.scalar.copy(out=xlr_g[:, ol : ol + S6C], in_=px[:])
        nc.scalar.dma_start(out=xlr_dram[:, g * NPG : (g + 1) * NPG], in_=xlr_g[:])

    def bisect_multi(score_slc, ngr, target, tag):
        # score_slc: [P, ngr*TJ]; returns per-graph thresholds lo [P, ngr]
        lo = bis.tile([P, ngr], F32, tag="lo", name=f"lo_{tag}")
        hi = bis.tile([P, ngr], F32, tag="hi", name=f"hi_{tag}")
        mid = bis.tile([P, ngr], F32, tag="mid", name=f"mid_{tag}")
        cmp = bis.tile([P, ngr * TJ], F32, tag="cmp", name=f"cmp_{tag}")
        cred = bis.tile([P, ngr], F32, tag="cred", name=f"cred_{tag}")
        ge = bis.tile([P, ngr], U8, tag="ge", name=f"ge_{tag}")
        lt = bis.tile([P, ngr], U8, tag="lt", name=f"lt_{tag}")
        nc.vector.memset(lo[:], -64.0)
        nc.vector.memset(hi[:], 64.0)
        sc_g = score_slc.rearrange("p (g t) -> p g t", g=ngr)
        cmp_g = cmp[:].rearrange("p (g t) -> p g t", g=ngr)
        for it in range(cfg.n_bisect):
            nc.vector.tensor_tensor(out=mid[:], in0=lo[:], in1=hi[:], op=OP.add)
            nc.vector.tensor_scalar(
                out=mid[:], in0=mid[:], scalar1=0.5, scalar2=None, op0=OP.mult
            )
            nc.vector.tensor_tensor(
                out=cmp_g, in0=sc_g,
                in1=mid[:, :, None].to_broadcast([P, ngr, TJ]), op=OP.is_gt,
            )
            nc.vector.tensor_reduce(out=cred[:], in_=cmp_g, axis=AX.X, op=OP.add)
            cps = smallps.tile([P, ngr], F32, tag="smB", name=f"cnt_{tag}_{it}")
            nc.tensor.matmul(cps[:], lhsT=ones128[:], rhs=cred[:], start=True, stop=True)
            nc.vector.tensor_scalar(
                out=ge[:], in0=cps[:], scalar1=float(target), scalar2=None, op0=OP.is_ge
            )
            nc.vector.tensor_scalar(
                out=lt[:], in0=cps[:], scalar1=float(target), scalar2=None, op0=OP.is_lt
            )
            nc.vector.copy_predicated(out=lo[:], mask=ge[:], data=mid[:])
            nc.vector.copy_predicated(out=hi[:], mask=lt[:], data=mid[:])
        return lo

    NH = NG // 2  # graphs per pooling half

    def pool1_h(h):
        g0 = h * NH
        gsl = slice(g0 * TJ, (g0 + NH) * TJ)
        nc.sync.dma_start(
            out=score1_t[:, gsl],
            in_=sc_dram[g0 * NPG : (g0 + NH) * NPG].rearrange("(j p) -> p j", p=P),
        )
        t1 = bisect_multi(score1_t[:, gsl], NH, cfg.k1, f"p1h{h}")
        nc.vector.tensor_tensor(
            out=kept1[:, gsl].rearrange("p (g t) -> p g t", g=NH),
            in0=score1_t[:, gsl].rearrange("p (g t) -> p g t", g=NH),
            in1=t1[:, :, None].to_broadcast([P, NH, TJ]), op=OP.is_gt,
        )
        nc.scalar.activation(out=tanh1[:, gsl], in_=score1_t[:, gsl], func=AF.Tanh)
        nc.vector.tensor_copy(out=gate1[:, gsl], in_=nantile[:, gsl])
        nc.vector.copy_predicated(out=gate1[:, gsl], mask=kept1[:, gsl], data=tanh1[:, gsl])
        nc.vector.memset(gate1z[:, gsl], 0.0)
        nc.vector.copy_predicated(out=gate1z[:, gsl], mask=kept1[:, gsl], data=tanh1[:, gsl])
        nc.vector.tensor_scalar(
            out=padd[:, gsl], in0=kept1[:, gsl], scalar1=-pb, scalar2=pb,
            op0=OP.mult, op1=OP.add,
        )
        # gtab for the half
        gs = gstage[:, gsl, :]
        nc.sync.dma_start_transpose(
            out=gs, in_=xlr_dram[:, g0 * NPG : (g0 + NH) * NPG]
        )
        nc.vector.tensor_tensor(
            out=gs, in0=gs,
            in1=gate1z[:, gsl, None].to_broadcast([P, NH * TJ, P]), op=OP.mult,
        )
        nc.vector.tensor_tensor(
            out=gs, in0=gs,
            in1=padd[:, gsl, None].to_broadcast([P, NH * TJ, P]), op=OP.add,
        )
        for gg in range(g0, g0 + NH):
            nc.sync.dma_start(
                out=gtab[gg].rearrange("(j p) f -> p j f", p=P),
                in_=gstage[:, gg * TJ : (gg + 1) * TJ, :],
            )

    def gat_pass(g):
        nmps = [
            gatps.tile([P, 8, HID], F32, tag=f"nmps{t}", name=f"nmps{t}_{g}")
            for t in range(2)
        ]
        dnps = smallps.tile([P, TJ], F32, tag="smB", name=f"dnps{g}")
        oh_de = {}
        for w in range(WPG):
            wg = g * WPG + w
            ssl = idx_slice(gap, wg, "assl")
            gx = gap.tile([P, CPW, P], BF16, tag="gx")
            nc.gpsimd.dma_gather(
                out_ap=gx[:], in_ap=gtab[g][:], idxs_ap=ssl[:],
                num_idxs=CH, num_idxs_reg=CH, elem_size=P, queue_num=wg % 4,
            )
            for b in range(CPW // 8):
                oh8 = aohp.tile([P, 8, P], BF16, tag="aoh8")
                c0 = g * TCH + w * CPW + b * 8
                nc.vector.tensor_tensor(
                    out=oh8[:],
                    in0=iota_bf[:, None, :].to_broadcast([P, 8, P]),
                    in1=dlo_sb[:, c0 : c0 + 8, None].to_broadcast([P, 8, P]),
                    op=OP.is_equal,
                )
                mxr = mxrp.tile([P, 8, HID], F32, tag="mxr", name=f"mxr{wg}_{b}")
                for cl in range(8):
                    c = b * 8 + cl
                    gc = w * CPW + c
                    r, pos = gc // SPD, gc % SPD
                    if pos == 0:
                        dlo_bc = runtmp.tile(
                            [P, SRUN], U8, tag="dlobc", name=f"dlobc{g}_{r}"
                        )
                        o = (g * TJ + r) * P
                        nc.sync.dma_start(
                            out=dlo_bc[:], in_=dlo_rep[o : o + P, :]
                        )
                        # one-hot on the Scalar engine: relu(1 - (dlo - d)^2)
                        ohsq = runtmp.tile([P, SRUN], BF16, tag="ohsq", name=f"ohsq{g}_{r}")
                        nc.scalar.activation(
                            out=ohsq[:], in_=dlo_bc[:], func=AF.Square,
                            bias=niota_col[:],
                        )
                        ohr = runp.tile([P, SRUN], BF16, tag="ohde", name=f"ohde{g}_{r}")
                        nc.scalar.activation(
                            out=ohr[:], in_=ohsq[:], func=AF.Relu,
                            bias=1.0, scale=-1.0,
                        )
                        oh_de[r] = ohr
                    nc.tensor.matmul(
                        mxr[:, cl, :],
                        lhsT=oh_de[r][:, pos * P : (pos + 1) * P],
                        rhs=gstage[:, g * TJ + r, HID:P],
                        start=True, stop=True,
                    )
                # e = att . leaky(xl_s + xr_d); w = exp(e); pay = w*xl
                gxs = gx[:, b * 8 : (b + 1) * 8, :]
                z = gap.tile([P, 8, HID], BF16, tag="z")
                nc.vector.tensor_tensor(
                    out=z[:], in0=gxs[:, :, 0:HID], in1=mxr[:], op=OP.add
                )
                nc.vector.scalar_tensor_tensor(
                    out=z[:], in0=z[:], scalar=0.2, in1=z[:], op0=OP.mult, op1=OP.max,
                )
                nc.vector.tensor_tensor(
                    out=z[:], in0=z[:],
                    in1=att_rep[:, None, :].to_broadcast([P, 8, HID]), op=OP.mult,
                )
                e8 = gap.tile([P, 8], F32, tag="e8")
                nc.vector.tensor_reduce(out=e8[:], in_=z[:], axis=AX.X, op=OP.add)
                w8b = gap.tile([P, 8], BF16, tag="w8b")
                nc.scalar.activation(out=w8b[:], in_=e8[:], func=AF.Exp)
                pay = gap.tile([P, 8, HID], BF16, tag="pay")
                nc.vector.tensor_tensor(
                    out=pay[:], in0=gxs[:, :, 0:HID],
                    in1=w8b[:, :, None].to_broadcast([P, 8, HID]), op=OP.mult,
                )
                for cl in range(8):
                    c = b * 8 + cl
                    gc = w * CPW + c
                    db, pos = gc // SPD, gc % SPD
                    nc.tensor.matmul(
                        nmps[db // 8][:, db % 8, :],
                        lhsT=oh8[:, cl, :], rhs=pay[:, cl, :],
                        start=(pos == 0), stop=(pos == SPD - 1),
                    )
                    nc.tensor.matmul(
                        dnps[:, db : db + 1],
                        lhsT=oh8[:, cl, :], rhs=w8b[:, cl : cl + 1],
                        start=(pos == 0), stop=(pos == SPD - 1),
                    )

        # ---- finalize graph g: h2 = leaky(numer/denom + b_gat) ----
        numsb = dsa.tile([P, TJ, HID], F32, tag="hsum", name=f"numsb{g}")
        nc.scalar.copy(out=numsb[:, 0:8, :], in_=nmps[0][:])
        nc.scalar.copy(out=numsb[:, 8:TJ, :], in_=nmps[1][:])
        den = gfin.tile([P, TJ], F32, tag="den", name=f"den{g}")
        rec = gfin.tile([P, TJ], F32, tag="rec", name=f"rec{g}")
        dtmp = gfin.tile([P, TJ], F32, tag="dtmp", name=f"dtmp{g}")
        nc.vector.tensor_scalar(
            out=den[:], in0=dnps[:], scalar1=1e-16, scalar2=None, op0=OP.add
        )
        nc.vector.reciprocal(out=rec[:], in_=den[:])
        recip_newton(rec[:], den[:], dtmp[:])
        hslc = h2t[:, g * TJ : (g + 1) * TJ, :]
        nc.vector.tensor_tensor(
            out=hslc, in0=numsb[:],
            in1=rec[:, :, None].to_broadcast([P, TJ, HID]), op=OP.mult,
        )
        nc.vector.tensor_tensor(
            out=hslc, in0=hslc,
            in1=bgat_rep[:, None, :].to_broadcast([P, TJ, HID]), op=OP.add,
        )
        nc.vector.scalar_tensor_tensor(
            out=hslc, in0=hslc, scalar=0.01, in1=hslc, op0=OP.mult, op1=OP.max
        )

    # ======== emission: utab phase, gcn stream, pools, gat stream ========
    for g in range(NG):
        utab_build(g)
    for g in range(NG):
        gcn_pass(g)
        dense_g(g)
    pool1_h(0)
    pool1_h(1)
    for g in range(NG):
        gat_pass(g)

    # ======== score2 (t-space, blocked), mask to kept1 ========
    score2_t = mpool.tile([P, NT], F32, tag="score2_t")
    for t in range(4):
        tsl = slice(t * 32, (t + 1) * 32)
        blk = ups.tile([P, 32, HID], F32, tag="xTg", name=f"s2blk{t}")
        nc.vector.tensor_tensor(
            out=blk[:], in0=h2t[:, tsl, :],
            in1=p2_rep[:, None, :].to_broadcast([P, 32, HID]), op=OP.mult,
        )
        nc.vector.tensor_reduce(out=score2_t[:, tsl], in_=blk[:], axis=AX.X, op=OP.add)
    kept1_t = mpool.tile([P, NT], U8, tag="kept1_t")
    nc.vector.tensor_tensor(out=kept1_t[:], in0=gate1[:], in1=gate1[:], op=OP.is_equal)
    sc2m = mpool.tile([P, NT], F32, tag="sc2m")
    nc.vector.tensor_copy(out=sc2m[:], in_=negbig[:])
    nc.vector.copy_predicated(out=sc2m[:], mask=kept1_t[:], data=score2_t[:])

    # ======== pool2 threshold + gate2 = tanh * mask ========
    gate2 = mpool.tile([P, NT], F32, tag="gate2")
    t2 = bisect_multi(sc2m[:], NG, cfg.k2, "p2")
    nc.vector.tensor_tensor(
        out=gate2[:].rearrange("p (g t) -> p g t", g=NG),
        in0=sc2m[:].rearrange("p (g t) -> p g t", g=NG),
        in1=t2[:, :, None].to_broadcast([P, NG, TJ]), op=OP.is_gt,
    )
    tanh2 = mpool.tile([P, NT], F32, tag="tanh2")
    sc2c = mpool.tile([P, NT], F32, tag="sc2c")
    nc.vector.tensor_scalar(
        out=sc2c[:], in0=sc2m[:], scalar1=-64.0, scalar2=None, op0=OP.max
    )
    nc.scalar.activation(out=tanh2[:], in_=sc2c[:], func=AF.Tanh)
    nc.vector.tensor_tensor(out=gate2[:], in0=gate2[:], in1=tanh2[:], op=OP.mult)

    # ======== T_g = sum_n gate2[n] * h2[n]; out = T @ W23 + C ========
    Tps = smallps.tile([P, NG], F32, tag="smB")
    for j in range(NT):
        g = j // TJ
        nc.tensor.matmul(
            Tps[:HID, g : g + 1], lhsT=h2t[:, j, :], rhs=gate2[:, j : j + 1],
            start=(j % TJ == 0), stop=(j % TJ == TJ - 1),
        )
    Tsb = mpool.tile([HID, NG], F32, tag="Tsb")
    nc.scalar.copy(out=Tsb[:], in_=Tps[:HID, :])
    hps2 = smallps.tile([NG, 1], F32, tag="smB")
    nc.tensor.matmul(hps2[:], lhsT=Tsb[:], rhs=Vs["W23"][:], start=True, stop=True)
    outsb = mpool.tile([NG, 1], F32, tag="outsb")
    nc.vector.tensor_tensor(out=outsb[:], in0=hps2[:], in1=Cc_sb[:], op=OP.add)
    nc.sync.dma_start(out=out_d[:], in_=outsb[:])


# ================= host side =================

def _wrap_idx(ix: np.ndarray) -> np.ndarray:
    n = ix.shape[0]
    w = ix.reshape(n // 16, 16).T.astype(np.int16)
    return np.tile(w, (8, 1)).copy()


def _prep_weights(cfg, W1, b1, bn_gamma, bn_beta, bn_mean, bn_var, W_lin1, b_lin1,
                  p1, Wl, Wr, att, b_gat, p2, W_lin2, b_lin2, W_lin3, b_lin3):
    f32 = np.float32
    bn_a = (bn_gamma / np.sqrt(bn_var + 1e-5)).astype(f32)
    bn_b = (bn_beta - bn_mean * bn_a).astype(f32)
    W23 = (W_lin2 @ W_lin3).reshape(-1).astype(f32)
    Cc = np.array([cfg.k2 * float(b_lin2 @ W_lin3[:, 0]) + float(b_lin3[0])], dtype=f32)
    p1n = (np.asarray(p1) / np.linalg.norm(np.asarray(p1))).astype(np.float64)
    Wlin1p = np.concatenate(
        [np.asarray(W_lin1, np.float64),
         (np.asarray(W_lin1, np.float64) @ p1n)[:, None]], axis=1
    ).astype(f32)
    c1 = np.array([float(p1n @ np.asarray(b_lin1, np.float64))], dtype=f32)
    return {
        "W1dup": np.ascontiguousarray(
            np.concatenate([np.asarray(W1, f32)] * 2, axis=1)
        ),
        "Wlin1p": Wlin1p,
        "Wl": np.ascontiguousarray(Wl, f32), "Wr": np.ascontiguousarray(Wr, f32),
        "bn_a": bn_a, "bn_bp": (np.asarray(b1, f32) * bn_a + bn_b).astype(f32),
        "b_lin1c": np.concatenate([np.asarray(b_lin1, f32), c1]),
        "att": np.ascontiguousarray(att, f32), "b_gat": np.ascontiguousarray(b_gat, f32),
        "p2": (np.asarray(p2) / np.linalg.norm(np.asarray(p2))).astype(f32),
        "W23": W23, "Cc": Cc,
    }


def _prep_core_edges(cfg: Cfg, src_core, dst_core):
    """src/dst core-local [ne]. Per graph: append self loops, bucket edges by
    dst block (db = dst>>7), pad each db run to spd*128 slots. Pad slots get
    src=0 (any valid row; killed by the one-hot) and dlo=255 (matches no
    iota value -> all-zero one-hot row/column)."""
    SPD, SRUN = cfg.spd, cfg.spd * P
    loops = np.arange(cfg.npg, dtype=np.int64)
    src_slots = np.zeros((cfg.ng, cfg.tj, SRUN), np.int64)
    dlo_slots = np.full((cfg.ng, cfg.tj, SRUN), 255, np.int64)
    deg = np.zeros((cfg.ng, cfg.npg), np.int64)
    for g in range(cfg.ng):
        e = slice(g * cfg.eg, (g + 1) * cfg.eg)
        s = np.concatenate([src_core[e] - g * cfg.npg, loops])
        d = np.concatenate([dst_core[e] - g * cfg.npg, loops])
        deg[g] = np.bincount(d, minlength=cfg.npg)
        db = d >> 7
        for b in range(cfg.tj):
            m = db == b
            cnt = int(m.sum())
            assert cnt <= SRUN, f"db run overflow: {cnt} > {SRUN}"
            src_slots[g, b, :cnt] = s[m]
            dlo_slots[g, b, :cnt] = d[m] & 127
    stream_src = src_slots.reshape(-1)
    stream_dlo = dlo_slots.reshape(-1)
    deg_t = np.ascontiguousarray(
        deg.reshape(cfg.ng, cfg.tj, P).transpose(2, 0, 1).reshape(P, cfg.nt)
    ).astype(np.float32)
    bf16 = ml_dtypes.bfloat16
    dinv = (1.0 / np.sqrt(np.maximum(deg.reshape(-1), 1.0))).astype(np.float32)
    return {
        "srcw": _wrap_idx(stream_src),
        "dinvF": dinv,
        "dlo_pm": np.ascontiguousarray(
            stream_dlo.reshape(-1, P).T.astype(bf16)
        ),
        "dlo_rep": np.ascontiguousarray(
            np.repeat(
                dlo_slots.reshape(cfg.ng * cfg.tj, 1, SRUN), P, axis=1
            ).astype(np.uint8)
        ),
        "degT": deg_t,
    }


def build_bass(cfg: Cfg):
    from contextlib import ExitStack
    nc = bacc.Bacc("TRN2", target_bir_lowering=False, debug=False,
                   num_swdge_queues=4)
    with tile.TileContext(nc) as tc:
        with ExitStack() as ctx:
            build_core_program(ctx, tc, cfg)
    nc.compile()
    return nc


_CFG = Cfg()
_NC_CACHE = {}
TRACE = False
LAST_RESULT = None


def kernel(x, edge_index, batch, W1, b1, bn_gamma, bn_beta, bn_mean, bn_var,
           W_lin1, b_lin1, p1, Wl, Wr, att, b_gat, p2,
           W_lin2, b_lin2, W_lin3, b_lin3):
    cfg = _CFG
    n_cores = 8
    s_att = float(np.sum(np.asarray(att, dtype=np.float64)))
    assert abs(s_att) > 1e-6, "degenerate att sum; poison scheme needs |sum(att)|>0"
    cfg.psign = -1.0 if s_att > 0 else 1.0
    slope = 0.2 if s_att > 0 else 1.0
    cfg.pb_mag = 40.0 / (slope * abs(s_att))
    weights = _prep_weights(cfg, W1, b1, bn_gamma, bn_beta, bn_mean, bn_var,
                            W_lin1, b_lin1, p1, Wl, Wr, att, b_gat, p2,
                            W_lin2, b_lin2, W_lin3, b_lin3)
    src_all = np.asarray(edge_index[0], dtype=np.int64)
    dst_all = np.asarray(edge_index[1], dtype=np.int64)
    x = np.asarray(x, dtype=np.float32)

    # choose the chunks-per-db-run capacity from the data (global max so the
    # single SPMD program fits every core)
    max_run = 0
    for c in range(n_cores):
        for g in range(cfg.ng):
            e0 = c * cfg.ne + g * cfg.eg
            d = dst_all[e0 : e0 + cfg.eg] - (c * cfg.nn + g * cfg.npg)
            cnts = np.bincount(d >> 7, minlength=cfg.tj) + P  # + self loops
            max_run = max(max_run, int(cnts.max()))
    cfg.spd = (max_run + P - 1) // P
    # windows of ch slots must tile a graph's slot range exactly
    while (cfg.tj * cfg.spd * P) % cfg.ch != 0:
        cfg.spd += 1

    in_maps = []
    for c in range(n_cores):
        n0 = c * cfg.nn
        e0 = c * cfg.ne
        d = dict(weights)
        d.update(
            _prep_core_edges(
                cfg, src_all[e0 : e0 + cfg.ne] - n0, dst_all[e0 : e0 + cfg.ne] - n0
            )
        )
        d["xT"] = np.ascontiguousarray(x[n0 : n0 + cfg.nn].T, np.float32)
        in_maps.append(d)

    key = ("nc", cfg.spd, cfg.psign, cfg.pb_mag)
    if key not in _NC_CACHE:
        _NC_CACHE[key] = build_bass(cfg)
    nc = _NC_CACHE[key]
    global LAST_RESULT
    res = run_bass_kernel_spmd(nc, in_maps, core_ids=list(range(n_cores)), trace=TRACE)
    LAST_RESULT = res
    outs = [np.asarray(res.results[c]["out"]).reshape(cfg.ng, 1) for c in range(n_cores)]
    return np.concatenate(outs, axis=0).astype(np.float32)


# revision 47
# speedup vs baseline: 1.5875x; 1.1818x over previous
"""Trainium2 Bass kernel for nn_AGNN_EFG (GCN -> TopK pool -> GATv2 -> TopK pool -> head).

Self-contained: shards the B=64 graphs across 8 NeuronCores (8 graphs/core),
runs one SPMD Bass program, gathers the [64, 1] head output on host.

v4 design:
- Edges (incl self loops) sorted per graph by dst block (db = dst>>7), each
  db run padded to a fixed spd chunks of 128 -> static chunk->db schedule
  (SPMD-safe; spd chosen on host from the data).
- Scatter-adds are ONE-HOT MATMULS on PE: per chunk a [128e,128d] bf16
  one-hot (DVE is_equal; pad slots carry sentinel 255 -> zero rows)
  accumulates messages into per-graph PSUM. No dma_scatter_add.
- GAT's xr[dst] is a [128d,128e]-orientation one-hot matmul (one-hot built
  on the Scalar engine as relu(1-(dlo-d)^2)) reading xr blocks from SBUF.
- Only 2 indirect passes remain (GpSimd Q7 descriptor generation is the
  machine bottleneck): gather u[src] (GCN) and [xl|xr][src] (GAT).
- Fully per-graph pipelined: utab(g+2) build, dense/pool/gtab(g),
  gcn(g+1) and gat(g) interleave so the GpSimd gather stream never idles.
- GCN u rows split [u_hi|u_lo] bf16 (~f32 accuracy, one 128-wide matmul).
- score1 fused into the lin1 matmul via host-folded [W_lin1 | W_lin1@p1n].
- Degrees are host-precomputed index data (bincount of dst); rsqrt on device.
"""

import sys

sys.path.insert(0, "/opt/trn_rl_repo")

from dataclasses import dataclass

import numpy as np
import ml_dtypes

import concourse.bass as bass
import concourse.mybir as mybir
import concourse.tile as tile
from concourse import bacc
from concourse.bass_utils import run_bass_kernel_spmd
from concourse.masks import make_identity

P = 128
F32 = mybir.dt.float32
BF16 = mybir.dt.bfloat16
I16 = mybir.dt.int16
U8 = mybir.dt.uint8
AF = mybir.ActivationFunctionType
OP = mybir.AluOpType
AX = mybir.AxisListType


@dataclass
class Cfg:
    ng: int = 8          # graphs per core
    npg: int = 2048      # nodes per graph
    hid: int = 64        # feature dim
    eg: int = 32768      # edges per graph (original, without self loops)
    spd: int = 19        # chunks (of 128 slots) per dst-block run; set at runtime
    ch: int = 1024       # gather window (slots per dma_gather call; >=2048
                         # overflows the SWDGE descriptor ring and faults)
    n_bisect: int = 30   # bisection iterations for topk threshold
    psign: float = -1.0  # -sign(sum(att)): poison sign so poisoned e < 0
    pb_mag: float = 200.0  # poison magnitude; set so poisoned e ~ -40 (LUT-safe)

    @property
    def nn(self):
        return self.ng * self.npg

    @property
    def ne(self):
        return self.ng * self.eg

    @property
    def tj(self):
        return self.npg // P  # dst blocks per graph (16)

    @property
    def nt(self):
        return self.ng * self.tj  # 128

    @property
    def tch(self):
        return self.tj * self.spd  # chunks per graph

    @property
    def slots_g(self):
        return self.tch * P  # padded edge slots per graph

    @property
    def slots(self):
        return self.ng * self.slots_g

    @property
    def wpg(self):
        assert self.slots_g % self.ch == 0
        return self.slots_g // self.ch  # gather windows per graph

    @property
    def cpw(self):
        return self.ch // P  # chunks per window

    @property
    def k1(self):
        return self.npg // 2

    @property
    def k2(self):
        return self.npg // 4


def build_core_program(ctx, tc, cfg: Cfg):
    nc = tc.nc
    NG, NPG, HID, NN = cfg.ng, cfg.npg, cfg.hid, cfg.nn
    NT, TJ, SPD = cfg.nt, cfg.tj, cfg.spd
    TCH, WPG, CPW, CH = cfg.tch, cfg.wpg, cfg.cpw, cfg.ch
    SRUN = SPD * P  # slots per db run
    assert HID == 64 and CPW % 8 == 0

    # ---- I/O ----
    xT = nc.dram_tensor("xT", [HID, NN], F32, kind="ExternalInput").ap()
    srcw = nc.dram_tensor("srcw", [P, cfg.slots // 16], I16, kind="ExternalInput").ap()
    dlo_pm = nc.dram_tensor("dlo_pm", [P, NG * TCH], BF16, kind="ExternalInput").ap()
    dlo_rep = nc.dram_tensor("dlo_rep", [NG * TJ * P, SRUN], U8, kind="ExternalInput").ap()
    degT = nc.dram_tensor("degT", [P, NT], F32, kind="ExternalInput").ap()
    w_names = ["Wl", "Wr"]
    Wd = {n: nc.dram_tensor(n, [HID, HID], F32, kind="ExternalInput").ap() for n in w_names}
    W1d = nc.dram_tensor("W1dup", [HID, P], F32, kind="ExternalInput").ap()
    W1p = nc.dram_tensor("Wlin1p", [HID, HID + 1], F32, kind="ExternalInput").ap()
    dinvF = nc.dram_tensor("dinvF", [NN], F32, kind="ExternalInput").ap()
    b1c = nc.dram_tensor("b_lin1c", [HID + 1], F32, kind="ExternalInput").ap()
    v_names = ["bn_a", "bn_bp", "att", "b_gat", "p2", "W23"]
    Vd = {n: nc.dram_tensor(n, [HID], F32, kind="ExternalInput").ap() for n in v_names}
    Cd = nc.dram_tensor("Cc", [1], F32, kind="ExternalInput").ap()
    out_d = nc.dram_tensor("out", [NG, 1], F32, kind="ExternalOutput").ap()

    # ---- DRAM scratch (per-graph tables so gathers only depend on their
    # own graph's writes) ----
    utab = [nc.dram_tensor(f"utab{g}", [NPG, P], BF16).ap() for g in range(NG)]
    gtab = [nc.dram_tensor(f"gtab{g}", [NPG, P], BF16).ap() for g in range(NG)]
    xlr_dram = nc.dram_tensor("xlr_dram", [P, NN], BF16).ap()
    ufm_dram = nc.dram_tensor("ufm_dram", [P, NN], BF16).ap()
    sc_dram = nc.dram_tensor("sc_dram", [NN], F32).ap()

    cpool = ctx.enter_context(tc.tile_pool(name="consts", bufs=1))
    mpool = ctx.enter_context(tc.tile_pool(name="main", bufs=1))
    smallps = ctx.enter_context(tc.tile_pool(name="smallps", bufs=1, space="PSUM"))

    # ---- constants ----
    ident = cpool.tile([P, P], F32)
    make_identity(nc, ident[:])
    ones128 = cpool.tile([P, P], F32)
    nc.vector.memset(ones128[:], 1.0)
    nantile = cpool.tile([P, NT], F32)
    nc.vector.memset(nantile[:], float("nan"))
    negbig = cpool.tile([P, NT], F32)
    nc.vector.memset(negbig[:], -1e9)
    io16 = cpool.tile([P, P], I16)
    nc.gpsimd.iota(io16[:], pattern=[[1, P]], base=0, channel_multiplier=0)
    iota_bf = cpool.tile([P, P], BF16)
    nc.vector.tensor_copy(out=iota_bf[:], in_=io16[:])
    ioc16 = cpool.tile([P, 1], I16)
    nc.gpsimd.iota(ioc16[:], pattern=[[0, 1]], base=0, channel_multiplier=1)
    niota_col = cpool.tile([P, 1], F32)
    nc.vector.tensor_scalar(
        out=niota_col[:], in0=ioc16[:], scalar1=-1.0, scalar2=None, op0=OP.mult
    )

    Ws = {}
    for n in w_names:
        t = cpool.tile([HID, HID], F32, tag=f"w_{n}")
        nc.sync.dma_start(out=t[:], in_=Wd[n][:])
        Ws[n] = t
    W1p_sb = cpool.tile([HID, HID + 1], F32, tag="w_Wlin1p")
    nc.sync.dma_start(out=W1p_sb[:], in_=W1p[:])
    W1d_sb = cpool.tile([HID, P], F32, tag="w_W1dup")
    nc.sync.dma_start(out=W1d_sb[:], in_=W1d[:])
    b1c_sb = cpool.tile([HID + 1, 1], F32, tag="v_b1c")
    nc.sync.dma_start(out=b1c_sb[:], in_=b1c[:, None])
    Vs = {}
    for n in v_names:
        t = cpool.tile([HID, 1], F32, tag=f"v_{n}")
        nc.sync.dma_start(out=t[:], in_=Vd[n][:, None])
        Vs[n] = t
    att_rep = cpool.tile([P, HID], BF16)
    nc.gpsimd.dma_start(out=att_rep[:], in_=Vd["att"][None, :].to_broadcast([P, HID]))
    p2_rep = cpool.tile([P, HID], F32)
    nc.sync.dma_start(out=p2_rep[:], in_=Vd["p2"][None, :].to_broadcast([P, HID]))
    bgat_rep = cpool.tile([P, HID], F32)
    nc.sync.dma_start(out=bgat_rep[:], in_=Vd["b_gat"][None, :].to_broadcast([P, HID]))
    Cc_sb = cpool.tile([NG, 1], F32)
    nc.sync.dma_start(out=Cc_sb[:], in_=Cd[None, :].to_broadcast([NG, 1]))

    # whole per-chunk dst-low-bit table (for one-hot builds in [e,d] orientation)
    dlo_sb = cpool.tile([P, NG * TCH], BF16)
    nc.sync.dma_start(out=dlo_sb[:], in_=dlo_pm[:])

    # ---- dinv from host degree counts ----
    dinv_t = mpool.tile([P, NT], F32, tag="dinv_t")
    sqd_t = mpool.tile([P, NT], F32, tag="sqd_t")
    ntmp = mpool.tile([P, NT], F32, tag="ntmp")

    def recip_newton(r_ap, x_ap, tmp_ap):
        nc.vector.tensor_tensor(out=tmp_ap, in0=x_ap, in1=r_ap, op=OP.mult)
        nc.vector.tensor_scalar(
            out=tmp_ap, in0=tmp_ap, scalar1=-1.0, scalar2=2.0, op0=OP.mult, op1=OP.add
        )
        nc.vector.tensor_tensor(out=r_ap, in0=r_ap, in1=tmp_ap, op=OP.mult)

    deg_sb = mpool.tile([P, NT], F32, tag="deg_sb")
    nc.sync.dma_start(out=deg_sb[:], in_=degT[:])
    nc.scalar.sqrt(out=sqd_t[:], in_=deg_sb[:])
    nc.vector.reciprocal(out=dinv_t[:], in_=sqd_t[:])
    recip_newton(dinv_t[:], sqd_t[:], ntmp[:])

    def idx_slice(pool, w_global, tag):
        t = pool.tile([P, CH // 16], I16, tag=tag)
        c0 = w_global * (CH // 16)
        nc.sync.dma_start(out=t[:], in_=srcw[:, c0 : c0 + CH // 16])
        return t

    # ---- pools (all phases interleave; PSUM budget: 2+1+2+2+1 = 8 banks) ----
    gep = ctx.enter_context(tc.tile_pool(name="gep", bufs=6))
    ohp = ctx.enter_context(tc.tile_pool(name="ohp", bufs=3))
    hps_pool = ctx.enter_context(tc.tile_pool(name="hpsp", bufs=1, space="PSUM"))
    dsa = ctx.enter_context(tc.tile_pool(name="dsa", bufs=1))
    dss = ctx.enter_context(tc.tile_pool(name="dss", bufs=2))
    bis = ctx.enter_context(tc.tile_pool(name="bis", bufs=2))
    gap = ctx.enter_context(tc.tile_pool(name="gap", bufs=6))
    aohp = ctx.enter_context(tc.tile_pool(name="aohp", bufs=3))
    runtmp = ctx.enter_context(tc.tile_pool(name="runtmp", bufs=1))
    runp = ctx.enter_context(tc.tile_pool(name="runp", bufs=2))
    gatps = ctx.enter_context(tc.tile_pool(name="gatps", bufs=1, space="PSUM"))
    mxrp = ctx.enter_context(tc.tile_pool(name="mxrp", bufs=2, space="PSUM"))
    gfin = ctx.enter_context(tc.tile_pool(name="gfin", bufs=1))
    from contextlib import ExitStack as _ES
    phase_u = _ES()
    ups = phase_u.enter_context(tc.tile_pool(name="ups", bufs=2))
    ubc = phase_u.enter_context(tc.tile_pool(name="ubc", bufs=1))

    # persistent t-space tiles
    h2t = mpool.tile([P, NT, HID], F32, tag="bigA")
    gstage = mpool.tile([P, NT, P], BF16, tag="bigC")
    score1_t = mpool.tile([P, NT], F32, tag="score1_t")
    tanh1 = mpool.tile([P, NT], F32, tag="tanh1")
    gate1 = mpool.tile([P, NT], F32, tag="gate1")
    gate1z = mpool.tile([P, NT], F32, tag="gate1z")
    padd = mpool.tile([P, NT], F32, tag="padd")
    kept1 = mpool.tile([P, NT], U8, tag="kept1")
    pb = cfg.psign * cfg.pb_mag

    hps_tiles = {}
    nmps_tiles = {}

    # ======== per-graph sections ========

    def utab_build(g):
        # xw (duplicated rows) -> dinv scale -> [hi|lo] bf16 split, all in
        # feature-major; node-major utab rows via one transpose DMA hop.
        xw2 = ups.tile([P, NPG], F32, tag="xw2", name=f"xw2_{g}")
        for jl in range(TJ):
            xTg = ups.tile([HID, P], F32, tag="xTg", name=f"xTg{g}_{jl}")
            nc.scalar.dma_start(
                out=xTg[:], in_=xT[:, g * NPG + jl * P : g * NPG + (jl + 1) * P]
            )
            pm = smallps.tile([P, P], F32, tag="smA", name=f"xwps{g}_{jl}")
            nc.tensor.matmul(pm[:], lhsT=W1d_sb[:], rhs=xTg[:], start=True, stop=True)
            nc.scalar.copy(out=xw2[:, jl * P : (jl + 1) * P], in_=pm[:])
        dbc = ubc.tile([P, NPG], F32, tag="dinvbc", name=f"dinvbc{g}")
        nc.scalar.dma_start(
            out=dbc[:], in_=dinvF[g * NPG : (g + 1) * NPG][None, :].to_broadcast([P, NPG])
        )
        nc.vector.tensor_tensor(out=xw2[:], in0=xw2[:], in1=dbc[:], op=OP.mult)
        u2 = ups.tile([P, NPG], BF16, tag="u2", name=f"u2_{g}")
        nc.vector.tensor_copy(out=u2[0:HID, :], in_=xw2[0:HID, :])
        nc.vector.tensor_copy(out=u2[HID:P, :], in_=xw2[HID:P, :])
        nc.vector.tensor_tensor(
            out=u2[HID:P, :], in0=xw2[HID:P, :], in1=u2[HID:P, :], op=OP.subtract
        )
        nc.scalar.dma_start(out=ufm_dram[:, g * NPG : (g + 1) * NPG], in_=u2[:])
        u_nm = ups.tile([P, TJ, P], BF16, tag="u2", name=f"unm{g}")
        nc.sync.dma_start_transpose(
            out=u_nm[:], in_=ufm_dram[:, g * NPG : (g + 1) * NPG]
        )
        nc.scalar.dma_start(
            out=utab[g].rearrange("(j p) f -> p j f", p=P), in_=u_nm[:]
        )

    def gcn_pass(g):
        hps = [
            hps_pool.tile([P, 8, HID], F32, tag=f"hps{t}", name=f"hps{t}_{g}")
            for t in range(2)
        ]
        hps_tiles[g] = hps
        for w in range(WPG):
            wg = g * WPG + w
            ssl = idx_slice(gep, wg, "ssl")
            ub = gep.tile([P, CPW, P], BF16, tag="ub")
            nc.gpsimd.dma_gather(
                out_ap=ub[:], in_ap=utab[g][:], idxs_ap=ssl[:],
                num_idxs=CH, num_idxs_reg=CH, elem_size=P, queue_num=wg % 4,
            )
            for b in range(CPW // 8):
                oh8 = ohp.tile([P, 8, P], BF16, tag="oh8")
                c0 = g * TCH + w * CPW + b * 8
                nc.vector.tensor_tensor(
                    out=oh8[:],
                    in0=iota_bf[:, None, :].to_broadcast([P, 8, P]),
                    in1=dlo_sb[:, c0 : c0 + 8, None].to_broadcast([P, 8, P]),
                    op=OP.is_equal,
                )
                for cl in range(8):
                    c = b * 8 + cl
                    gc = w * CPW + c
                    db, pos = gc // SPD, gc % SPD
                    out_slc = hps[db // 8][:, db % 8, :]
                    nc.tensor.matmul(
                        out_slc, lhsT=oh8[:, cl, :], rhs=ub[:, c, 0:HID],
                        start=(pos == 0), stop=False,
                    )
                    nc.tensor.matmul(
                        out_slc, lhsT=oh8[:, cl, :], rhs=ub[:, c, HID:P],
                        start=False, stop=(pos == SPD - 1),
                    )

    S6C = 512

    def dense_g(g):
        # h = dinv*(hi+lo); BN+leaky (one ACT Lrelu); lin1(+score); xl/xr
        hps = hps_tiles.pop(g)
        gsl = slice(g * TJ, (g + 1) * TJ)
        hsum = dsa.tile([P, TJ, HID], F32, tag="hsum", name=f"hsum{g}")
        for t in range(2):
            nc.vector.tensor_tensor(
                out=hsum[:, t * 8 : (t + 1) * 8, :], in0=hps[t][:],
                in1=dinv_t[:, g * TJ + t * 8 : g * TJ + (t + 1) * 8, None].to_broadcast(
                    [P, 8, HID]
                ),
                op=OP.mult,
            )
        hfm = dsa.tile([HID, NPG], F32, tag="hfm", name=f"hfm{g}")
        for jl in range(TJ):
            pt = smallps.tile([HID, P], F32, tag="smA", name=f"htr{g}_{jl}")
            nc.tensor.transpose(out=pt[:], in_=hsum[:, jl, :], identity=ident[:])
            nc.scalar.copy(out=hfm[:, jl * P : (jl + 1) * P], in_=pt[:])
        nc.scalar.activation(
            out=hfm[:], in_=hfm[:], func=AF.Lrelu, scale=Vs["bn_a"][:],
            bias=Vs["bn_bp"][:], alpha=0.01,
        )
        xlr_g = dss.tile([P, NPG], BF16, tag="xlrg", name=f"xlrg{g}")
        for ol in range(0, NPG, S6C):
            o = g * NPG + ol
            pm = hps_pool.tile([HID + 1, S6C], F32, tag="hps0", name=f"l1ps{o}")
            nc.tensor.matmul(
                pm[:], lhsT=W1p_sb[:], rhs=hfm[:, ol : ol + S6C],
                start=True, stop=True,
            )
            hc = dss.tile([HID + 1, S6C], F32, tag="hc", name=f"hc{o}")
            nc.scalar.activation(
                out=hc[:], in_=pm[:], func=AF.Identity, bias=b1c_sb[:]
            )
            nc.sync.dma_start(out=sc_dram[None, o : o + S6C], in_=hc[HID : HID + 1, :])
            px = hps_pool.tile([P, S6C], F32, tag="hps1", name=f"xlrps{o}")
            nc.tensor.matmul(px[:HID, :], lhsT=Ws["Wl"][:], rhs=hc[0:HID, :], start=True, stop=True)
            nc.tensor.matmul(px[HID:, :], lhsT=Ws["Wr"][:], rhs=hc[0:HID, :], start=True, stop=True)
            nc.scalar.copy(out=xlr_g[:, ol : ol + S6C], in_=px[:])
        nc.scalar.dma_start(out=xlr_dram[:, g * NPG : (g + 1) * NPG], in_=xlr_g[:])

    def bisect_multi(score_slc, ngr, target, tag):
        # score_slc: [P, ngr*TJ]; returns per-graph thresholds lo [P, ngr]
        lo = bis.tile([P, ngr], F32, tag="lo", name=f"lo_{tag}")
        hi = bis.tile([P, ngr], F32, tag="hi", name=f"hi_{tag}")
        mid = bis.tile([P, ngr], F32, tag="mid", name=f"mid_{tag}")
        cmp = bis.tile([P, ngr * TJ], F32, tag="cmp", name=f"cmp_{tag}")
        cred = bis.tile([P, ngr], F32, tag="cred", name=f"cred_{tag}")
        ge = bis.tile([P, ngr], U8, tag="ge", name=f"ge_{tag}")
        lt = bis.tile([P, ngr], U8, tag="lt", name=f"lt_{tag}")
        nc.vector.memset(lo[:], -64.0)
        nc.vector.memset(hi[:], 64.0)
        sc_g = score_slc.rearrange("p (g t) -> p g t", g=ngr)
        cmp_g = cmp[:].rearrange("p (g t) -> p g t", g=ngr)
        for it in range(cfg.n_bisect):
            nc.vector.tensor_tensor(out=mid[:], in0=lo[:], in1=hi[:], op=OP.add)
            nc.vector.tensor_scalar(
                out=mid[:], in0=mid[:], scalar1=0.5, scalar2=None, op0=OP.mult
            )
            nc.vector.tensor_tensor(
                out=cmp_g, in0=sc_g,
                in1=mid[:, :, None].to_broadcast([P, ngr, TJ]), op=OP.is_gt,
            )
            nc.vector.tensor_reduce(out=cred[:], in_=cmp_g, axis=AX.X, op=OP.add)
            cps = smallps.tile([P, ngr], F32, tag="smB", name=f"cnt_{tag}_{it}")
            nc.tensor.matmul(cps[:], lhsT=ones128[:], rhs=cred[:], start=True, stop=True)
            nc.vector.tensor_scalar(
                out=ge[:], in0=cps[:], scalar1=float(target), scalar2=None, op0=OP.is_ge
            )
            nc.vector.tensor_scalar(
                out=lt[:], in0=cps[:], scalar1=float(target), scalar2=None, op0=OP.is_lt
            )
            nc.vector.copy_predicated(out=lo[:], mask=ge[:], data=mid[:])
            nc.vector.copy_predicated(out=hi[:], mask=lt[:], data=mid[:])
        return lo

    NH = NG // 2  # graphs per pooling half

    def pool1_h(h):
        g0 = h * NH
        gsl = slice(g0 * TJ, (g0 + NH) * TJ)
        nc.sync.dma_start(
            out=score1_t[:, gsl],
            in_=sc_dram[g0 * NPG : (g0 + NH) * NPG].rearrange("(j p) -> p j", p=P),
        )
        t1 = bisect_multi(score1_t[:, gsl], NH, cfg.k1, f"p1h{h}")
        nc.vector.tensor_tensor(
            out=kept1[:, gsl].rearrange("p (g t) -> p g t", g=NH),
            in0=score1_t[:, gsl].rearrange("p (g t) -> p g t", g=NH),
            in1=t1[:, :, None].to_broadcast([P, NH, TJ]), op=OP.is_gt,
        )
        nc.scalar.activation(out=tanh1[:, gsl], in_=score1_t[:, gsl], func=AF.Tanh)
        nc.vector.tensor_copy(out=gate1[:, gsl], in_=nantile[:, gsl])
        nc.vector.copy_predicated(out=gate1[:, gsl], mask=kept1[:, gsl], data=tanh1[:, gsl])
        nc.vector.memset(gate1z[:, gsl], 0.0)
        nc.vector.copy_predicated(out=gate1z[:, gsl], mask=kept1[:, gsl], data=tanh1[:, gsl])
        nc.vector.tensor_scalar(
            out=padd[:, gsl], in0=kept1[:, gsl], scalar1=-pb, scalar2=pb,
            op0=OP.mult, op1=OP.add,
        )
        # gtab for the half
        gs = gstage[:, gsl, :]
        nc.sync.dma_start_transpose(
            out=gs, in_=xlr_dram[:, g0 * NPG : (g0 + NH) * NPG]
        )
        nc.vector.tensor_tensor(
            out=gs, in0=gs,
            in1=gate1z[:, gsl, None].to_broadcast([P, NH * TJ, P]), op=OP.mult,
        )
        nc.vector.tensor_tensor(
            out=gs, in0=gs,
            in1=padd[:, gsl, None].to_broadcast([P, NH * TJ, P]), op=OP.add,
        )
        for gg in range(g0, g0 + NH):
            nc.sync.dma_start(
                out=gtab[gg].rearrange("(j p) f -> p j f", p=P),
                in_=gstage[:, gg * TJ : (gg + 1) * TJ, :],
            )

    def gat_pass(g):
        nmps = [
            gatps.tile([P, 8, HID], F32, tag=f"nmps{t}", name=f"nmps{t}_{g}")
            for t in range(2)
        ]
        dnps = smallps.tile([P, TJ], F32, tag="smB", name=f"dnps{g}")
        oh_de = {}
        for w in range(WPG):
            wg = g * WPG + w
            ssl = idx_slice(gap, wg, "assl")
            gx = gap.tile([P, CPW, P], BF16, tag="gx")
            nc.gpsimd.dma_gather(
                out_ap=gx[:], in_ap=gtab[g][:], idxs_ap=ssl[:],
                num_idxs=CH, num_idxs_reg=CH, elem_size=P, queue_num=wg % 4,
            )
            for b in range(CPW // 8):
                oh8 = aohp.tile([P, 8, P], BF16, tag="aoh8")
                c0 = g * TCH + w * CPW + b * 8
                nc.vector.tensor_tensor(
                    out=oh8[:],
                    in0=iota_bf[:, None, :].to_broadcast([P, 8, P]),
                    in1=dlo_sb[:, c0 : c0 + 8, None].to_broadcast([P, 8, P]),
                    op=OP.is_equal,
                )
                mxr = mxrp.tile([P, 8, HID], F32, tag="mxr", name=f"mxr{wg}_{b}")
                for cl in range(8):
                    c = b * 8 + cl
                    gc = w * CPW + c
                    r, pos = gc // SPD, gc % SPD
                    if pos == 0:
                        dlo_bc = runtmp.tile(
                            [P, SRUN], U8, tag="dlobc", name=f"dlobc{g}_{r}"
                        )
                        o = (g * TJ + r) * P
                        nc.sync.dma_start(
                            out=dlo_bc[:], in_=dlo_rep[o : o + P, :]
                        )
                        # one-hot on the Scalar engine: relu(1 - (dlo - d)^2)
                        ohsq = runtmp.tile([P, SRUN], BF16, tag="ohsq", name=f"ohsq{g}_{r}")
                        nc.scalar.activation(
                            out=ohsq[:], in_=dlo_bc[:], func=AF.Square,
                            bias=niota_col[:],
                        )
                        ohr = runp.tile([P, SRUN], BF16, tag="ohde", name=f"ohde{g}_{r}")
                        nc.scalar.activation(
                            out=ohr[:], in_=ohsq[:], func=AF.Relu,
                            bias=1.0, scale=-1.0,
                        )
                        oh_de[r] = ohr
                    nc.tensor.matmul(
                        mxr[:, cl, :],
                        lhsT=oh_de[r][:, pos * P : (pos + 1) * P],
                        rhs=gstage[:, g * TJ + r, HID:P],
                        start=True, stop=True,
                    )
                # e = att . leaky(xl_s + xr_d); w = exp(e); pay = w*xl
                gxs = gx[:, b * 8 : (b + 1) * 8, :]
                z = gap.tile([P, 8, HID], BF16, tag="z")
                nc.vector.tensor_tensor(
                    out=z[:], in0=gxs[:, :, 0:HID], in1=mxr[:], op=OP.add
                )
                nc.vector.scalar_tensor_tensor(
                    out=z[:], in0=z[:], scalar=0.2, in1=z[:], op0=OP.mult, op1=OP.max,
                )
                nc.vector.tensor_tensor(
                    out=z[:], in0=z[:],
                    in1=att_rep[:, None, :].to_broadcast([P, 8, HID]), op=OP.mult,
                )
                e8 = gap.tile([P, 8], F32, tag="e8")
                nc.vector.tensor_reduce(out=e8[:], in_=z[:], axis=AX.X, op=OP.add)
                w8b = gap.tile([P, 8], BF16, tag="w8b")
                nc.scalar.activation(out=w8b[:], in_=e8[:], func=AF.Exp)
                pay = gap.tile([P, 8, HID], BF16, tag="pay")
                nc.vector.tensor_tensor(
                    out=pay[:], in0=gxs[:, :, 0:HID],
                    in1=w8b[:, :, None].to_broadcast([P, 8, HID]), op=OP.mult,
                )
                for cl in range(8):
                    c = b * 8 + cl
                    gc = w * CPW + c
                    db, pos = gc // SPD, gc % SPD
                    nc.tensor.matmul(
                        nmps[db // 8][:, db % 8, :],
                        lhsT=oh8[:, cl, :], rhs=pay[:, cl, :],
                        start=(pos == 0), stop=(pos == SPD - 1),
                    )
                    nc.tensor.matmul(
                        dnps[:, db : db + 1],
                        lhsT=oh8[:, cl, :], rhs=w8b[:, cl : cl + 1],
                        start=(pos == 0), stop=(pos == SPD - 1),
                    )

        # ---- finalize graph g: h2 = leaky(numer/denom + b_gat) ----
        numsb = dsa.tile([P, TJ, HID], F32, tag="hsum", name=f"numsb{g}")
        nc.scalar.copy(out=numsb[:, 0:8, :], in_=nmps[0][:])
        nc.scalar.copy(out=numsb[:, 8:TJ, :], in_=nmps[1][:])
        den = gfin.tile([P, TJ], F32, tag="den", name=f"den{g}")
        rec = gfin.tile([P, TJ], F32, tag="rec", name=f"rec{g}")
        dtmp = gfin.tile([P, TJ], F32, tag="dtmp", name=f"dtmp{g}")
        nc.vector.tensor_scalar(
            out=den[:], in0=dnps[:], scalar1=1e-16, scalar2=None, op0=OP.add
        )
        nc.vector.reciprocal(out=rec[:], in_=den[:])
        recip_newton(rec[:], den[:], dtmp[:])
        hslc = h2t[:, g * TJ : (g + 1) * TJ, :]
        nc.vector.tensor_tensor(
            out=hslc, in0=numsb[:],
            in1=rec[:, :, None].to_broadcast([P, TJ, HID]), op=OP.mult,
        )
        nc.vector.tensor_tensor(
            out=hslc, in0=hslc,
            in1=bgat_rep[:, None, :].to_broadcast([P, TJ, HID]), op=OP.add,
        )
        nc.vector.scalar_tensor_tensor(
            out=hslc, in0=hslc, scalar=0.01, in1=hslc, op0=OP.mult, op1=OP.max
        )

    # ======== emission: utab phase, gcn stream, pools, gat stream ========
    for g in range(NG):
        utab_build(g)
    phase_u.close()
    for g in range(NG):
        gcn_pass(g)
        dense_g(g)
    pool1_h(0)
    pool1_h(1)
    for g in range(NG):
        gat_pass(g)

    # ======== score2 (t-space, blocked), mask to kept1 ========
    score2_t = mpool.tile([P, NT], F32, tag="score2_t")
    for t in range(4):
        tsl = slice(t * 32, (t + 1) * 32)
        blk = dsa.tile([P, 32, HID], F32, tag="hfm", name=f"s2blk{t}")
        nc.vector.tensor_tensor(
            out=blk[:], in0=h2t[:, tsl, :],
            in1=p2_rep[:, None, :].to_broadcast([P, 32, HID]), op=OP.mult,
        )
        nc.vector.tensor_reduce(out=score2_t[:, tsl], in_=blk[:], axis=AX.X, op=OP.add)
    kept1_t = mpool.tile([P, NT], U8, tag="kept1_t")
    nc.vector.tensor_tensor(out=kept1_t[:], in0=gate1[:], in1=gate1[:], op=OP.is_equal)
    sc2m = mpool.tile([P, NT], F32, tag="sc2m")
    nc.vector.tensor_copy(out=sc2m[:], in_=negbig[:])
    nc.vector.copy_predicated(out=sc2m[:], mask=kept1_t[:], data=score2_t[:])

    # ======== pool2 threshold + gate2 = tanh * mask ========
    gate2 = mpool.tile([P, NT], F32, tag="gate2")
    t2 = bisect_multi(sc2m[:], NG, cfg.k2, "p2")
    nc.vector.tensor_tensor(
        out=gate2[:].rearrange("p (g t) -> p g t", g=NG),
        in0=sc2m[:].rearrange("p (g t) -> p g t", g=NG),
        in1=t2[:, :, None].to_broadcast([P, NG, TJ]), op=OP.is_gt,
    )
    tanh2 = mpool.tile([P, NT], F32, tag="tanh2")
    sc2c = mpool.tile([P, NT], F32, tag="sc2c")
    nc.vector.tensor_scalar(
        out=sc2c[:], in0=sc2m[:], scalar1=-64.0, scalar2=None, op0=OP.max
    )
    nc.scalar.activation(out=tanh2[:], in_=sc2c[:], func=AF.Tanh)
    nc.vector.tensor_tensor(out=gate2[:], in0=gate2[:], in1=tanh2[:], op=OP.mult)

    # ======== T_g = sum_n gate2[n] * h2[n]; out = T @ W23 + C ========
    Tps = smallps.tile([P, NG], F32, tag="smB")
    for j in range(NT):
        g = j // TJ
        nc.tensor.matmul(
            Tps[:HID, g : g + 1], lhsT=h2t[:, j, :], rhs=gate2[:, j : j + 1],
            start=(j % TJ == 0), stop=(j % TJ == TJ - 1),
        )
    Tsb = mpool.tile([HID, NG], F32, tag="Tsb")
    nc.scalar.copy(out=Tsb[:], in_=Tps[:HID, :])
    hps2 = smallps.tile([NG, 1], F32, tag="smB")
    nc.tensor.matmul(hps2[:], lhsT=Tsb[:], rhs=Vs["W23"][:], start=True, stop=True)
    outsb = mpool.tile([NG, 1], F32, tag="outsb")
    nc.vector.tensor_tensor(out=outsb[:], in0=hps2[:], in1=Cc_sb[:], op=OP.add)
    nc.sync.dma_start(out=out_d[:], in_=outsb[:])


# ================= host side =================

def _wrap_idx(ix: np.ndarray) -> np.ndarray:
    n = ix.shape[0]
    w = ix.reshape(n // 16, 16).T.astype(np.int16)
    return np.tile(w, (8, 1)).copy()


def _prep_weights(cfg, W1, b1, bn_gamma, bn_beta, bn_mean, bn_var, W_lin1, b_lin1,
                  p1, Wl, Wr, att, b_gat, p2, W_lin2, b_lin2, W_lin3, b_lin3):
    f32 = np.float32
    bn_a = (bn_gamma / np.sqrt(bn_var + 1e-5)).astype(f32)
    bn_b = (bn_beta - bn_mean * bn_a).astype(f32)
    W23 = (W_lin2 @ W_lin3).reshape(-1).astype(f32)
    Cc = np.array([cfg.k2 * float(b_lin2 @ W_lin3[:, 0]) + float(b_lin3[0])], dtype=f32)
    p1n = (np.asarray(p1) / np.linalg.norm(np.asarray(p1))).astype(np.float64)
    Wlin1p = np.concatenate(
        [np.asarray(W_lin1, np.float64),
         (np.asarray(W_lin1, np.float64) @ p1n)[:, None]], axis=1
    ).astype(f32)
    c1 = np.array([float(p1n @ np.asarray(b_lin1, np.float64))], dtype=f32)
    return {
        "W1dup": np.ascontiguousarray(
            np.concatenate([np.asarray(W1, f32)] * 2, axis=1)
        ),
        "Wlin1p": Wlin1p,
        "Wl": np.ascontiguousarray(Wl, f32), "Wr": np.ascontiguousarray(Wr, f32),
        "bn_a": bn_a, "bn_bp": (np.asarray(b1, f32) * bn_a + bn_b).astype(f32),
        "b_lin1c": np.concatenate([np.asarray(b_lin1, f32), c1]),
        "att": np.ascontiguousarray(att, f32), "b_gat": np.ascontiguousarray(b_gat, f32),
        "p2": (np.asarray(p2) / np.linalg.norm(np.asarray(p2))).astype(f32),
        "W23": W23, "Cc": Cc,
    }


def _prep_core_edges(cfg: Cfg, src_core, dst_core):
    """src/dst core-local [ne]. Per graph: append self loops, bucket edges by
    dst block (db = dst>>7), pad each db run to spd*128 slots. Pad slots get
    src=0 (any valid row; killed by the one-hot) and dlo=255 (matches no
    iota value -> all-zero one-hot row/column)."""
    SPD, SRUN = cfg.spd, cfg.spd * P
    loops = np.arange(cfg.npg, dtype=np.int64)
    src_slots = np.zeros((cfg.ng, cfg.tj, SRUN), np.int64)
    dlo_slots = np.full((cfg.ng, cfg.tj, SRUN), 255, np.int64)
    deg = np.zeros((cfg.ng, cfg.npg), np.int64)
    for g in range(cfg.ng):
        e = slice(g * cfg.eg, (g + 1) * cfg.eg)
        s = np.concatenate([src_core[e] - g * cfg.npg, loops])
        d = np.concatenate([dst_core[e] - g * cfg.npg, loops])
        deg[g] = np.bincount(d, minlength=cfg.npg)
        db = d >> 7
        for b in range(cfg.tj):
            m = db == b
            cnt = int(m.sum())
            assert cnt <= SRUN, f"db run overflow: {cnt} > {SRUN}"
            src_slots[g, b, :cnt] = s[m]
            dlo_slots[g, b, :cnt] = d[m] & 127
    stream_src = src_slots.reshape(-1)
    stream_dlo = dlo_slots.reshape(-1)
    deg_t = np.ascontiguousarray(
        deg.reshape(cfg.ng, cfg.tj, P).transpose(2, 0, 1).reshape(P, cfg.nt)
    ).astype(np.float32)
    bf16 = ml_dtypes.bfloat16
    dinv = (1.0 / np.sqrt(np.maximum(deg.reshape(-1), 1.0))).astype(np.float32)
    return {
        "srcw": _wrap_idx(stream_src),
        "dinvF": dinv,
        "dlo_pm": np.ascontiguousarray(
            stream_dlo.reshape(-1, P).T.astype(bf16)
        ),
        "dlo_rep": np.ascontiguousarray(
            np.repeat(
                dlo_slots.reshape(cfg.ng * cfg.tj, 1, SRUN), P, axis=1
            ).astype(np.uint8)
        ),
        "degT": deg_t,
    }


def build_bass(cfg: Cfg):
    from contextlib import ExitStack
    nc = bacc.Bacc("TRN2", target_bir_lowering=False, debug=False,
                   num_swdge_queues=4)
    with tile.TileContext(nc) as tc:
        with ExitStack() as ctx:
            build_core_program(ctx, tc, cfg)
    nc.compile()
    return nc


_CFG = Cfg()
_NC_CACHE = {}
TRACE = False
LAST_RESULT = None


def kernel(x, edge_index, batch, W1, b1, bn_gamma, bn_beta, bn_mean, bn_var,
           W_lin1, b_lin1, p1, Wl, Wr, att, b_gat, p2,
           W_lin2, b_lin2, W_lin3, b_lin3):
    cfg = _CFG
    n_cores = 8
    s_att = float(np.sum(np.asarray(att, dtype=np.float64)))
    assert abs(s_att) > 1e-6, "degenerate att sum; poison scheme needs |sum(att)|>0"
    cfg.psign = -1.0 if s_att > 0 else 1.0
    slope = 0.2 if s_att > 0 else 1.0
    cfg.pb_mag = 40.0 / (slope * abs(s_att))
    weights = _prep_weights(cfg, W1, b1, bn_gamma, bn_beta, bn_mean, bn_var,
                            W_lin1, b_lin1, p1, Wl, Wr, att, b_gat, p2,
                            W_lin2, b_lin2, W_lin3, b_lin3)
    src_all = np.asarray(edge_index[0], dtype=np.int64)
    dst_all = np.asarray(edge_index[1], dtype=np.int64)
    x = np.asarray(x, dtype=np.float32)

    # choose the chunks-per-db-run capacity from the data (global max so the
    # single SPMD program fits every core)
    max_run = 0
    for c in range(n_cores):
        for g in range(cfg.ng):
            e0 = c * cfg.ne + g * cfg.eg
            d = dst_all[e0 : e0 + cfg.eg] - (c * cfg.nn + g * cfg.npg)
            cnts = np.bincount(d >> 7, minlength=cfg.tj) + P  # + self loops
            max_run = max(max_run, int(cnts.max()))
    cfg.spd = (max_run + P - 1) // P
    # windows of ch slots must tile a graph's slot range exactly
    while (cfg.tj * cfg.spd * P) % cfg.ch != 0:
        cfg.spd += 1

    in_maps = []
    for c in range(n_cores):
        n0 = c * cfg.nn
        e0 = c * cfg.ne
        d = dict(weights)
        d.update(
            _prep_core_edges(
                cfg, src_all[e0 : e0 + cfg.ne] - n0, dst_all[e0 : e0 + cfg.ne] - n0
            )
        )
        d["xT"] = np.ascontiguousarray(x[n0 : n0 + cfg.nn].T, np.float32)
        in_maps.append(d)

    key = ("nc", cfg.spd, cfg.psign, cfg.pb_mag)
    if key not in _NC_CACHE:
        _NC_CACHE[key] = build_bass(cfg)
    nc = _NC_CACHE[key]
    global LAST_RESULT
    res = run_bass_kernel_spmd(nc, in_maps, core_ids=list(range(n_cores)), trace=TRACE)
    LAST_RESULT = res
    outs = [np.asarray(res.results[c]["out"]).reshape(cfg.ng, 1) for c in range(n_cores)]
    return np.concatenate(outs, axis=0).astype(np.float32)


# revision 49
# speedup vs baseline: 1.6896x; 1.0644x over previous
"""Trainium2 Bass kernel for nn_AGNN_EFG (GCN -> TopK pool -> GATv2 -> TopK pool -> head).

Self-contained: shards the B=64 graphs across 8 NeuronCores (8 graphs/core),
runs one SPMD Bass program, gathers the [64, 1] head output on host.

v4 design:
- Edges (incl self loops) sorted per graph by dst block (db = dst>>7), each
  db run padded to a fixed spd chunks of 128 -> static chunk->db schedule
  (SPMD-safe; spd chosen on host from the data).
- Scatter-adds are ONE-HOT MATMULS on PE: per chunk a [128e,128d] bf16
  one-hot (DVE is_equal; pad slots carry sentinel 255 -> zero rows)
  accumulates messages into per-graph PSUM. No dma_scatter_add.
- GAT's xr[dst] is a [128d,128e]-orientation one-hot matmul (one-hot built
  on the Scalar engine as relu(1-(dlo-d)^2)) reading xr blocks from SBUF.
- Only 2 indirect passes remain (GpSimd Q7 descriptor generation is the
  machine bottleneck): gather u[src] (GCN) and [xl|xr][src] (GAT).
- Fully per-graph pipelined: utab(g+2) build, dense/pool/gtab(g),
  gcn(g+1) and gat(g) interleave so the GpSimd gather stream never idles.
- GCN u rows split [u_hi|u_lo] bf16 (~f32 accuracy, one 128-wide matmul).
- score1 fused into the lin1 matmul via host-folded [W_lin1 | W_lin1@p1n].
- Degrees are host-precomputed index data (bincount of dst); rsqrt on device.
"""

import sys

sys.path.insert(0, "/opt/trn_rl_repo")

from dataclasses import dataclass

import numpy as np
import ml_dtypes

import concourse.bass as bass
import concourse.mybir as mybir
import concourse.tile as tile
from concourse import bacc
from concourse.bass_utils import run_bass_kernel_spmd
from concourse.masks import make_identity

P = 128
F32 = mybir.dt.float32
BF16 = mybir.dt.bfloat16
I16 = mybir.dt.int16
U8 = mybir.dt.uint8
AF = mybir.ActivationFunctionType
OP = mybir.AluOpType
AX = mybir.AxisListType


@dataclass
class Cfg:
    ng: int = 8          # graphs per core
    npg: int = 2048      # nodes per graph
    hid: int = 64        # feature dim
    eg: int = 32768      # edges per graph (original, without self loops)
    spd: int = 19        # chunks (of 128 slots) per dst-block run; set at runtime
    ch: int = 1024       # gather window (slots per dma_gather call; >=2048
                         # overflows the SWDGE descriptor ring and faults)
    n_bisect: int = 30   # bisection iterations for topk threshold
    psign: float = -1.0  # -sign(sum(att)): poison sign so poisoned e < 0
    pb_mag: float = 200.0  # poison magnitude; set so poisoned e ~ -40 (LUT-safe)

    @property
    def nn(self):
        return self.ng * self.npg

    @property
    def ne(self):
        return self.ng * self.eg

    @property
    def tj(self):
        return self.npg // P  # dst blocks per graph (16)

    @property
    def nt(self):
        return self.ng * self.tj  # 128

    @property
    def tch(self):
        return self.tj * self.spd  # chunks per graph

    @property
    def slots_g(self):
        return self.tch * P  # padded edge slots per graph

    @property
    def slots(self):
        return self.ng * self.slots_g

    @property
    def wpg(self):
        assert self.slots_g % self.ch == 0
        return self.slots_g // self.ch  # gather windows per graph

    @property
    def cpw(self):
        return self.ch // P  # chunks per window

    @property
    def k1(self):
        return self.npg // 2

    @property
    def k2(self):
        return self.npg // 4


def build_core_program(ctx, tc, cfg: Cfg):
    from contextlib import ExitStack as _ES0
    nc = tc.nc
    NG, NPG, HID, NN = cfg.ng, cfg.npg, cfg.hid, cfg.nn
    NT, TJ, SPD = cfg.nt, cfg.tj, cfg.spd
    TCH, WPG, CPW, CH = cfg.tch, cfg.wpg, cfg.cpw, cfg.ch
    SRUN = SPD * P  # slots per db run
    assert HID == 64 and CPW % 8 == 0

    # ---- I/O ----
    xT = nc.dram_tensor("xT", [HID, NN], F32, kind="ExternalInput").ap()
    srcw = nc.dram_tensor("srcw", [P, cfg.slots // 16], I16, kind="ExternalInput").ap()
    dlo_pm = nc.dram_tensor("dlo_pm", [P, NG * TCH], BF16, kind="ExternalInput").ap()
    dlo_rep = nc.dram_tensor("dlo_rep", [NG * TJ * P, SRUN], U8, kind="ExternalInput").ap()
    degT = nc.dram_tensor("degT", [P, NT], F32, kind="ExternalInput").ap()
    w_names = ["Wl", "Wr"]
    Wd = {n: nc.dram_tensor(n, [HID, HID], F32, kind="ExternalInput").ap() for n in w_names}
    W1d = nc.dram_tensor("W1dup", [HID, P], F32, kind="ExternalInput").ap()
    W1p = nc.dram_tensor("Wlin1p", [HID, HID + 1], F32, kind="ExternalInput").ap()
    dinvF = nc.dram_tensor("dinvF", [NN], F32, kind="ExternalInput").ap()
    b1c = nc.dram_tensor("b_lin1c", [HID + 1], F32, kind="ExternalInput").ap()
    v_names = ["bn_a", "bn_bp", "att", "b_gat", "p2", "W23"]
    Vd = {n: nc.dram_tensor(n, [HID], F32, kind="ExternalInput").ap() for n in v_names}
    Cd = nc.dram_tensor("Cc", [1], F32, kind="ExternalInput").ap()
    out_d = nc.dram_tensor("out", [NG, 1], F32, kind="ExternalOutput").ap()

    # ---- DRAM scratch (per-graph tables so gathers only depend on their
    # own graph's writes) ----
    utab = [nc.dram_tensor(f"utab{g}", [NPG, P], BF16).ap() for g in range(NG)]
    gtab = [nc.dram_tensor(f"gtab{g}", [NPG, P], BF16).ap() for g in range(NG)]
    xlr_dram = nc.dram_tensor("xlr_dram", [P, NN], BF16).ap()
    ufm_dram = nc.dram_tensor("ufm_dram", [P, NN], BF16).ap()
    sc_dram = nc.dram_tensor("sc_dram", [NN], F32).ap()

    cpool = ctx.enter_context(tc.tile_pool(name="consts", bufs=1))
    mpool = ctx.enter_context(tc.tile_pool(name="main", bufs=1))
    smallps = ctx.enter_context(tc.tile_pool(name="smallps", bufs=1, space="PSUM"))

    # ---- constants ----
    ident = cpool.tile([P, P], F32)
    make_identity(nc, ident[:])
    ones128 = cpool.tile([P, P], F32)
    nc.vector.memset(ones128[:], 1.0)
    nantile = cpool.tile([P, NT], F32)
    nc.vector.memset(nantile[:], float("nan"))
    negbig = cpool.tile([P, NT], F32)
    nc.vector.memset(negbig[:], -1e9)
    io16 = cpool.tile([P, P], I16)
    nc.gpsimd.iota(io16[:], pattern=[[1, P]], base=0, channel_multiplier=0)
    iota_bf = cpool.tile([P, P], BF16)
    nc.vector.tensor_copy(out=iota_bf[:], in_=io16[:])
    ioc16 = cpool.tile([P, 1], I16)
    nc.gpsimd.iota(ioc16[:], pattern=[[0, 1]], base=0, channel_multiplier=1)
    niota_col = cpool.tile([P, 1], F32)
    nc.vector.tensor_scalar(
        out=niota_col[:], in0=ioc16[:], scalar1=-1.0, scalar2=None, op0=OP.mult
    )

    Ws = {}
    for n in w_names:
        t = cpool.tile([HID, HID], F32, tag=f"w_{n}")
        nc.sync.dma_start(out=t[:], in_=Wd[n][:])
        Ws[n] = t
    W1p_sb = cpool.tile([HID, HID + 1], F32, tag="w_Wlin1p")
    nc.sync.dma_start(out=W1p_sb[:], in_=W1p[:])
    W1d_sb = cpool.tile([HID, P], F32, tag="w_W1dup")
    nc.sync.dma_start(out=W1d_sb[:], in_=W1d[:])
    b1c_sb = cpool.tile([HID + 1, 1], F32, tag="v_b1c")
    nc.sync.dma_start(out=b1c_sb[:], in_=b1c[:, None])
    Vs = {}
    for n in v_names:
        t = cpool.tile([HID, 1], F32, tag=f"v_{n}")
        nc.sync.dma_start(out=t[:], in_=Vd[n][:, None])
        Vs[n] = t
    att_rep = cpool.tile([P, HID], BF16)
    nc.gpsimd.dma_start(out=att_rep[:], in_=Vd["att"][None, :].to_broadcast([P, HID]))
    p2_rep = cpool.tile([P, HID], F32)
    nc.sync.dma_start(out=p2_rep[:], in_=Vd["p2"][None, :].to_broadcast([P, HID]))
    bgat_rep = cpool.tile([P, HID], F32)
    nc.sync.dma_start(out=bgat_rep[:], in_=Vd["b_gat"][None, :].to_broadcast([P, HID]))
    Cc_sb = cpool.tile([NG, 1], F32)
    nc.sync.dma_start(out=Cc_sb[:], in_=Cd[None, :].to_broadcast([NG, 1]))

    # whole per-chunk dst-low-bit table (for one-hot builds in [e,d] orientation)
    dlo_sb = cpool.tile([P, NG * TCH], BF16)
    nc.sync.dma_start(out=dlo_sb[:], in_=dlo_pm[:])

    # ---- dinv from host degree counts ----
    dinv_t = mpool.tile([P, NT], F32, tag="dinv_t")
    sqd_t = mpool.tile([P, NT], F32, tag="sqd_t")
    ntmp = mpool.tile([P, NT], F32, tag="ntmp")

    def recip_newton(r_ap, x_ap, tmp_ap):
        nc.vector.tensor_tensor(out=tmp_ap, in0=x_ap, in1=r_ap, op=OP.mult)
        nc.vector.tensor_scalar(
            out=tmp_ap, in0=tmp_ap, scalar1=-1.0, scalar2=2.0, op0=OP.mult, op1=OP.add
        )
        nc.vector.tensor_tensor(out=r_ap, in0=r_ap, in1=tmp_ap, op=OP.mult)

    deg_sb = mpool.tile([P, NT], F32, tag="deg_sb")
    nc.sync.dma_start(out=deg_sb[:], in_=degT[:])
    nc.scalar.sqrt(out=sqd_t[:], in_=deg_sb[:])
    nc.vector.reciprocal(out=dinv_t[:], in_=sqd_t[:])
    recip_newton(dinv_t[:], sqd_t[:], ntmp[:])

    def idx_slice(pool, w_global, tag):
        t = pool.tile([P, CH // 16], I16, tag=tag)
        c0 = w_global * (CH // 16)
        nc.sync.dma_start(out=t[:], in_=srcw[:, c0 : c0 + CH // 16])
        return t

    # ---- pools (all phases interleave; PSUM budget: 2+1+2+2+1 = 8 banks) ----
    gep = ctx.enter_context(tc.tile_pool(name="gep", bufs=6))
    ohp = ctx.enter_context(tc.tile_pool(name="ohp", bufs=3))
    phase_gc = _ES0()
    hps_pool = phase_gc.enter_context(tc.tile_pool(name="hpsp", bufs=1, space="PSUM"))
    dsa = ctx.enter_context(tc.tile_pool(name="dsa", bufs=1))
    dss = ctx.enter_context(tc.tile_pool(name="dss", bufs=2))
    bis = ctx.enter_context(tc.tile_pool(name="bis", bufs=2))
    gap = ctx.enter_context(tc.tile_pool(name="gap", bufs=6))
    aohp = ctx.enter_context(tc.tile_pool(name="aohp", bufs=3))
    runtmp = ctx.enter_context(tc.tile_pool(name="runtmp", bufs=1))
    runp = ctx.enter_context(tc.tile_pool(name="runp", bufs=2))
    gatps = None
    mxrp = None
    gfin = ctx.enter_context(tc.tile_pool(name="gfin", bufs=1))
    phase_u = _ES0()
    ups = phase_u.enter_context(tc.tile_pool(name="ups", bufs=2))
    ubc = phase_u.enter_context(tc.tile_pool(name="ubc", bufs=1))

    # persistent t-space tiles
    h2t = mpool.tile([P, NT, HID], F32, tag="bigA")
    gstage = mpool.tile([P, NT, P], BF16, tag="bigC")
    score1_t = mpool.tile([P, NT], F32, tag="score1_t")
    tanh1 = mpool.tile([P, NT], F32, tag="tanh1")
    gate1 = mpool.tile([P, NT], F32, tag="gate1")
    gate1z = mpool.tile([P, NT], F32, tag="gate1z")
    padd = mpool.tile([P, NT], F32, tag="padd")
    kept1 = mpool.tile([P, NT], U8, tag="kept1")
    pb = cfg.psign * cfg.pb_mag

    hps_tiles = {}
    nmps_tiles = {}

    # ======== per-graph sections ========

    def utab_build(g):
        # xw (duplicated rows) -> dinv scale -> [hi|lo] bf16 split, all in
        # feature-major; node-major utab rows via one transpose DMA hop.
        xw2 = ups.tile([P, NPG], F32, tag="xw2", name=f"xw2_{g}")
        for jl in range(TJ):
            xTg = ups.tile([HID, P], F32, tag="xTg", name=f"xTg{g}_{jl}")
            nc.scalar.dma_start(
                out=xTg[:], in_=xT[:, g * NPG + jl * P : g * NPG + (jl + 1) * P]
            )
            pm = smallps.tile([P, P], F32, tag="smA", name=f"xwps{g}_{jl}")
            nc.tensor.matmul(pm[:], lhsT=W1d_sb[:], rhs=xTg[:], start=True, stop=True)
            nc.scalar.copy(out=xw2[:, jl * P : (jl + 1) * P], in_=pm[:])
        dbc = ubc.tile([P, NPG], F32, tag="dinvbc", name=f"dinvbc{g}")
        nc.scalar.dma_start(
            out=dbc[:], in_=dinvF[g * NPG : (g + 1) * NPG][None, :].to_broadcast([P, NPG])
        )
        nc.vector.tensor_tensor(out=xw2[:], in0=xw2[:], in1=dbc[:], op=OP.mult)
        u2 = ups.tile([P, NPG], BF16, tag="u2", name=f"u2_{g}")
        nc.vector.tensor_copy(out=u2[0:HID, :], in_=xw2[0:HID, :])
        nc.vector.tensor_copy(out=u2[HID:P, :], in_=xw2[HID:P, :])
        nc.vector.tensor_tensor(
            out=u2[HID:P, :], in0=xw2[HID:P, :], in1=u2[HID:P, :], op=OP.subtract
        )
        nc.scalar.dma_start(out=ufm_dram[:, g * NPG : (g + 1) * NPG], in_=u2[:])
        u_nm = ups.tile([P, TJ, P], BF16, tag="u2", name=f"unm{g}")
        nc.sync.dma_start_transpose(
            out=u_nm[:], in_=ufm_dram[:, g * NPG : (g + 1) * NPG]
        )
        nc.scalar.dma_start(
            out=utab[g].rearrange("(j p) f -> p j f", p=P), in_=u_nm[:]
        )

    def gcn_pass(g):
        par = g % 2
        hps = [
            hps_pool.tile([P, 8, HID], F32, tag=f"hps{par}{t}", name=f"hps{t}_{g}")
            for t in range(2)
        ]
        hps_tiles[g] = hps
        for w in range(WPG):
            wg = g * WPG + w
            ssl = idx_slice(gep, wg, "ssl")
            ub = gep.tile([P, CPW, P], BF16, tag="ub")
            nc.gpsimd.dma_gather(
                out_ap=ub[:], in_ap=utab[g][:], idxs_ap=ssl[:],
                num_idxs=CH, num_idxs_reg=CH, elem_size=P, queue_num=wg % 4,
            )
            for b in range(CPW // 8):
                oh8 = ohp.tile([P, 8, P], BF16, tag="oh8")
                c0 = g * TCH + w * CPW + b * 8
                nc.vector.tensor_tensor(
                    out=oh8[:],
                    in0=iota_bf[:, None, :].to_broadcast([P, 8, P]),
                    in1=dlo_sb[:, c0 : c0 + 8, None].to_broadcast([P, 8, P]),
                    op=OP.is_equal,
                )
                for cl in range(8):
                    c = b * 8 + cl
                    gc = w * CPW + c
                    db, pos = gc // SPD, gc % SPD
                    out_slc = hps[db // 8][:, db % 8, :]
                    nc.tensor.matmul(
                        out_slc, lhsT=oh8[:, cl, :], rhs=ub[:, c, 0:HID],
                        start=(pos == 0), stop=False,
                    )
                    nc.tensor.matmul(
                        out_slc, lhsT=oh8[:, cl, :], rhs=ub[:, c, HID:P],
                        start=False, stop=(pos == SPD - 1),
                    )

    S6C = 512

    def dense_g(g):
        # h = dinv*(hi+lo); BN+leaky (one ACT Lrelu); lin1(+score); xl/xr
        hps = hps_tiles.pop(g)
        gsl = slice(g * TJ, (g + 1) * TJ)
        hsum = dsa.tile([P, TJ, HID], F32, tag="hsum", name=f"hsum{g}")
        for t in range(2):
            nc.vector.tensor_tensor(
                out=hsum[:, t * 8 : (t + 1) * 8, :], in0=hps[t][:],
                in1=dinv_t[:, g * TJ + t * 8 : g * TJ + (t + 1) * 8, None].to_broadcast(
                    [P, 8, HID]
                ),
                op=OP.mult,
            )
        hfm = dsa.tile([HID, NPG], F32, tag="hfm", name=f"hfm{g}")
        for jl in range(TJ):
            pt = smallps.tile([HID, P], F32, tag=("smA" if jl % 2 == 0 else "smB"), name=f"htr{g}_{jl}")
            nc.tensor.transpose(out=pt[:], in_=hsum[:, jl, :], identity=ident[:])
            nc.scalar.copy(out=hfm[:, jl * P : (jl + 1) * P], in_=pt[:])
        nc.scalar.activation(
            out=hfm[:], in_=hfm[:], func=AF.Lrelu, scale=Vs["bn_a"][:],
            bias=Vs["bn_bp"][:], alpha=0.01,
        )
        xlr_g = dss.tile([P, NPG], BF16, tag="xlrg", name=f"xlrg{g}")
        for ol in range(0, NPG, S6C):
            o = g * NPG + ol
            pm = hps_pool.tile([HID + 1, S6C], F32, tag=f"hps{g % 2}0", name=f"l1ps{o}")
            nc.tensor.matmul(
                pm[:], lhsT=W1p_sb[:], rhs=hfm[:, ol : ol + S6C],
                start=True, stop=True,
            )
            hc = dss.tile([HID + 1, S6C], F32, tag="hc", name=f"hc{o}")
            nc.scalar.activation(
                out=hc[:], in_=pm[:], func=AF.Identity, bias=b1c_sb[:]
            )
            nc.sync.dma_start(out=sc_dram[None, o : o + S6C], in_=hc[HID : HID + 1, :])
            px = hps_pool.tile([P, S6C], F32, tag=f"hps{g % 2}1", name=f"xlrps{o}")
            nc.tensor.matmul(px[:HID, :], lhsT=Ws["Wl"][:], rhs=hc[0:HID, :], start=True, stop=True)
            nc.tensor.matmul(px[HID:, :], lhsT=Ws["Wr"][:], rhs=hc[0:HID, :], start=True, stop=True)
            nc.scalar.copy(out=xlr_g[:, ol : ol + S6C], in_=px[:])
        nc.scalar.dma_start(out=xlr_dram[:, g * NPG : (g + 1) * NPG], in_=xlr_g[:])

    def bisect_multi(score_slc, ngr, target, tag):
        # score_slc: [P, ngr*TJ]; returns per-graph thresholds lo [P, ngr]
        lo = bis.tile([P, ngr], F32, tag="lo", name=f"lo_{tag}")
        hi = bis.tile([P, ngr], F32, tag="hi", name=f"hi_{tag}")
        mid = bis.tile([P, ngr], F32, tag="mid", name=f"mid_{tag}")
        cmp = bis.tile([P, ngr * TJ], F32, tag="cmp", name=f"cmp_{tag}")
        cred = bis.tile([P, ngr], F32, tag="cred", name=f"cred_{tag}")
        ge = bis.tile([P, ngr], U8, tag="ge", name=f"ge_{tag}")
        lt = bis.tile([P, ngr], U8, tag="lt", name=f"lt_{tag}")
        nc.vector.memset(lo[:], -64.0)
        nc.vector.memset(hi[:], 64.0)
        sc_g = score_slc.rearrange("p (g t) -> p g t", g=ngr)
        cmp_g = cmp[:].rearrange("p (g t) -> p g t", g=ngr)
        for it in range(cfg.n_bisect):
            nc.vector.tensor_tensor(out=mid[:], in0=lo[:], in1=hi[:], op=OP.add)
            nc.vector.tensor_scalar(
                out=mid[:], in0=mid[:], scalar1=0.5, scalar2=None, op0=OP.mult
            )
            nc.vector.tensor_tensor(
                out=cmp_g, in0=sc_g,
                in1=mid[:, :, None].to_broadcast([P, ngr, TJ]), op=OP.is_gt,
            )
            nc.vector.tensor_reduce(out=cred[:], in_=cmp_g, axis=AX.X, op=OP.add)
            cps = smallps.tile([P, ngr], F32, tag="smB", name=f"cnt_{tag}_{it}")
            nc.tensor.matmul(cps[:], lhsT=ones128[:], rhs=cred[:], start=True, stop=True)
            nc.vector.tensor_scalar(
                out=ge[:], in0=cps[:], scalar1=float(target), scalar2=None, op0=OP.is_ge
            )
            nc.vector.tensor_scalar(
                out=lt[:], in0=cps[:], scalar1=float(target), scalar2=None, op0=OP.is_lt
            )
            nc.vector.copy_predicated(out=lo[:], mask=ge[:], data=mid[:])
            nc.vector.copy_predicated(out=hi[:], mask=lt[:], data=mid[:])
        return lo

    NH = NG // 2  # graphs per pooling half

    def pool1_h(h):
        g0 = h * NH
        gsl = slice(g0 * TJ, (g0 + NH) * TJ)
        nc.sync.dma_start(
            out=score1_t[:, gsl],
            in_=sc_dram[g0 * NPG : (g0 + NH) * NPG].rearrange("(j p) -> p j", p=P),
        )
        t1 = bisect_multi(score1_t[:, gsl], NH, cfg.k1, f"p1h{h}")
        nc.vector.tensor_tensor(
            out=kept1[:, gsl].rearrange("p (g t) -> p g t", g=NH),
            in0=score1_t[:, gsl].rearrange("p (g t) -> p g t", g=NH),
            in1=t1[:, :, None].to_broadcast([P, NH, TJ]), op=OP.is_gt,
        )
        nc.scalar.activation(out=tanh1[:, gsl], in_=score1_t[:, gsl], func=AF.Tanh)
        nc.vector.tensor_copy(out=gate1[:, gsl], in_=nantile[:, gsl])
        nc.vector.copy_predicated(out=gate1[:, gsl], mask=kept1[:, gsl], data=tanh1[:, gsl])
        nc.vector.memset(gate1z[:, gsl], 0.0)
        nc.vector.copy_predicated(out=gate1z[:, gsl], mask=kept1[:, gsl], data=tanh1[:, gsl])
        nc.vector.tensor_scalar(
            out=padd[:, gsl], in0=kept1[:, gsl], scalar1=-pb, scalar2=pb,
            op0=OP.mult, op1=OP.add,
        )
        # gtab for the half
        gs = gstage[:, gsl, :]
        nc.sync.dma_start_transpose(
            out=gs, in_=xlr_dram[:, g0 * NPG : (g0 + NH) * NPG]
        )
        nc.vector.tensor_tensor(
            out=gs, in0=gs,
            in1=gate1z[:, gsl, None].to_broadcast([P, NH * TJ, P]), op=OP.mult,
        )
        nc.vector.tensor_tensor(
            out=gs, in0=gs,
            in1=padd[:, gsl, None].to_broadcast([P, NH * TJ, P]), op=OP.add,
        )
        for gg in range(g0, g0 + NH):
            nc.sync.dma_start(
                out=gtab[gg].rearrange("(j p) f -> p j f", p=P),
                in_=gstage[:, gg * TJ : (gg + 1) * TJ, :],
            )

    gat_pools = {}

    def gat_pass(g):
        if "gatps" not in gat_pools:
            gat_pools["gatps"] = ctx.enter_context(tc.tile_pool(name="gatps", bufs=1, space="PSUM"))
            gat_pools["mxrp"] = ctx.enter_context(tc.tile_pool(name="mxrp", bufs=2, space="PSUM"))
        gatps, mxrp = gat_pools["gatps"], gat_pools["mxrp"]
        nmps = [
            gatps.tile([P, 8, HID], F32, tag=f"nmps{t}", name=f"nmps{t}_{g}")
            for t in range(2)
        ]
        dnps = smallps.tile([P, TJ], F32, tag="smB", name=f"dnps{g}")
        oh_de = {}
        for w in range(WPG):
            wg = g * WPG + w
            ssl = idx_slice(gap, wg, "assl")
            gx = gap.tile([P, CPW, P], BF16, tag="gx")
            nc.gpsimd.dma_gather(
                out_ap=gx[:], in_ap=gtab[g][:], idxs_ap=ssl[:],
                num_idxs=CH, num_idxs_reg=CH, elem_size=P, queue_num=wg % 4,
            )
            for b in range(CPW // 8):
                oh8 = aohp.tile([P, 8, P], BF16, tag="aoh8")
                c0 = g * TCH + w * CPW + b * 8
                nc.vector.tensor_tensor(
                    out=oh8[:],
                    in0=iota_bf[:, None, :].to_broadcast([P, 8, P]),
                    in1=dlo_sb[:, c0 : c0 + 8, None].to_broadcast([P, 8, P]),
                    op=OP.is_equal,
                )
                mxr = mxrp.tile([P, 8, HID], F32, tag="mxr", name=f"mxr{wg}_{b}")
                for cl in range(8):
                    c = b * 8 + cl
                    gc = w * CPW + c
                    r, pos = gc // SPD, gc % SPD
                    if pos == 0:
                        dlo_bc = runtmp.tile(
                            [P, SRUN], U8, tag="dlobc", name=f"dlobc{g}_{r}"
                        )
                        o = (g * TJ + r) * P
                        nc.sync.dma_start(
                            out=dlo_bc[:], in_=dlo_rep[o : o + P, :]
                        )
                        # one-hot on the Scalar engine: relu(1 - (dlo - d)^2)
                        ohsq = runtmp.tile([P, SRUN], BF16, tag="ohsq", name=f"ohsq{g}_{r}")
                        nc.scalar.activation(
                            out=ohsq[:], in_=dlo_bc[:], func=AF.Square,
                            bias=niota_col[:],
                        )
                        ohr = runp.tile([P, SRUN], BF16, tag="ohde", name=f"ohde{g}_{r}")
                        nc.scalar.activation(
                            out=ohr[:], in_=ohsq[:], func=AF.Relu,
                            bias=1.0, scale=-1.0,
                        )
                        oh_de[r] = ohr
                    nc.tensor.matmul(
                        mxr[:, cl, :],
                        lhsT=oh_de[r][:, pos * P : (pos + 1) * P],
                        rhs=gstage[:, g * TJ + r, HID:P],
                        start=True, stop=True,
                    )
                # e = att . leaky(xl_s + xr_d); w = exp(e); pay = w*xl
                gxs = gx[:, b * 8 : (b + 1) * 8, :]
                z = gap.tile([P, 8, HID], BF16, tag="z")
                nc.vector.tensor_tensor(
                    out=z[:], in0=gxs[:, :, 0:HID], in1=mxr[:], op=OP.add
                )
                nc.vector.scalar_tensor_tensor(
                    out=z[:], in0=z[:], scalar=0.2, in1=z[:], op0=OP.mult, op1=OP.max,
                )
                nc.vector.tensor_tensor(
                    out=z[:], in0=z[:],
                    in1=att_rep[:, None, :].to_broadcast([P, 8, HID]), op=OP.mult,
                )
                e8 = gap.tile([P, 8], F32, tag="e8")
                nc.vector.tensor_reduce(out=e8[:], in_=z[:], axis=AX.X, op=OP.add)
                w8b = gap.tile([P, 8], BF16, tag="w8b")
                nc.scalar.activation(out=w8b[:], in_=e8[:], func=AF.Exp)
                pay = gap.tile([P, 8, HID], BF16, tag="pay")
                nc.vector.tensor_tensor(
                    out=pay[:], in0=gxs[:, :, 0:HID],
                    in1=w8b[:, :, None].to_broadcast([P, 8, HID]), op=OP.mult,
                )
                for cl in range(8):
                    c = b * 8 + cl
                    gc = w * CPW + c
                    db, pos = gc // SPD, gc % SPD
                    nc.tensor.matmul(
                        nmps[db // 8][:, db % 8, :],
                        lhsT=oh8[:, cl, :], rhs=pay[:, cl, :],
                        start=(pos == 0), stop=(pos == SPD - 1),
                    )
                    nc.tensor.matmul(
                        dnps[:, db : db + 1],
                        lhsT=oh8[:, cl, :], rhs=w8b[:, cl : cl + 1],
                        start=(pos == 0), stop=(pos == SPD - 1),
                    )

        # ---- finalize graph g: h2 = leaky(numer/denom + b_gat) ----
        numsb = dsa.tile([P, TJ, HID], F32, tag="hsum", name=f"numsb{g}")
        nc.scalar.copy(out=numsb[:, 0:8, :], in_=nmps[0][:])
        nc.scalar.copy(out=numsb[:, 8:TJ, :], in_=nmps[1][:])
        den = gfin.tile([P, TJ], F32, tag="den", name=f"den{g}")
        rec = gfin.tile([P, TJ], F32, tag="rec", name=f"rec{g}")
        dtmp = gfin.tile([P, TJ], F32, tag="dtmp", name=f"dtmp{g}")
        nc.vector.tensor_scalar(
            out=den[:], in0=dnps[:], scalar1=1e-16, scalar2=None, op0=OP.add
        )
        nc.vector.reciprocal(out=rec[:], in_=den[:])
        recip_newton(rec[:], den[:], dtmp[:])
        hslc = h2t[:, g * TJ : (g + 1) * TJ, :]
        nc.vector.tensor_tensor(
            out=hslc, in0=numsb[:],
            in1=rec[:, :, None].to_broadcast([P, TJ, HID]), op=OP.mult,
        )
        nc.vector.tensor_tensor(
            out=hslc, in0=hslc,
            in1=bgat_rep[:, None, :].to_broadcast([P, TJ, HID]), op=OP.add,
        )
        nc.vector.scalar_tensor_tensor(
            out=hslc, in0=hslc, scalar=0.01, in1=hslc, op0=OP.mult, op1=OP.max
        )

    # ======== emission: utab phase, gcn stream, pools, gat stream ========
    for g in range(NG):
        utab_build(g)
    phase_u.close()
    for g in range(NG):
        gcn_pass(g)
        dense_g(g)
    pool1_h(0)
    pool1_h(1)
    phase_gc.close()
    for g in range(NG):
        gat_pass(g)

    # ======== score2 (t-space, blocked), mask to kept1 ========
    score2_t = mpool.tile([P, NT], F32, tag="score2_t")
    for t in range(4):
        tsl = slice(t * 32, (t + 1) * 32)
        blk = dsa.tile([P, 32, HID], F32, tag="hfm", name=f"s2blk{t}")
        nc.vector.tensor_tensor(
            out=blk[:], in0=h2t[:, tsl, :],
            in1=p2_rep[:, None, :].to_broadcast([P, 32, HID]), op=OP.mult,
        )
        nc.vector.tensor_reduce(out=score2_t[:, tsl], in_=blk[:], axis=AX.X, op=OP.add)
    kept1_t = mpool.tile([P, NT], U8, tag="kept1_t")
    nc.vector.tensor_tensor(out=kept1_t[:], in0=gate1[:], in1=gate1[:], op=OP.is_equal)
    sc2m = mpool.tile([P, NT], F32, tag="sc2m")
    nc.vector.tensor_copy(out=sc2m[:], in_=negbig[:])
    nc.vector.copy_predicated(out=sc2m[:], mask=kept1_t[:], data=score2_t[:])

    # ======== pool2 threshold + gate2 = tanh * mask ========
    gate2 = mpool.tile([P, NT], F32, tag="gate2")
    t2 = bisect_multi(sc2m[:], NG, cfg.k2, "p2")
    nc.vector.tensor_tensor(
        out=gate2[:].rearrange("p (g t) -> p g t", g=NG),
        in0=sc2m[:].rearrange("p (g t) -> p g t", g=NG),
        in1=t2[:, :, None].to_broadcast([P, NG, TJ]), op=OP.is_gt,
    )
    tanh2 = mpool.tile([P, NT], F32, tag="tanh2")
    sc2c = mpool.tile([P, NT], F32, tag="sc2c")
    nc.vector.tensor_scalar(
        out=sc2c[:], in0=sc2m[:], scalar1=-64.0, scalar2=None, op0=OP.max
    )
    nc.scalar.activation(out=tanh2[:], in_=sc2c[:], func=AF.Tanh)
    nc.vector.tensor_tensor(out=gate2[:], in0=gate2[:], in1=tanh2[:], op=OP.mult)

    # ======== T_g = sum_n gate2[n] * h2[n]; out = T @ W23 + C ========
    Tps = smallps.tile([P, NG], F32, tag="smB")
    for j in range(NT):
        g = j // TJ
        nc.tensor.matmul(
            Tps[:HID, g : g + 1], lhsT=h2t[:, j, :], rhs=gate2[:, j : j + 1],
            start=(j % TJ == 0), stop=(j % TJ == TJ - 1),
        )
    Tsb = mpool.tile([HID, NG], F32, tag="Tsb")
    nc.scalar.copy(out=Tsb[:], in_=Tps[:HID, :])
    hps2 = smallps.tile([NG, 1], F32, tag="smB")
    nc.tensor.matmul(hps2[:], lhsT=Tsb[:], rhs=Vs["W23"][:], start=True, stop=True)
    outsb = mpool.tile([NG, 1], F32, tag="outsb")
    nc.vector.tensor_tensor(out=outsb[:], in0=hps2[:], in1=Cc_sb[:], op=OP.add)
    nc.sync.dma_start(out=out_d[:], in_=outsb[:])


# ================= host side =================

def _wrap_idx(ix: np.ndarray) -> np.ndarray:
    n = ix.shape[0]
    w = ix.reshape(n // 16, 16).T.astype(np.int16)
    return np.tile(w, (8, 1)).copy()


def _prep_weights(cfg, W1, b1, bn_gamma, bn_beta, bn_mean, bn_var, W_lin1, b_lin1,
                  p1, Wl, Wr, att, b_gat, p2, W_lin2, b_lin2, W_lin3, b_lin3):
    f32 = np.float32
    bn_a = (bn_gamma / np.sqrt(bn_var + 1e-5)).astype(f32)
    bn_b = (bn_beta - bn_mean * bn_a).astype(f32)
    W23 = (W_lin2 @ W_lin3).reshape(-1).astype(f32)
    Cc = np.array([cfg.k2 * float(b_lin2 @ W_lin3[:, 0]) + float(b_lin3[0])], dtype=f32)
    p1n = (np.asarray(p1) / np.linalg.norm(np.asarray(p1))).astype(np.float64)
    Wlin1p = np.concatenate(
        [np.asarray(W_lin1, np.float64),
         (np.asarray(W_lin1, np.float64) @ p1n)[:, None]], axis=1
    ).astype(f32)
    c1 = np.array([float(p1n @ np.asarray(b_lin1, np.float64))], dtype=f32)
    return {
        "W1dup": np.ascontiguousarray(
            np.concatenate([np.asarray(W1, f32)] * 2, axis=1)
        ),
        "Wlin1p": Wlin1p,
        "Wl": np.ascontiguousarray(Wl, f32), "Wr": np.ascontiguousarray(Wr, f32),
        "bn_a": bn_a, "bn_bp": (np.asarray(b1, f32) * bn_a + bn_b).astype(f32),
        "b_lin1c": np.concatenate([np.asarray(b_lin1, f32), c1]),
        "att": np.ascontiguousarray(att, f32), "b_gat": np.ascontiguousarray(b_gat, f32),
        "p2": (np.asarray(p2) / np.linalg.norm(np.asarray(p2))).astype(f32),
        "W23": W23, "Cc": Cc,
    }


def _prep_core_edges(cfg: Cfg, src_core, dst_core):
    """src/dst core-local [ne]. Per graph: append self loops, bucket edges by
    dst block (db = dst>>7), pad each db run to spd*128 slots. Pad slots get
    src=0 (any valid row; killed by the one-hot) and dlo=255 (matches no
    iota value -> all-zero one-hot row/column)."""
    SPD, SRUN = cfg.spd, cfg.spd * P
    loops = np.arange(cfg.npg, dtype=np.int64)
    src_slots = np.zeros((cfg.ng, cfg.tj, SRUN), np.int64)
    dlo_slots = np.full((cfg.ng, cfg.tj, SRUN), 255, np.int64)
    deg = np.zeros((cfg.ng, cfg.npg), np.int64)
    for g in range(cfg.ng):
        e = slice(g * cfg.eg, (g + 1) * cfg.eg)
        s = np.concatenate([src_core[e] - g * cfg.npg, loops])
        d = np.concatenate([dst_core[e] - g * cfg.npg, loops])
        deg[g] = np.bincount(d, minlength=cfg.npg)
        db = d >> 7
        for b in range(cfg.tj):
            m = db == b
            cnt = int(m.sum())
            assert cnt <= SRUN, f"db run overflow: {cnt} > {SRUN}"
            src_slots[g, b, :cnt] = s[m]
            dlo_slots[g, b, :cnt] = d[m] & 127
    stream_src = src_slots.reshape(-1)
    stream_dlo = dlo_slots.reshape(-1)
    deg_t = np.ascontiguousarray(
        deg.reshape(cfg.ng, cfg.tj, P).transpose(2, 0, 1).reshape(P, cfg.nt)
    ).astype(np.float32)
    bf16 = ml_dtypes.bfloat16
    dinv = (1.0 / np.sqrt(np.maximum(deg.reshape(-1), 1.0))).astype(np.float32)
    return {
        "srcw": _wrap_idx(stream_src),
        "dinvF": dinv,
        "dlo_pm": np.ascontiguousarray(
            stream_dlo.reshape(-1, P).T.astype(bf16)
        ),
        "dlo_rep": np.ascontiguousarray(
            np.repeat(
                dlo_slots.reshape(cfg.ng * cfg.tj, 1, SRUN), P, axis=1
            ).astype(np.uint8)
        ),
        "degT": deg_t,
    }


def build_bass(cfg: Cfg):
    from contextlib import ExitStack
    nc = bacc.Bacc("TRN2", target_bir_lowering=False, debug=False,
                   num_swdge_queues=4)
    with tile.TileContext(nc) as tc:
        with ExitStack() as ctx:
            build_core_program(ctx, tc, cfg)
    nc.compile()
    return nc


_CFG = Cfg()
_NC_CACHE = {}
TRACE = False
LAST_RESULT = None


def kernel(x, edge_index, batch, W1, b1, bn_gamma, bn_beta, bn_mean, bn_var,
           W_lin1, b_lin1, p1, Wl, Wr, att, b_gat, p2,
           W_lin2, b_lin2, W_lin3, b_lin3):
    cfg = _CFG
    n_cores = 8
    s_att = float(np.sum(np.asarray(att, dtype=np.float64)))
    assert abs(s_att) > 1e-6, "degenerate att sum; poison scheme needs |sum(att)|>0"
    cfg.psign = -1.0 if s_att > 0 else 1.0
    slope = 0.2 if s_att > 0 else 1.0
    cfg.pb_mag = 40.0 / (slope * abs(s_att))
    weights = _prep_weights(cfg, W1, b1, bn_gamma, bn_beta, bn_mean, bn_var,
                            W_lin1, b_lin1, p1, Wl, Wr, att, b_gat, p2,
                            W_lin2, b_lin2, W_lin3, b_lin3)
    src_all = np.asarray(edge_index[0], dtype=np.int64)
    dst_all = np.asarray(edge_index[1], dtype=np.int64)
    x = np.asarray(x, dtype=np.float32)

    # choose the chunks-per-db-run capacity from the data (global max so the
    # single SPMD program fits every core)
    max_run = 0
    for c in range(n_cores):
        for g in range(cfg.ng):
            e0 = c * cfg.ne + g * cfg.eg
            d = dst_all[e0 : e0 + cfg.eg] - (c * cfg.nn + g * cfg.npg)
            cnts = np.bincount(d >> 7, minlength=cfg.tj) + P  # + self loops
            max_run = max(max_run, int(cnts.max()))
    cfg.spd = (max_run + P - 1) // P
    # windows of ch slots must tile a graph's slot range exactly
    while (cfg.tj * cfg.spd * P) % cfg.ch != 0:
        cfg.spd += 1

    in_maps = []
    for c in range(n_cores):
        n0 = c * cfg.nn
        e0 = c * cfg.ne
        d = dict(weights)
        d.update(
            _prep_core_edges(
                cfg, src_all[e0 : e0 + cfg.ne] - n0, dst_all[e0 : e0 + cfg.ne] - n0
            )
        )
        d["xT"] = np.ascontiguousarray(x[n0 : n0 + cfg.nn].T, np.float32)
        in_maps.append(d)

    key = ("nc", cfg.spd, cfg.psign, cfg.pb_mag)
    if key not in _NC_CACHE:
        _NC_CACHE[key] = build_bass(cfg)
    nc = _NC_CACHE[key]
    global LAST_RESULT
    res = run_bass_kernel_spmd(nc, in_maps, core_ids=list(range(n_cores)), trace=TRACE)
    LAST_RESULT = res
    outs = [np.asarray(res.results[c]["out"]).reshape(cfg.ng, 1) for c in range(n_cores)]
    return np.concatenate(outs, axis=0).astype(np.float32)


# revision 51
# speedup vs baseline: 1.8006x; 1.0656x over previous
"""Trainium2 Bass kernel for nn_AGNN_EFG (GCN -> TopK pool -> GATv2 -> TopK pool -> head).

Self-contained: shards the B=64 graphs across 8 NeuronCores (8 graphs/core),
runs one SPMD Bass program, gathers the [64, 1] head output on host.

v4 design:
- Edges (incl self loops) sorted per graph by dst block (db = dst>>7), each
  db run padded to a fixed spd chunks of 128 -> static chunk->db schedule
  (SPMD-safe; spd chosen on host from the data).
- Scatter-adds are ONE-HOT MATMULS on PE: per chunk a [128e,128d] bf16
  one-hot (DVE is_equal; pad slots carry sentinel 255 -> zero rows)
  accumulates messages into per-graph PSUM. No dma_scatter_add.
- GAT's xr[dst] is a [128d,128e]-orientation one-hot matmul (one-hot built
  on the Scalar engine as relu(1-(dlo-d)^2)) reading xr blocks from SBUF.
- Only 2 indirect passes remain (GpSimd Q7 descriptor generation is the
  machine bottleneck): gather u[src] (GCN) and [xl|xr][src] (GAT).
- Fully per-graph pipelined: utab(g+2) build, dense/pool/gtab(g),
  gcn(g+1) and gat(g) interleave so the GpSimd gather stream never idles.
- GCN u rows split [u_hi|u_lo] bf16 (~f32 accuracy, one 128-wide matmul).
- score1 fused into the lin1 matmul via host-folded [W_lin1 | W_lin1@p1n].
- Degrees are host-precomputed index data (bincount of dst); rsqrt on device.
"""

import sys

sys.path.insert(0, "/opt/trn_rl_repo")

from dataclasses import dataclass

import numpy as np
import ml_dtypes

import concourse.bass as bass
import concourse.mybir as mybir
import concourse.tile as tile
from concourse import bacc
from concourse.bass_utils import run_bass_kernel_spmd
from concourse.masks import make_identity

P = 128
F32 = mybir.dt.float32
BF16 = mybir.dt.bfloat16
I16 = mybir.dt.int16
U8 = mybir.dt.uint8
AF = mybir.ActivationFunctionType
OP = mybir.AluOpType
AX = mybir.AxisListType


@dataclass
class Cfg:
    ng: int = 8          # graphs per core
    npg: int = 2048      # nodes per graph
    hid: int = 64        # feature dim
    eg: int = 32768      # edges per graph (original, without self loops)
    spd: int = 19        # chunks (of 128 slots) per dst-block run; set at runtime
    ch: int = 1024       # gather window (slots per dma_gather call; >=2048
                         # overflows the SWDGE descriptor ring and faults)
    n_bisect: int = 30   # bisection iterations for topk threshold
    psign: float = -1.0  # -sign(sum(att)): poison sign so poisoned e < 0
    pb_mag: float = 200.0  # poison magnitude; set so poisoned e ~ -40 (LUT-safe)

    @property
    def nn(self):
        return self.ng * self.npg

    @property
    def ne(self):
        return self.ng * self.eg

    @property
    def tj(self):
        return self.npg // P  # dst blocks per graph (16)

    @property
    def nt(self):
        return self.ng * self.tj  # 128

    @property
    def tch(self):
        return self.tj * self.spd  # chunks per graph

    @property
    def slots_g(self):
        return self.tch * P  # padded edge slots per graph

    @property
    def slots(self):
        return self.ng * self.slots_g

    @property
    def wpg(self):
        assert self.slots_g % self.ch == 0
        return self.slots_g // self.ch  # gather windows per graph

    @property
    def cpw(self):
        return self.ch // P  # chunks per window

    @property
    def k1(self):
        return self.npg // 2

    @property
    def k2(self):
        return self.npg // 4


def build_core_program(ctx, tc, cfg: Cfg):
    from contextlib import ExitStack as _ES0
    nc = tc.nc
    NG, NPG, HID, NN = cfg.ng, cfg.npg, cfg.hid, cfg.nn
    NT, TJ, SPD = cfg.nt, cfg.tj, cfg.spd
    TCH, WPG, CPW, CH = cfg.tch, cfg.wpg, cfg.cpw, cfg.ch
    SRUN = SPD * P  # slots per db run
    assert HID == 64 and CPW % 8 == 0

    # ---- I/O ----
    xT = nc.dram_tensor("xT", [HID, NN], F32, kind="ExternalInput").ap()
    srcw = nc.dram_tensor("srcw", [P, cfg.slots // 16], I16, kind="ExternalInput").ap()
    dlo_pm = nc.dram_tensor("dlo_pm", [P, NG * TCH], BF16, kind="ExternalInput").ap()
    dlo_rep = nc.dram_tensor("dlo_rep", [NG * TJ * P, SRUN], U8, kind="ExternalInput").ap()
    degT = nc.dram_tensor("degT", [P, NT], F32, kind="ExternalInput").ap()
    w_names = ["Wl", "Wr"]
    Wd = {n: nc.dram_tensor(n, [HID, HID], F32, kind="ExternalInput").ap() for n in w_names}
    W1d = nc.dram_tensor("W1dup", [HID, P], F32, kind="ExternalInput").ap()
    W1p = nc.dram_tensor("Wlin1p", [HID, HID + 1], F32, kind="ExternalInput").ap()
    dinvF = nc.dram_tensor("dinvF", [NN], F32, kind="ExternalInput").ap()
    b1c = nc.dram_tensor("b_lin1c", [HID + 1], F32, kind="ExternalInput").ap()
    v_names = ["bn_a", "bn_bp", "att", "b_gat", "p2", "W23"]
    Vd = {n: nc.dram_tensor(n, [HID], F32, kind="ExternalInput").ap() for n in v_names}
    Cd = nc.dram_tensor("Cc", [1], F32, kind="ExternalInput").ap()
    out_d = nc.dram_tensor("out", [NG, 1], F32, kind="ExternalOutput").ap()

    # ---- DRAM scratch (per-graph tables so gathers only depend on their
    # own graph's writes) ----
    utab = [nc.dram_tensor(f"utab{g}", [NPG, P], BF16).ap() for g in range(NG)]
    gtab = [nc.dram_tensor(f"gtab{g}", [NPG, P], BF16).ap() for g in range(NG)]
    xlr_dram = nc.dram_tensor("xlr_dram", [P, NN], BF16).ap()
    ufm_dram = nc.dram_tensor("ufm_dram", [P, NN], BF16).ap()
    sc_dram = nc.dram_tensor("sc_dram", [NN], F32).ap()

    cpool = ctx.enter_context(tc.tile_pool(name="consts", bufs=1))
    mpool = ctx.enter_context(tc.tile_pool(name="main", bufs=1))
    smallps = ctx.enter_context(tc.tile_pool(name="smallps", bufs=1, space="PSUM"))

    # ---- constants ----
    ident = cpool.tile([P, P], F32)
    make_identity(nc, ident[:])
    ones128 = cpool.tile([P, P], F32)
    nc.vector.memset(ones128[:], 1.0)
    nantile = cpool.tile([P, NT], F32)
    nc.vector.memset(nantile[:], float("nan"))
    negbig = cpool.tile([P, NT], F32)
    nc.vector.memset(negbig[:], -1e9)
    io16 = cpool.tile([P, P], I16)
    nc.gpsimd.iota(io16[:], pattern=[[1, P]], base=0, channel_multiplier=0)
    iota_bf = cpool.tile([P, P], BF16)
    nc.vector.tensor_copy(out=iota_bf[:], in_=io16[:])
    ioc16 = cpool.tile([P, 1], I16)
    nc.gpsimd.iota(ioc16[:], pattern=[[0, 1]], base=0, channel_multiplier=1)
    niota_col = cpool.tile([P, 1], F32)
    nc.vector.tensor_scalar(
        out=niota_col[:], in0=ioc16[:], scalar1=-1.0, scalar2=None, op0=OP.mult
    )

    Ws = {}
    for n in w_names:
        t = cpool.tile([HID, HID], F32, tag=f"w_{n}")
        nc.sync.dma_start(out=t[:], in_=Wd[n][:])
        Ws[n] = t
    W1p_sb = cpool.tile([HID, HID + 1], F32, tag="w_Wlin1p")
    nc.sync.dma_start(out=W1p_sb[:], in_=W1p[:])
    W1d_sb = cpool.tile([HID, P], F32, tag="w_W1dup")
    nc.sync.dma_start(out=W1d_sb[:], in_=W1d[:])
    b1c_sb = cpool.tile([HID + 1, 1], F32, tag="v_b1c")
    nc.sync.dma_start(out=b1c_sb[:], in_=b1c[:, None])
    Vs = {}
    for n in v_names:
        t = cpool.tile([HID, 1], F32, tag=f"v_{n}")
        nc.sync.dma_start(out=t[:], in_=Vd[n][:, None])
        Vs[n] = t
    att_rep = cpool.tile([P, HID], BF16)
    nc.gpsimd.dma_start(out=att_rep[:], in_=Vd["att"][None, :].to_broadcast([P, HID]))
    p2_rep = cpool.tile([P, HID], F32)
    nc.sync.dma_start(out=p2_rep[:], in_=Vd["p2"][None, :].to_broadcast([P, HID]))
    bgat_rep = cpool.tile([P, HID], F32)
    nc.sync.dma_start(out=bgat_rep[:], in_=Vd["b_gat"][None, :].to_broadcast([P, HID]))
    Cc_sb = cpool.tile([NG, 1], F32)
    nc.sync.dma_start(out=Cc_sb[:], in_=Cd[None, :].to_broadcast([NG, 1]))

    # whole per-chunk dst-low-bit table (for one-hot builds in [e,d] orientation)
    dlo_sb = cpool.tile([P, NG * TCH], BF16)
    nc.sync.dma_start(out=dlo_sb[:], in_=dlo_pm[:])

    # ---- dinv from host degree counts ----
    dinv_t = mpool.tile([P, NT], F32, tag="dinv_t")
    sqd_t = mpool.tile([P, NT], F32, tag="sqd_t")
    ntmp = mpool.tile([P, NT], F32, tag="ntmp")

    def recip_newton(r_ap, x_ap, tmp_ap):
        nc.vector.tensor_tensor(out=tmp_ap, in0=x_ap, in1=r_ap, op=OP.mult)
        nc.vector.tensor_scalar(
            out=tmp_ap, in0=tmp_ap, scalar1=-1.0, scalar2=2.0, op0=OP.mult, op1=OP.add
        )
        nc.vector.tensor_tensor(out=r_ap, in0=r_ap, in1=tmp_ap, op=OP.mult)

    deg_sb = mpool.tile([P, NT], F32, tag="deg_sb")
    nc.sync.dma_start(out=deg_sb[:], in_=degT[:])
    nc.scalar.sqrt(out=sqd_t[:], in_=deg_sb[:])
    nc.vector.reciprocal(out=dinv_t[:], in_=sqd_t[:])
    recip_newton(dinv_t[:], sqd_t[:], ntmp[:])

    def idx_slice(pool, w_global, tag):
        t = pool.tile([P, CH // 16], I16, tag=tag)
        c0 = w_global * (CH // 16)
        nc.sync.dma_start(out=t[:], in_=srcw[:, c0 : c0 + CH // 16])
        return t

    # ---- pools (all phases interleave; PSUM budget: 2+1+2+2+1 = 8 banks) ----
    gep = ctx.enter_context(tc.tile_pool(name="gep", bufs=6))
    ohp = ctx.enter_context(tc.tile_pool(name="ohp", bufs=3))
    phase_gc = _ES0()
    hps_pool = phase_gc.enter_context(tc.tile_pool(name="hpsp", bufs=1, space="PSUM"))
    dsa = ctx.enter_context(tc.tile_pool(name="dsa", bufs=1))
    dss = ctx.enter_context(tc.tile_pool(name="dss", bufs=2))
    bis = ctx.enter_context(tc.tile_pool(name="bis", bufs=2))
    gap = ctx.enter_context(tc.tile_pool(name="gap", bufs=7))
    aohp = ctx.enter_context(tc.tile_pool(name="aohp", bufs=3))
    runtmp = ctx.enter_context(tc.tile_pool(name="runtmp", bufs=1))
    runp = ctx.enter_context(tc.tile_pool(name="runp", bufs=2))
    gatps = None
    mxrp = None
    gfin = ctx.enter_context(tc.tile_pool(name="gfin", bufs=1))
    phase_u = _ES0()
    ups = phase_u.enter_context(tc.tile_pool(name="ups", bufs=2))
    ubc = phase_u.enter_context(tc.tile_pool(name="ubc", bufs=1))

    # persistent t-space tiles
    h2t = mpool.tile([P, NT, HID], F32, tag="bigA")
    gstage = mpool.tile([P, NT, P], BF16, tag="bigC")
    score1_t = mpool.tile([P, NT], F32, tag="score1_t")
    tanh1 = mpool.tile([P, NT], F32, tag="tanh1")
    gate1 = mpool.tile([P, NT], F32, tag="gate1")
    gate1z = mpool.tile([P, NT], F32, tag="gate1z")
    padd = mpool.tile([P, NT], F32, tag="padd")
    kept1 = mpool.tile([P, NT], U8, tag="kept1")
    pb = cfg.psign * cfg.pb_mag

    hps_tiles = {}
    nmps_tiles = {}

    # ======== per-graph sections ========

    def utab_build(g):
        # xw (duplicated rows) -> dinv scale -> [hi|lo] bf16 split, all in
        # feature-major; node-major utab rows via one transpose DMA hop.
        xw2 = ups.tile([P, NPG], F32, tag="xw2", name=f"xw2_{g}")
        for jl in range(TJ):
            xTg = ups.tile([HID, P], F32, tag="xTg", name=f"xTg{g}_{jl}")
            nc.scalar.dma_start(
                out=xTg[:], in_=xT[:, g * NPG + jl * P : g * NPG + (jl + 1) * P]
            )
            pm = smallps.tile([P, P], F32, tag="smA", name=f"xwps{g}_{jl}")
            nc.tensor.matmul(pm[:], lhsT=W1d_sb[:], rhs=xTg[:], start=True, stop=True)
            nc.scalar.copy(out=xw2[:, jl * P : (jl + 1) * P], in_=pm[:])
        for hh in range(2):
            dbc = ubc.tile([P, NPG // 2], F32, tag="dinvbc", name=f"dinvbc{g}_{hh}")
            o0 = g * NPG + hh * (NPG // 2)
            nc.scalar.dma_start(
                out=dbc[:],
                in_=dinvF[o0 : o0 + NPG // 2][None, :].to_broadcast([P, NPG // 2]),
            )
            nc.vector.tensor_tensor(
                out=xw2[:, hh * (NPG // 2) : (hh + 1) * (NPG // 2)],
                in0=xw2[:, hh * (NPG // 2) : (hh + 1) * (NPG // 2)],
                in1=dbc[:], op=OP.mult,
            )
        u2 = ups.tile([P, NPG], BF16, tag="u2", name=f"u2_{g}")
        nc.vector.tensor_copy(out=u2[0:HID, :], in_=xw2[0:HID, :])
        nc.vector.tensor_copy(out=u2[HID:P, :], in_=xw2[HID:P, :])
        nc.vector.tensor_tensor(
            out=u2[HID:P, :], in0=xw2[HID:P, :], in1=u2[HID:P, :], op=OP.subtract
        )
        nc.scalar.dma_start(out=ufm_dram[:, g * NPG : (g + 1) * NPG], in_=u2[:])
        u_nm = ups.tile([P, TJ, P], BF16, tag="u2", name=f"unm{g}")
        nc.sync.dma_start_transpose(
            out=u_nm[:], in_=ufm_dram[:, g * NPG : (g + 1) * NPG]
        )
        nc.scalar.dma_start(
            out=utab[g].rearrange("(j p) f -> p j f", p=P), in_=u_nm[:]
        )

    def gcn_pass(g):
        par = g % 2
        hps = [
            hps_pool.tile([P, 8, HID], F32, tag=f"hps{par}{t}", name=f"hps{t}_{g}")
            for t in range(2)
        ]
        hps_tiles[g] = hps
        for w in range(WPG):
            wg = g * WPG + w
            ssl = idx_slice(gep, wg, "ssl")
            ub = gep.tile([P, CPW, P], BF16, tag="ub")
            nc.gpsimd.dma_gather(
                out_ap=ub[:], in_ap=utab[g][:], idxs_ap=ssl[:],
                num_idxs=CH, num_idxs_reg=CH, elem_size=P, queue_num=wg % 4,
            )
            for b in range(CPW // 8):
                oh8 = ohp.tile([P, 8, P], BF16, tag="oh8")
                c0 = g * TCH + w * CPW + b * 8
                nc.vector.tensor_tensor(
                    out=oh8[:],
                    in0=iota_bf[:, None, :].to_broadcast([P, 8, P]),
                    in1=dlo_sb[:, c0 : c0 + 8, None].to_broadcast([P, 8, P]),
                    op=OP.is_equal,
                )
                for cl in range(8):
                    c = b * 8 + cl
                    gc = w * CPW + c
                    db, pos = gc // SPD, gc % SPD
                    out_slc = hps[db // 8][:, db % 8, :]
                    nc.tensor.matmul(
                        out_slc, lhsT=oh8[:, cl, :], rhs=ub[:, c, 0:HID],
                        start=(pos == 0), stop=False,
                    )
                    nc.tensor.matmul(
                        out_slc, lhsT=oh8[:, cl, :], rhs=ub[:, c, HID:P],
                        start=False, stop=(pos == SPD - 1),
                    )

    S6C = 512

    def dense_g(g):
        # h = dinv*(hi+lo); BN+leaky (one ACT Lrelu); lin1(+score); xl/xr
        hps = hps_tiles.pop(g)
        gsl = slice(g * TJ, (g + 1) * TJ)
        hsum = dsa.tile([P, TJ, HID], F32, tag="hsum", name=f"hsum{g}")
        for t in range(2):
            nc.vector.tensor_tensor(
                out=hsum[:, t * 8 : (t + 1) * 8, :], in0=hps[t][:],
                in1=dinv_t[:, g * TJ + t * 8 : g * TJ + (t + 1) * 8, None].to_broadcast(
                    [P, 8, HID]
                ),
                op=OP.mult,
            )
        hfm = dsa.tile([HID, NPG], F32, tag="hfm", name=f"hfm{g}")
        for jl in range(TJ):
            pt = smallps.tile([HID, P], F32, tag=("smA" if jl % 2 == 0 else "smB"), name=f"htr{g}_{jl}")
            nc.tensor.transpose(out=pt[:], in_=hsum[:, jl, :], identity=ident[:])
            nc.scalar.copy(out=hfm[:, jl * P : (jl + 1) * P], in_=pt[:])
        nc.scalar.activation(
            out=hfm[:], in_=hfm[:], func=AF.Lrelu, scale=Vs["bn_a"][:],
            bias=Vs["bn_bp"][:], alpha=0.01,
        )
        xlr_g = dss.tile([P, NPG], BF16, tag="xlrg", name=f"xlrg{g}")
        for ol in range(0, NPG, S6C):
            o = g * NPG + ol
            pm = hps_pool.tile([HID + 1, S6C], F32, tag=f"hps{g % 2}0", name=f"l1ps{o}")
            nc.tensor.matmul(
                pm[:], lhsT=W1p_sb[:], rhs=hfm[:, ol : ol + S6C],
                start=True, stop=True,
            )
            hc = dss.tile([HID + 1, S6C], F32, tag="hc", name=f"hc{o}")
            nc.scalar.activation(
                out=hc[:], in_=pm[:], func=AF.Identity, bias=b1c_sb[:]
            )
            nc.sync.dma_start(out=sc_dram[None, o : o + S6C], in_=hc[HID : HID + 1, :])
            px = hps_pool.tile([P, S6C], F32, tag=f"hps{g % 2}1", name=f"xlrps{o}")
            nc.tensor.matmul(px[:HID, :], lhsT=Ws["Wl"][:], rhs=hc[0:HID, :], start=True, stop=True)
            nc.tensor.matmul(px[HID:, :], lhsT=Ws["Wr"][:], rhs=hc[0:HID, :], start=True, stop=True)
            nc.scalar.copy(out=xlr_g[:, ol : ol + S6C], in_=px[:])
        nc.scalar.dma_start(out=xlr_dram[:, g * NPG : (g + 1) * NPG], in_=xlr_g[:])

    def bisect_multi(score_slc, ngr, target, tag):
        # score_slc: [P, ngr*TJ]; returns per-graph thresholds lo [P, ngr]
        lo = bis.tile([P, ngr], F32, tag="lo", name=f"lo_{tag}")
        hi = bis.tile([P, ngr], F32, tag="hi", name=f"hi_{tag}")
        mid = bis.tile([P, ngr], F32, tag="mid", name=f"mid_{tag}")
        cmp = bis.tile([P, ngr * TJ], F32, tag="cmp", name=f"cmp_{tag}")
        cred = bis.tile([P, ngr], F32, tag="cred", name=f"cred_{tag}")
        ge = bis.tile([P, ngr], U8, tag="ge", name=f"ge_{tag}")
        lt = bis.tile([P, ngr], U8, tag="lt", name=f"lt_{tag}")
        nc.vector.memset(lo[:], -64.0)
        nc.vector.memset(hi[:], 64.0)
        sc_g = score_slc.rearrange("p (g t) -> p g t", g=ngr)
        cmp_g = cmp[:].rearrange("p (g t) -> p g t", g=ngr)
        for it in range(cfg.n_bisect):
            nc.vector.tensor_tensor(out=mid[:], in0=lo[:], in1=hi[:], op=OP.add)
            nc.vector.tensor_scalar(
                out=mid[:], in0=mid[:], scalar1=0.5, scalar2=None, op0=OP.mult
            )
            nc.vector.tensor_tensor(
                out=cmp_g, in0=sc_g,
                in1=mid[:, :, None].to_broadcast([P, ngr, TJ]), op=OP.is_gt,
            )
            nc.vector.tensor_reduce(out=cred[:], in_=cmp_g, axis=AX.X, op=OP.add)
            cps = smallps.tile([P, ngr], F32, tag="smB", name=f"cnt_{tag}_{it}")
            nc.tensor.matmul(cps[:], lhsT=ones128[:], rhs=cred[:], start=True, stop=True)
            nc.vector.tensor_scalar(
                out=ge[:], in0=cps[:], scalar1=float(target), scalar2=None, op0=OP.is_ge
            )
            nc.vector.tensor_scalar(
                out=lt[:], in0=cps[:], scalar1=float(target), scalar2=None, op0=OP.is_lt
            )
            nc.vector.copy_predicated(out=lo[:], mask=ge[:], data=mid[:])
            nc.vector.copy_predicated(out=hi[:], mask=lt[:], data=mid[:])
        return lo

    NH = NG // 2  # graphs per pooling half

    def pool1_h(h):
        g0 = h * NH
        gsl = slice(g0 * TJ, (g0 + NH) * TJ)
        nc.sync.dma_start(
            out=score1_t[:, gsl],
            in_=sc_dram[g0 * NPG : (g0 + NH) * NPG].rearrange("(j p) -> p j", p=P),
        )
        t1 = bisect_multi(score1_t[:, gsl], NH, cfg.k1, f"p1h{h}")
        nc.vector.tensor_tensor(
            out=kept1[:, gsl].rearrange("p (g t) -> p g t", g=NH),
            in0=score1_t[:, gsl].rearrange("p (g t) -> p g t", g=NH),
            in1=t1[:, :, None].to_broadcast([P, NH, TJ]), op=OP.is_gt,
        )
        nc.scalar.activation(out=tanh1[:, gsl], in_=score1_t[:, gsl], func=AF.Tanh)
        nc.vector.tensor_copy(out=gate1[:, gsl], in_=nantile[:, gsl])
        nc.vector.copy_predicated(out=gate1[:, gsl], mask=kept1[:, gsl], data=tanh1[:, gsl])
        nc.vector.memset(gate1z[:, gsl], 0.0)
        nc.vector.copy_predicated(out=gate1z[:, gsl], mask=kept1[:, gsl], data=tanh1[:, gsl])
        nc.vector.tensor_scalar(
            out=padd[:, gsl], in0=kept1[:, gsl], scalar1=-pb, scalar2=pb,
            op0=OP.mult, op1=OP.add,
        )
        # gtab for the half
        gs = gstage[:, gsl, :]
        nc.sync.dma_start_transpose(
            out=gs, in_=xlr_dram[:, g0 * NPG : (g0 + NH) * NPG]
        )
        nc.vector.tensor_tensor(
            out=gs, in0=gs,
            in1=gate1z[:, gsl, None].to_broadcast([P, NH * TJ, P]), op=OP.mult,
        )
        nc.vector.tensor_tensor(
            out=gs, in0=gs,
            in1=padd[:, gsl, None].to_broadcast([P, NH * TJ, P]), op=OP.add,
        )
        for gg in range(g0, g0 + NH):
            nc.sync.dma_start(
                out=gtab[gg].rearrange("(j p) f -> p j f", p=P),
                in_=gstage[:, gg * TJ : (gg + 1) * TJ, :],
            )

    gat_pools = {}

    def gat_pass(g):
        if "gatps" not in gat_pools:
            gat_pools["gatps"] = ctx.enter_context(tc.tile_pool(name="gatps", bufs=1, space="PSUM"))
            gat_pools["mxrp"] = ctx.enter_context(tc.tile_pool(name="mxrp", bufs=2, space="PSUM"))
        gatps, mxrp = gat_pools["gatps"], gat_pools["mxrp"]
        nmps = [
            gatps.tile([P, 8, HID], F32, tag=f"nmps{t}", name=f"nmps{t}_{g}")
            for t in range(2)
        ]
        dnps = smallps.tile([P, TJ], F32, tag="smB", name=f"dnps{g}")
        oh_de = {}
        for w in range(WPG):
            wg = g * WPG + w
            ssl = idx_slice(gap, wg, "assl")
            gx = gap.tile([P, CPW, P], BF16, tag="gx")
            nc.gpsimd.dma_gather(
                out_ap=gx[:], in_ap=gtab[g][:], idxs_ap=ssl[:],
                num_idxs=CH, num_idxs_reg=CH, elem_size=P, queue_num=wg % 4,
            )
            for b in range(CPW // 8):
                oh8 = aohp.tile([P, 8, P], BF16, tag="aoh8")
                c0 = g * TCH + w * CPW + b * 8
                nc.vector.tensor_tensor(
                    out=oh8[:],
                    in0=iota_bf[:, None, :].to_broadcast([P, 8, P]),
                    in1=dlo_sb[:, c0 : c0 + 8, None].to_broadcast([P, 8, P]),
                    op=OP.is_equal,
                )
                mxr = mxrp.tile([P, 8, HID], F32, tag="mxr", name=f"mxr{wg}_{b}")
                for cl in range(8):
                    c = b * 8 + cl
                    gc = w * CPW + c
                    r, pos = gc // SPD, gc % SPD
                    if pos == 0:
                        dlo_bc = runtmp.tile(
                            [P, SRUN], U8, tag="dlobc", name=f"dlobc{g}_{r}"
                        )
                        o = (g * TJ + r) * P
                        nc.sync.dma_start(
                            out=dlo_bc[:], in_=dlo_rep[o : o + P, :]
                        )
                        # one-hot on the Scalar engine: relu(1 - (dlo - d)^2)
                        ohsq = runtmp.tile([P, SRUN], BF16, tag="ohsq", name=f"ohsq{g}_{r}")
                        nc.scalar.activation(
                            out=ohsq[:], in_=dlo_bc[:], func=AF.Square,
                            bias=niota_col[:],
                        )
                        ohr = runp.tile([P, SRUN], BF16, tag="ohde", name=f"ohde{g}_{r}")
                        nc.scalar.activation(
                            out=ohr[:], in_=ohsq[:], func=AF.Relu,
                            bias=1.0, scale=-1.0,
                        )
                        oh_de[r] = ohr
                    nc.tensor.matmul(
                        mxr[:, cl, :],
                        lhsT=oh_de[r][:, pos * P : (pos + 1) * P],
                        rhs=gstage[:, g * TJ + r, HID:P],
                        start=True, stop=True,
                    )
                # e = att . leaky(xl_s + xr_d); w = exp(e); pay = w*xl
                gxs = gx[:, b * 8 : (b + 1) * 8, :]
                z = gap.tile([P, 8, HID], BF16, tag="z")
                nc.vector.tensor_tensor(
                    out=z[:], in0=gxs[:, :, 0:HID], in1=mxr[:], op=OP.add
                )
                nc.vector.scalar_tensor_tensor(
                    out=z[:], in0=z[:], scalar=0.2, in1=z[:], op0=OP.mult, op1=OP.max,
                )
                nc.vector.tensor_tensor(
                    out=z[:], in0=z[:],
                    in1=att_rep[:, None, :].to_broadcast([P, 8, HID]), op=OP.mult,
                )
                e8 = gap.tile([P, 8], F32, tag="e8")
                nc.vector.tensor_reduce(out=e8[:], in_=z[:], axis=AX.X, op=OP.add)
                w8b = gap.tile([P, 8], BF16, tag="w8b")
                nc.scalar.activation(out=w8b[:], in_=e8[:], func=AF.Exp)
                pay = gap.tile([P, 8, HID], BF16, tag="pay")
                nc.vector.tensor_tensor(
                    out=pay[:], in0=gxs[:, :, 0:HID],
                    in1=w8b[:, :, None].to_broadcast([P, 8, HID]), op=OP.mult,
                )
                for cl in range(8):
                    c = b * 8 + cl
                    gc = w * CPW + c
                    db, pos = gc // SPD, gc % SPD
                    nc.tensor.matmul(
                        nmps[db // 8][:, db % 8, :],
                        lhsT=oh8[:, cl, :], rhs=pay[:, cl, :],
                        start=(pos == 0), stop=(pos == SPD - 1),
                    )
                    nc.tensor.matmul(
                        dnps[:, db : db + 1],
                        lhsT=oh8[:, cl, :], rhs=w8b[:, cl : cl + 1],
                        start=(pos == 0), stop=(pos == SPD - 1),
                    )

        # ---- finalize graph g: h2 = leaky(numer/denom + b_gat) ----
        numsb = dsa.tile([P, TJ, HID], F32, tag="hsum", name=f"numsb{g}")
        nc.scalar.copy(out=numsb[:, 0:8, :], in_=nmps[0][:])
        nc.scalar.copy(out=numsb[:, 8:TJ, :], in_=nmps[1][:])
        den = gfin.tile([P, TJ], F32, tag="den", name=f"den{g}")
        rec = gfin.tile([P, TJ], F32, tag="rec", name=f"rec{g}")
        dtmp = gfin.tile([P, TJ], F32, tag="dtmp", name=f"dtmp{g}")
        nc.vector.tensor_scalar(
            out=den[:], in0=dnps[:], scalar1=1e-16, scalar2=None, op0=OP.add
        )
        nc.vector.reciprocal(out=rec[:], in_=den[:])
        recip_newton(rec[:], den[:], dtmp[:])
        hslc = h2t[:, g * TJ : (g + 1) * TJ, :]
        nc.vector.tensor_tensor(
            out=hslc, in0=numsb[:],
            in1=rec[:, :, None].to_broadcast([P, TJ, HID]), op=OP.mult,
        )
        nc.vector.tensor_tensor(
            out=hslc, in0=hslc,
            in1=bgat_rep[:, None, :].to_broadcast([P, TJ, HID]), op=OP.add,
        )
        nc.vector.scalar_tensor_tensor(
            out=hslc, in0=hslc, scalar=0.01, in1=hslc, op0=OP.mult, op1=OP.max
        )

    # ======== emission: utab phase, gcn stream, pools, gat stream ========
    utab_build(0)
    utab_build(1)
    for g in range(NG):
        gcn_pass(g)
        if g + 2 < NG:
            utab_build(g + 2)
        dense_g(g)
        if g + 2 == NG:
            phase_u.close()
    pool1_h(0)
    pool1_h(1)
    phase_gc.close()
    for g in range(NG):
        gat_pass(g)

    # ======== score2 (t-space, blocked), mask to kept1 ========
    score2_t = mpool.tile([P, NT], F32, tag="score2_t")
    for t in range(4):
        tsl = slice(t * 32, (t + 1) * 32)
        blk = dsa.tile([P, 32, HID], F32, tag="hfm", name=f"s2blk{t}")
        nc.vector.tensor_tensor(
            out=blk[:], in0=h2t[:, tsl, :],
            in1=p2_rep[:, None, :].to_broadcast([P, 32, HID]), op=OP.mult,
        )
        nc.vector.tensor_reduce(out=score2_t[:, tsl], in_=blk[:], axis=AX.X, op=OP.add)
    kept1_t = mpool.tile([P, NT], U8, tag="kept1_t")
    nc.vector.tensor_tensor(out=kept1_t[:], in0=gate1[:], in1=gate1[:], op=OP.is_equal)
    sc2m = mpool.tile([P, NT], F32, tag="sc2m")
    nc.vector.tensor_copy(out=sc2m[:], in_=negbig[:])
    nc.vector.copy_predicated(out=sc2m[:], mask=kept1_t[:], data=score2_t[:])

    # ======== pool2 threshold + gate2 = tanh * mask ========
    gate2 = mpool.tile([P, NT], F32, tag="gate2")
    t2 = bisect_multi(sc2m[:], NG, cfg.k2, "p2")
    nc.vector.tensor_tensor(
        out=gate2[:].rearrange("p (g t) -> p g t", g=NG),
        in0=sc2m[:].rearrange("p (g t) -> p g t", g=NG),
        in1=t2[:, :, None].to_broadcast([P, NG, TJ]), op=OP.is_gt,
    )
    tanh2 = mpool.tile([P, NT], F32, tag="tanh2")
    sc2c = mpool.tile([P, NT], F32, tag="sc2c")
    nc.vector.tensor_scalar(
        out=sc2c[:], in0=sc2m[:], scalar1=-64.0, scalar2=None, op0=OP.max
    )
    nc.scalar.activation(out=tanh2[:], in_=sc2c[:], func=AF.Tanh)
    nc.vector.tensor_tensor(out=gate2[:], in0=gate2[:], in1=tanh2[:], op=OP.mult)

    # ======== T_g = sum_n gate2[n] * h2[n]; out = T @ W23 + C ========
    Tps = smallps.tile([P, NG], F32, tag="smB")
    for j in range(NT):
        g = j // TJ
        nc.tensor.matmul(
            Tps[:HID, g : g + 1], lhsT=h2t[:, j, :], rhs=gate2[:, j : j + 1],
            start=(j % TJ == 0), stop=(j % TJ == TJ - 1),
        )
    Tsb = mpool.tile([HID, NG], F32, tag="Tsb")
    nc.scalar.copy(out=Tsb[:], in_=Tps[:HID, :])
    hps2 = smallps.tile([NG, 1], F32, tag="smB")
    nc.tensor.matmul(hps2[:], lhsT=Tsb[:], rhs=Vs["W23"][:], start=True, stop=True)
    outsb = mpool.tile([NG, 1], F32, tag="outsb")
    nc.vector.tensor_tensor(out=outsb[:], in0=hps2[:], in1=Cc_sb[:], op=OP.add)
    nc.sync.dma_start(out=out_d[:], in_=outsb[:])


# ================= host side =================

def _wrap_idx(ix: np.ndarray) -> np.ndarray:
    n = ix.shape[0]
    w = ix.reshape(n // 16, 16).T.astype(np.int16)
    return np.tile(w, (8, 1)).copy()


def _prep_weights(cfg, W1, b1, bn_gamma, bn_beta, bn_mean, bn_var, W_lin1, b_lin1,
                  p1, Wl, Wr, att, b_gat, p2, W_lin2, b_lin2, W_lin3, b_lin3):
    f32 = np.float32
    bn_a = (bn_gamma / np.sqrt(bn_var + 1e-5)).astype(f32)
    bn_b = (bn_beta - bn_mean * bn_a).astype(f32)
    W23 = (W_lin2 @ W_lin3).reshape(-1).astype(f32)
    Cc = np.array([cfg.k2 * float(b_lin2 @ W_lin3[:, 0]) + float(b_lin3[0])], dtype=f32)
    p1n = (np.asarray(p1) / np.linalg.norm(np.asarray(p1))).astype(np.float64)
    Wlin1p = np.concatenate(
        [np.asarray(W_lin1, np.float64),
         (np.asarray(W_lin1, np.float64) @ p1n)[:, None]], axis=1
    ).astype(f32)
    c1 = np.array([float(p1n @ np.asarray(b_lin1, np.float64))], dtype=f32)
    return {
        "W1dup": np.ascontiguousarray(
            np.concatenate([np.asarray(W1, f32)] * 2, axis=1)
        ),
        "Wlin1p": Wlin1p,
        "Wl": np.ascontiguousarray(Wl, f32), "Wr": np.ascontiguousarray(Wr, f32),
        "bn_a": bn_a, "bn_bp": (np.asarray(b1, f32) * bn_a + bn_b).astype(f32),
        "b_lin1c": np.concatenate([np.asarray(b_lin1, f32), c1]),
        "att": np.ascontiguousarray(att, f32), "b_gat": np.ascontiguousarray(b_gat, f32),
        "p2": (np.asarray(p2) / np.linalg.norm(np.asarray(p2))).astype(f32),
        "W23": W23, "Cc": Cc,
    }


def _prep_core_edges(cfg: Cfg, src_core, dst_core):
    """src/dst core-local [ne]. Per graph: append self loops, bucket edges by
    dst block (db = dst>>7), pad each db run to spd*128 slots. Pad slots get
    src=0 (any valid row; killed by the one-hot) and dlo=255 (matches no
    iota value -> all-zero one-hot row/column)."""
    SPD, SRUN = cfg.spd, cfg.spd * P
    loops = np.arange(cfg.npg, dtype=np.int64)
    src_slots = np.zeros((cfg.ng, cfg.tj, SRUN), np.int64)
    dlo_slots = np.full((cfg.ng, cfg.tj, SRUN), 255, np.int64)
    deg = np.zeros((cfg.ng, cfg.npg), np.int64)
    for g in range(cfg.ng):
        e = slice(g * cfg.eg, (g + 1) * cfg.eg)
        s = np.concatenate([src_core[e] - g * cfg.npg, loops])
        d = np.concatenate([dst_core[e] - g * cfg.npg, loops])
        deg[g] = np.bincount(d, minlength=cfg.npg)
        db = d >> 7
        for b in range(cfg.tj):
            m = db == b
            cnt = int(m.sum())
            assert cnt <= SRUN, f"db run overflow: {cnt} > {SRUN}"
            src_slots[g, b, :cnt] = s[m]
            dlo_slots[g, b, :cnt] = d[m] & 127
    stream_src = src_slots.reshape(-1)
    stream_dlo = dlo_slots.reshape(-1)
    deg_t = np.ascontiguousarray(
        deg.reshape(cfg.ng, cfg.tj, P).transpose(2, 0, 1).reshape(P, cfg.nt)
    ).astype(np.float32)
    bf16 = ml_dtypes.bfloat16
    dinv = (1.0 / np.sqrt(np.maximum(deg.reshape(-1), 1.0))).astype(np.float32)
    return {
        "srcw": _wrap_idx(stream_src),
        "dinvF": dinv,
        "dlo_pm": np.ascontiguousarray(
            stream_dlo.reshape(-1, P).T.astype(bf16)
        ),
        "dlo_rep": np.ascontiguousarray(
            np.repeat(
                dlo_slots.reshape(cfg.ng * cfg.tj, 1, SRUN), P, axis=1
            ).astype(np.uint8)
        ),
        "degT": deg_t,
    }


def build_bass(cfg: Cfg):
    from contextlib import ExitStack
    nc = bacc.Bacc("TRN2", target_bir_lowering=False, debug=False,
                   num_swdge_queues=4)
    with tile.TileContext(nc) as tc:
        with ExitStack() as ctx:
            build_core_program(ctx, tc, cfg)
    nc.compile()
    return nc


_CFG = Cfg()
_NC_CACHE = {}
TRACE = False
LAST_RESULT = None


def kernel(x, edge_index, batch, W1, b1, bn_gamma, bn_beta, bn_mean, bn_var,
           W_lin1, b_lin1, p1, Wl, Wr, att, b_gat, p2,
           W_lin2, b_lin2, W_lin3, b_lin3):
    cfg = _CFG
    n_cores = 8
    s_att = float(np.sum(np.asarray(att, dtype=np.float64)))
    assert abs(s_att) > 1e-6, "degenerate att sum; poison scheme needs |sum(att)|>0"
    cfg.psign = -1.0 if s_att > 0 else 1.0
    slope = 0.2 if s_att > 0 else 1.0
    cfg.pb_mag = 40.0 / (slope * abs(s_att))
    weights = _prep_weights(cfg, W1, b1, bn_gamma, bn_beta, bn_mean, bn_var,
                            W_lin1, b_lin1, p1, Wl, Wr, att, b_gat, p2,
                            W_lin2, b_lin2, W_lin3, b_lin3)
    src_all = np.asarray(edge_index[0], dtype=np.int64)
    dst_all = np.asarray(edge_index[1], dtype=np.int64)
    x = np.asarray(x, dtype=np.float32)

    # choose the chunks-per-db-run capacity from the data (global max so the
    # single SPMD program fits every core)
    max_run = 0
    for c in range(n_cores):
        for g in range(cfg.ng):
            e0 = c * cfg.ne + g * cfg.eg
            d = dst_all[e0 : e0 + cfg.eg] - (c * cfg.nn + g * cfg.npg)
            cnts = np.bincount(d >> 7, minlength=cfg.tj) + P  # + self loops
            max_run = max(max_run, int(cnts.max()))
    cfg.spd = (max_run + P - 1) // P
    # windows of ch slots must tile a graph's slot range exactly
    while (cfg.tj * cfg.spd * P) % cfg.ch != 0:
        cfg.spd += 1

    in_maps = []
    for c in range(n_cores):
        n0 = c * cfg.nn
        e0 = c * cfg.ne
        d = dict(weights)
        d.update(
            _prep_core_edges(
                cfg, src_all[e0 : e0 + cfg.ne] - n0, dst_all[e0 : e0 + cfg.ne] - n0
            )
        )
        d["xT"] = np.ascontiguousarray(x[n0 : n0 + cfg.nn].T, np.float32)
        in_maps.append(d)

    key = ("nc", cfg.spd, cfg.psign, cfg.pb_mag)
    if key not in _NC_CACHE:
        _NC_CACHE[key] = build_bass(cfg)
    nc = _NC_CACHE[key]
    global LAST_RESULT
    res = run_bass_kernel_spmd(nc, in_maps, core_ids=list(range(n_cores)), trace=TRACE)
    LAST_RESULT = res
    outs = [np.asarray(res.results[c]["out"]).reshape(cfg.ng, 1) for c in range(n_cores)]
    return np.concatenate(outs, axis=0).astype(np.float32)
